# revision 1
# baseline (speedup 1.0000x reference)
"""Trainium2 Bass kernel for fused LoRA-attention block (nn_Attention_18846316494887).

Reference computation:
  qkv = y @ Wqkv.T + bqkv (+ LoRA deltas y @ (B@A) per Q/K/V)  -> Q,K,V [B,H,S,D]
  attn = softmax(Q K^T / sqrt(D)); o = attn @ V -> [B,S,E]
  msa = o @ Wmsa.T + o @ (Bo@Ao); res = msa + y; out = LayerNorm(res)*gamma + beta

Sharding: tensor-parallel over heads (2 heads/core, 8 cores), AllToAll to
reshard head-dim -> token-dim before the output projection, token-parallel
msa + LayerNorm, host-side gather of per-core token shards.

Host-side prep (exact algebra, no approximation):
  - LoRA folded into Wqkv / Wmsa (y@W.T + y@(B@A) == y@(W.T + B@A))
  - 1/sqrt(D) folded into Wq, bq
  - V bias applied post-softmax on o (exact since attn rows sum to 1)
  - y pre-transposed to [E, T] for the QKV matmuls
"""
import functools
import numpy as np
import ml_dtypes

import concourse.mybir as mybir
import concourse.tile as tile
from concourse import bacc
from concourse import bass_utils
from concourse.bass import _add_dep_helper

# problem shapes (hardcoded per harness contract)
E = 1024
H = 16
D = 64
B = 2
S = 2048
T = B * S          # 4096 tokens
N_CORES = 8
EPS = 1e-6

BF16 = mybir.dt.bfloat16
F32 = mybir.dt.float32
AF = mybir.ActivationFunctionType
ALU = mybir.AluOpType

# per-core worksizes
TOK = T // N_CORES          # 512 tokens per core for msa/LN
QC = 512                    # attention q-chunk
N_QC = S // QC              # 4 q-chunks per (b, head-pair)
N_KT = S // 128             # 16 k-tiles
VW = 72                     # padded V row (64 d + 1 ones + pad)
import os as _os
USE_PARTITION_BCAST = _os.environ.get("USE_PB", "1") == "1"


def _build(use_gamma: bool, use_beta: bool):
    nc = bacc.Bacc("TRN2", target_bir_lowering=False, debug=False, num_devices=N_CORES)

    # ---- DRAM parameters -------------------------------------------------
    yT = nc.dram_tensor("yT", [E, T], BF16, kind="ExternalInput")
    wqT = nc.dram_tensor("wqT", [E, 128], BF16, kind="ExternalInput")
    wkT = nc.dram_tensor("wkT", [E, 128], BF16, kind="ExternalInput")
    wvT = nc.dram_tensor("wvT", [E, 128], BF16, kind="ExternalInput")
    bq = nc.dram_tensor("bq", [128, 1], F32, kind="ExternalInput")
    bk = nc.dram_tensor("bk", [128, 1], F32, kind="ExternalInput")
    bva = nc.dram_tensor("bva", [64, 1], F32, kind="ExternalInput")
    bvb = nc.dram_tensor("bvb", [64, 1], F32, kind="ExternalInput")
    msa_w = nc.dram_tensor("msa_w", [E, E], BF16, kind="ExternalInput")
    y_shard = nc.dram_tensor("y_shard", [TOK, E], F32, kind="ExternalInput")
    if use_gamma:
        gamma_b = nc.dram_tensor("gamma_b", [128, E], F32, kind="ExternalInput")
    if use_beta:
        beta_b = nc.dram_tensor("beta_b", [128, E], F32, kind="ExternalInput")
    out = nc.dram_tensor("out", [TOK, E], F32, kind="ExternalOutput")

    # internal DRAM: A2A bounce buffers (shard k: (b, q-half) -> 128 tok/core)
    # plus denominators bounce for partition-broadcast
    a2a_in = [nc.dram_tensor(f"a2a_in{k}", [N_CORES, 128, 128], BF16) for k in range(4)]
    a2a_out = [nc.dram_tensor(f"a2a_out{k}", [N_CORES, 128, 128], BF16) for k in range(4)]
    if not USE_PARTITION_BCAST:
        den_dram = nc.dram_tensor("den_dram", [B, N_QC, 2, QC], F32)

    with tile.TileContext(nc) as tc:
        with (
            tc.tile_pool(name="const", bufs=1) as cpool,
            tc.tile_pool(name="yt", bufs=5) as ytp,
            tc.tile_pool(name="qk", bufs=1) as qkp,
            tc.tile_pool(name="exp", bufs=6) as expp,
            tc.tile_pool(name="stage", bufs=1) as stp,
            tc.tile_pool(name="fin", bufs=2) as finp,
            tc.tile_pool(name="a2asb", bufs=4) as a2ap,
            tc.tile_pool(name="ps_acc", bufs=2, space="PSUM") as ps_acc,
            tc.tile_pool(name="ps_sc", bufs=2, space="PSUM") as ps_sc,
            tc.tile_pool(name="ps_av", bufs=2, space="PSUM") as ps_av,
        ):
            # ---- constants -------------------------------------------------
            wqT_sb = cpool.tile([128, 8, 128], BF16)
            wkT_sb = cpool.tile([128, 8, 128], BF16)
            wvT_sb = cpool.tile([128, 8, 128], BF16)
            nc.sync.dma_start(wqT_sb[:], wqT[:, :].rearrange("(a p) n -> p a n", p=128))
            nc.sync.dma_start(wkT_sb[:], wkT[:, :].rearrange("(a p) n -> p a n", p=128))
            nc.sync.dma_start(wvT_sb[:], wvT[:, :].rearrange("(a p) n -> p a n", p=128))
            bq_sb = cpool.tile([128, 1], F32)
            bk_sb = cpool.tile([128, 1], F32)
            bva_sb = cpool.tile([64, 1], F32)
            bvb_sb = cpool.tile([64, 1], F32)
            nc.sync.dma_start(bq_sb[:], bq[:, :])
            nc.sync.dma_start(bk_sb[:], bk[:, :])
            nc.sync.dma_start(bva_sb[:], bva[:, :])
            nc.sync.dma_start(bvb_sb[:], bvb[:, :])
            # (msa weights / LN consts are DMA'd later, after the attention
            # loops are issued, so startup DMA bandwidth goes to yT tiles)
            msa_w_sb = cpool.tile([128, 8, E], BF16)
            y_shard_sb = cpool.tile([128, 4, E], F32)
            if use_gamma:
                gamma_sb = cpool.tile([128, E], F32)
            if use_beta:
                beta_sb = cpool.tile([128, E], F32)

            # V tiles, padded: [k-part, b, head, ktile, VW]; col 64 = ones
            v_sb = cpool.tile([128, B, 2, N_KT, VW], BF16)
            nc.vector.memset(v_sb[:, :, :, :, 64:VW], 0.0)
            nc.vector.memset(v_sb[:, :, :, :, 64:65], 1.0)

            # Q^T/K^T: [d-part(2 heads), b, q]
            qT_sb = qkp.tile([128, B, S], BF16)
            kT_sb = qkp.tile([128, B, S], BF16)
            # o^T staging for A2A: [d-part, b, q]
            stage = stp.tile([128, B, S], BF16)

            # ============== main per-batch pipeline ==============
            def make_qkv_steps(b):
                """QKV projection for batch b as a list of small closures so the
                PE work can be interleaved into the other batch's attention
                (fills the in-order PE stream's exp-wait slots)."""
                loads, qs, ks, vs = [], [], [], []
                for tc8 in range(4):
                    st8 = {}

                    def load(b=b, tc8=tc8, st8=st8):
                        yt = ytp.tile([128, 8, 512], BF16, tag="yt")
                        st8["yt"] = yt
                        for et in range(8):
                            nc.sync.dma_start(
                                yt[:, et, :], yT[128 * et:128 * (et + 1),
                                                 b * S + 512 * tc8: b * S + 512 * (tc8 + 1)])
                    loads.append(load)
                    qs.append([])
                    ks.append([])
                    vs.append([])

                    for eg in range(4):
                        def qstep(b=b, tc8=tc8, eg=eg, st8=st8):
                            if eg == 0:
                                st8["ps_q"] = ps_acc.tile([128, 512], F32, tag="acc", name="ps_q")
                            ps_q, yt = st8["ps_q"], st8["yt"]
                            for et in (2 * eg, 2 * eg + 1):
                                st, sp = (et == 0), (et == 7)
                                nc.tensor.matmul(ps_q[0:64, :], wqT_sb[:, et, 0:64],
                                                 yt[:, et, :], start=st, stop=sp,
                                                 tile_position=(0, 0))
                                nc.tensor.matmul(ps_q[64:128, :], wqT_sb[:, et, 64:128],
                                                 yt[:, et, :], start=st, stop=sp,
                                                 tile_position=(0, 64))
                            if eg == 3:
                                nc.vector.tensor_scalar(
                                    qT_sb[:, b, 512 * tc8:512 * (tc8 + 1)], ps_q[:],
                                    bq_sb[:], None, ALU.add)
                        qs[tc8].append(qstep)

                    for eg in range(4):
                        def kstep(b=b, tc8=tc8, eg=eg, st8=st8):
                            if eg == 0:
                                st8["ps_k"] = ps_acc.tile([128, 512], F32, tag="acc", name="ps_k")
                            ps_k, yt = st8["ps_k"], st8["yt"]
                            for et in (2 * eg, 2 * eg + 1):
                                st, sp = (et == 0), (et == 7)
                                nc.tensor.matmul(ps_k[0:64, :], wkT_sb[:, et, 0:64],
                                                 yt[:, et, :], start=st, stop=sp,
                                                 tile_position=(0, 0))
                                nc.tensor.matmul(ps_k[64:128, :], wkT_sb[:, et, 64:128],
                                                 yt[:, et, :], start=st, stop=sp,
                                                 tile_position=(0, 64))
                            if eg == 3:
                                nc.vector.tensor_scalar(
                                    kT_sb[:, b, 512 * tc8:512 * (tc8 + 1)], ps_k[:],
                                    bk_sb[:], None, ALU.add)
                        ks[tc8].append(kstep)

                    for eg in range(4):
                        def vstep(b=b, tc8=tc8, eg=eg, st8=st8):
                            if eg == 0:
                                st8["ps_v"] = ps_acc.tile([128, 512], F32, tag="acc", name="ps_v")
                            ps_v, yt = st8["ps_v"], st8["yt"]
                            for et in (2 * eg, 2 * eg + 1):
                                st, sp = (et == 0), (et == 7)
                                for s4 in range(4):
                                    nc.tensor.matmul(ps_v[:, 128 * s4:128 * (s4 + 1)],
                                                     yt[:, et, 128 * s4:128 * (s4 + 1)],
                                                     wvT_sb[:, et, :], start=st, stop=sp)
                            if eg == 3:
                                for h in range(2):
                                    src = ps_v[:, :].rearrange(
                                        "p (s n) -> p s n", s=4)[:, :, 64 * h:64 * (h + 1)]
                                    nc.vector.tensor_copy(
                                        v_sb[:, b, h, 4 * tc8:4 * (tc8 + 1), 0:64], src)
                        vs[tc8].append(vstep)
                return loads, qs, ks, vs

            def attention(b, bg, av_last=None, qcs=range(N_QC), pops=2):
                # software-pipelined ACROSS q-chunks: qk/exp runs OV steps
                # ahead of av, so the ACT engine never drains at qc
                # boundaries; bg closures (other QKV work) fill PE wait slots
                if av_last is None:
                    av_last = []
                OV = 3
                states = {}

                def qk_exp(qc, kt):
                    sc = ps_sc.tile([128, 1024], F32, tag="sc", name="sc")
                    nc.tensor.matmul(sc[:, 0:512],
                                     kT_sb[0:64, b, 128 * kt:128 * (kt + 1)],
                                     qT_sb[0:64, b, QC * qc:QC * (qc + 1)],
                                     start=True, stop=True, tile_position=(0, 0))
                    nc.tensor.matmul(sc[:, 512:1024],
                                     kT_sb[64:128, b, 128 * kt:128 * (kt + 1)],
                                     qT_sb[64:128, b, QC * qc:QC * (qc + 1)],
                                     start=True, stop=True, tile_position=(64, 0))
                    ex = expp.tile([128, 1024], BF16, name="ex")
                    nc.scalar.activation(ex[:], sc[:], AF.Exp)
                    states[qc]["exs"][kt] = ex

                def av_a(qc, kt):
                    stq = states[qc]
                    if kt == 0:
                        stq["av_a"] = ps_av.tile([128, 512], F32, tag="av", name="av_a")
                        stq["av_b"] = ps_av.tile([128, 512], F32, tag="av", name="av_b")
                    ex = stq["exs"][kt]
                    nc.tensor.matmul(stq["av_a"][0:65, :], v_sb[:, b, 0, kt, 0:65],
                                     ex[:, 0:512], start=(kt == 0), stop=(kt == N_KT - 1))

                def av_b(qc, kt):
                    stq = states[qc]
                    ex = stq["exs"][kt]
                    i2 = nc.tensor.matmul(stq["av_b"][0:65, :], v_sb[:, b, 1, kt, 0:65],
                                          ex[:, 512:1024], start=(kt == 0), stop=(kt == N_KT - 1))
                    if kt == N_KT - 1:
                        av_last.append(i2)

                def finalize(qc):
                    av_a, av_b = states[qc]["av_a"], states[qc]["av_b"]
                    # drain AV psum to SBUF fast (releases psum for next q-chunk)
                    af = finp.tile([128, 1024], F32, tag="af", name="af")
                    nc.vector.tensor_copy(af[0:65, 0:512], av_a[0:65, :])
                    nc.vector.tensor_copy(af[0:65, 512:1024], av_b[0:65, :])
                    # denominator row -> partition 0 (DMA shifts partitions),
                    # fast reciprocal there, then gpsimd broadcast to all lanes
                    rc = finp.tile([128, 1024], F32, tag="rc", name="rc")
                    nc.gpsimd.dma_start(rc[0:1, :], af[64:65, :])
                    rc2 = finp.tile([128, 1024], F32, tag="rc2", name="rc2")
                    nc.vector.reciprocal_approx_fast(rc2[0:1, :], rc[0:1, :])
                    rb = finp.tile([128, 1024], F32, tag="rb", name="rb")
                    nc.gpsimd.partition_broadcast(rb[:, :], rc2[0:1, :])
                    # o^T = o_raw^T * recip + bv; all on partitions 0..63, then
                    # head B is partition-shifted into the stage via DMA
                    osc = stage[:, b, QC * qc:QC * (qc + 1)]
                    nc.vector.tensor_tensor(osc[0:64, :], af[0:64, 0:512], rb[0:64, 0:512], ALU.mult)
                    nc.vector.tensor_scalar(osc[0:64, :], osc[0:64, :], bva_sb[:], None, ALU.add)
                    tb = finp.tile([64, 512], BF16, tag="tb", name="tb")
                    nc.vector.tensor_tensor(tb[:], af[0:64, 512:1024], rb[0:64, 512:1024], ALU.mult)
                    nc.vector.tensor_scalar(tb[:], tb[:], bvb_sb[:], None, ALU.add)
                    nc.gpsimd.dma_start(osc[64:128, :], tb[:])
                    # A2A per q-half: upload each qc's blocks as soon as
                    # staged; issue the collective after the odd qc
                    hf = qc // 2
                    k = 2 * b + hf
                    half = a2a_in[k].ap().rearrange("j p n -> p j n")
                    if qc % 2 == 0:
                        nc.gpsimd.dma_start(
                            half[:, 0:4, :],
                            stage[:, b, 1024 * hf:1024 * hf + 512].rearrange(
                                "p (j n) -> p j n", j=4))
                    else:
                        nc.gpsimd.dma_start(
                            half[:, 4:8, :],
                            stage[:, b, 1024 * hf + 512:1024 * (hf + 1)].rearrange(
                                "p (j n) -> p j n", j=4))
                        nc.gpsimd.collective_compute(
                            "AllToAll", ALU.bypass,
                            replica_groups=[list(range(N_CORES))],
                            ins=[a2a_in[k].ap().opt()],
                            outs=[a2a_out[k].ap().opt()],
                        )

                seq = [(qc, kt) for qc in qcs for kt in range(N_KT)]
                for i, (qc, kt) in enumerate(seq):
                    states.setdefault(qc, {"exs": [None] * N_KT})
                    qk_exp(qc, kt)
                    j = i - OV
                    if j >= 0:
                        av_a(*seq[j])
                    for _ in range(pops):
                        if bg:
                            bg.pop(0)()
                    if j >= 0:
                        av_b(*seq[j])
                        if seq[j][1] == N_KT - 1:
                            finalize(seq[j][0])
                for j in range(max(0, len(seq) - OV), len(seq)):
                    av_a(*seq[j])
                    av_b(*seq[j])
                    if seq[j][1] == N_KT - 1:
                        finalize(seq[j][0])
                return av_last

            # drive: emit only chunk 0 of b0's QKV up front, then start
            # attention qc0 with chunks 1-3 interleaved as background steps
            # (order [K,V] per chunk matches the kt windows that consume them);
            # b1's QKV interleaves into b0's qc1-3.
            l0, q0, k0, v0 = make_qkv_steps(0)
            for step in l0:
                step()
            for s in k0[0]:
                s()
            for s in q0[0]:
                s()
            for s in v0[0]:
                s()
            bgA = []
            for tc8 in (1, 2, 3):
                bgA.extend(k0[tc8])
                bgA.extend(v0[tc8])
            attention(0, bgA, qcs=[0])
            while bgA:
                bgA.pop(0)()
            for tc8 in (1, 2, 3):
                for s in q0[tc8]:
                    s()
            l1, q1, k1, v1 = make_qkv_steps(1)
            bg1 = []
            for tc8 in range(4):
                bg1.append(l1[tc8])
                bg1.extend(q1[tc8])
                bg1.extend(k1[tc8])
                bg1.extend(v1[tc8])
            attention(0, bg1, qcs=[1, 2, 3], pops=1)
            while bg1:
                bg1.pop(0)()
            av_anchors = attention(1, [])

            # deferred bulk const loads (issued after attention DMAs in queue order)
            nc.sync.dma_start(msa_w_sb[:], msa_w[:, :].rearrange("(a p) n -> p a n", p=128))
            nc.sync.dma_start(y_shard_sb[:], y_shard[:, :].rearrange("(a p) n -> p a n", p=128))
            if use_gamma:
                nc.sync.dma_start(gamma_sb[:], gamma_b[:, :])
            if use_beta:
                nc.sync.dma_start(beta_sb[:], beta_b[:, :])

            # ============== msa + residual + LayerNorm per shard ==============
            # rstd = exp(-0.5*ln(var+eps)) keeps everything in the exp table set
            sum_sb = cpool.tile([128, 8], F32)
            ssq_sb = cpool.tile([128, 8], F32)
            res_sb = stp.tile([128, 4, E], F32)
            inv_e = 1.0 / E
            for k in range(4):
                lhs = a2ap.tile([128, 8, 128], BF16, tag="lhs")
                nc.sync.dma_start(lhs[:], a2a_out[k].ap().rearrange("j p n -> p j n"))
                # i-major so consecutive matmuls share lhs weights (LDW dedup);
                # both e-halves accumulate concurrently in two psum tiles
                ps_m0 = ps_acc.tile([128, 512], F32, tag="acc", name="ps_m0")
                ps_m1 = ps_acc.tile([128, 512], F32, tag="acc", name="ps_m1")
                for i in range(8):
                    for ec, ps_m in ((0, ps_m0), (1, ps_m1)):
                        mi = nc.tensor.matmul(ps_m[:], lhs[:, i, :],
                                              msa_w_sb[:, i, 512 * ec:512 * (ec + 1)],
                                              start=(i == 0), stop=(i == 7))
                        if ec == 0 and i == 0:
                            # keep msa out of the PE stream until b1 attention
                            # has progressed past qc k+1 (the A2A data won't be
                            # there earlier; an early msa blocks the in-order PE)
                            _add_dep_helper(
                                mi.ins, av_anchors[min(k + 1, 3)].ins, sync=False,
                                reason="msa gated behind b1 attention progress")
                for ec, ps_m in ((0, ps_m0), (1, ps_m1)):
                    # residual add + per-half stats, all on DVE (no ACT table switch)
                    rhalf = res_sb[:, k, 512 * ec:512 * (ec + 1)]
                    nc.vector.tensor_tensor(
                        rhalf, ps_m[:],
                        y_shard_sb[:, k, 512 * ec:512 * (ec + 1)], ALU.add)
                    dump = finp.tile([128, 512], F32, tag="dump")
                    dump2 = finp.tile([128, 512], BF16, tag="dump2")
                    nc.vector.tensor_scalar(dump2[:], rhalf, 0.0, 0.0, ALU.add,
                                            ALU.add, accum_out=sum_sb[:, 2 * k + ec:2 * k + ec + 1])
                    nc.vector.tensor_tensor(dump[:], rhalf, rhalf, ALU.mult)
                    nc.vector.tensor_scalar(dump2[:], dump[:], 0.0, 0.0, ALU.add,
                                            ALU.add, accum_out=ssq_sb[:, 2 * k + ec:2 * k + ec + 1])
                # per-shard LN scalars [128, 1]
                mu = cpool.tile([128, 4], F32, name=f"mu{k}")
                nc.vector.tensor_tensor(mu[:, 0:1], sum_sb[:, 2 * k:2 * k + 1],
                                        sum_sb[:, 2 * k + 1:2 * k + 2], ALU.add)
                nc.vector.tensor_scalar(mu[:, 0:1], mu[:, 0:1], inv_e, None, ALU.mult)
                nc.vector.tensor_tensor(mu[:, 1:2], ssq_sb[:, 2 * k:2 * k + 1],
                                        ssq_sb[:, 2 * k + 1:2 * k + 2], ALU.add)
                nc.vector.tensor_scalar(mu[:, 1:2], mu[:, 1:2], inv_e, None, ALU.mult)
                nc.vector.tensor_tensor(mu[:, 2:3], mu[:, 0:1], mu[:, 0:1], ALU.mult)
                nc.vector.tensor_tensor(mu[:, 1:2], mu[:, 1:2], mu[:, 2:3], ALU.subtract)
                nc.vector.tensor_scalar(mu[:, 1:2], mu[:, 1:2], EPS, None, ALU.add)
                # rstd = sqrt(1/var) + one Newton polish (keeps msa-era ACT
                # in the sqrt table set only -> a single table load, no thrash)
                nc.vector.reciprocal_approx_fast(mu[:, 2:3], mu[:, 1:2])
                nc.scalar.activation(mu[:, 3:4], mu[:, 2:3], AF.Sqrt)
                nc.vector.tensor_tensor(mu[:, 2:3], mu[:, 3:4], mu[:, 3:4], ALU.mult)
                nc.vector.tensor_tensor(mu[:, 2:3], mu[:, 2:3], mu[:, 1:2], ALU.mult)
                nc.vector.tensor_scalar(mu[:, 2:3], mu[:, 2:3], -0.5, 1.5, ALU.mult, ALU.add)
                nc.vector.tensor_tensor(mu[:, 3:4], mu[:, 3:4], mu[:, 2:3], ALU.mult)
                # -mu * rstd
                nc.vector.tensor_scalar(mu[:, 0:1], mu[:, 0:1], -1.0, None, ALU.mult)
                nc.vector.tensor_tensor(mu[:, 0:1], mu[:, 0:1], mu[:, 3:4], ALU.mult)
                # normalize on DVE (fused scale+bias) so ACT runs only Ln/Exp
                # in this phase -- avoids per-shard activation-table reloads
                o1 = finp.tile([128, E], F32, tag="o1")
                nc.vector.tensor_scalar(o1[:], res_sb[:, k, :], mu[:, 3:4],
                                        mu[:, 0:1], ALU.mult, ALU.add)
                if use_gamma:
                    nc.vector.tensor_tensor(o1[:], o1[:], gamma_sb[:], ALU.mult)
                if use_beta:
                    nc.vector.tensor_tensor(o1[:], o1[:], beta_sb[:], ALU.add)
                nc.sync.dma_start(out[128 * k:128 * (k + 1), :], o1[:])

    nc.compile()
    return nc


@functools.lru_cache(maxsize=4)
def _get_nc(use_gamma: bool, use_beta: bool):
    return _build(use_gamma, use_beta)


def kernel(**inputs) -> np.ndarray:
    y = np.asarray(inputs["y"], np.float32)
    Wqkv = np.asarray(inputs["Wqkv"], np.float32)
    bqkv = np.asarray(inputs["bqkv"], np.float32)
    Wmsa = np.asarray(inputs["Wmsa"], np.float32)
    Bq_, Aq_ = np.asarray(inputs["Bq"], np.float32), np.asarray(inputs["Aq"], np.float32)
    Bk_, Ak_ = np.asarray(inputs["Bk"], np.float32), np.asarray(inputs["Ak"], np.float32)
    Bv_, Av_ = np.asarray(inputs["Bv"], np.float32), np.asarray(inputs["Av"], np.float32)
    Bo_, Ao_ = np.asarray(inputs["Bo"], np.float32), np.asarray(inputs["Ao"], np.float32)
    gamma = np.asarray(inputs["gamma"], np.float32)
    beta = np.asarray(inputs["beta"], np.float32)

    scale = np.float32(1.0 / np.sqrt(D))

    # effective weights: qkv = y @ (Wqkv.T + blockdiag-ish LoRA) + bqkv
    # y @ W.T: W rows are output dims. LoRA adds y @ (B@A): effective W += (B@A).T
    W_eff = Wqkv.copy()
    W_eff[0:E] += (Bq_ @ Aq_).T
    W_eff[E:2 * E] += (Bk_ @ Ak_).T
    W_eff[2 * E:3 * E] += (Bv_ @ Av_).T
    # fold 1/sqrt(D) into Q projection
    W_eff[0:E] *= scale
    bq_eff = bqkv[0:E] * scale
    bk_eff = bqkv[E:2 * E]
    bv_eff = bqkv[2 * E:3 * E]
    # msa: o @ Wmsa.T + o @ (Bo@Ao) = o @ M with M = Wmsa.T + Bo@Ao  [E(d), E(out)]
    M = Wmsa.T + Bo_ @ Ao_

    y_flat = y.reshape(T, E)
    yT_bf = np.ascontiguousarray(y_flat.T).astype(ml_dtypes.bfloat16)
    M_bf = np.ascontiguousarray(M).astype(ml_dtypes.bfloat16)

    use_gamma = not np.allclose(gamma, 1.0)
    use_beta = not np.allclose(beta, 0.0)
    nc = _get_nc(use_gamma, use_beta)

    in_maps = []
    for c in range(N_CORES):
        r0 = c * 128
        r1 = r0 + 128
        wq_c = np.ascontiguousarray(W_eff[0:E][r0:r1].T).astype(ml_dtypes.bfloat16)
        wk_c = np.ascontiguousarray(W_eff[E:2 * E][r0:r1].T).astype(ml_dtypes.bfloat16)
        wv_c = np.ascontiguousarray(W_eff[2 * E:3 * E][r0:r1].T).astype(ml_dtypes.bfloat16)
        tok = np.concatenate([
            np.arange(128 * c, 128 * c + 128),
            np.arange(1024 + 128 * c, 1024 + 128 * c + 128),
            np.arange(2048 + 128 * c, 2048 + 128 * c + 128),
            np.arange(3072 + 128 * c, 3072 + 128 * c + 128),
        ])
        m = {
            "yT": yT_bf,
            "wqT": wq_c,
            "wkT": wk_c,
            "wvT": wv_c,
            "bq": bq_eff[r0:r1].reshape(128, 1).copy(),
            "bk": bk_eff[r0:r1].reshape(128, 1).copy(),
            "bva": bv_eff[r0:r0 + 64].reshape(64, 1).copy(),
            "bvb": bv_eff[r0 + 64:r1].reshape(64, 1).copy(),
            "msa_w": M_bf,
            "y_shard": np.ascontiguousarray(y_flat[tok]),
        }
        if use_gamma:
            m["gamma_b"] = np.broadcast_to(gamma, (128, E)).copy()
        if use_beta:
            m["beta_b"] = np.broadcast_to(beta, (128, E)).copy()
        in_maps.append(m)

    res = bass_utils.run_bass_kernel_spmd(nc, in_maps, core_ids=list(range(N_CORES)))

    out_full = np.empty((T, E), np.float32)
    for c in range(N_CORES):
        oc = res.results[c]["out"]
        out_full[128 * c:128 * c + 128] = oc[0:128]
        out_full[1024 + 128 * c:1024 + 128 * c + 128] = oc[128:256]
        out_full[2048 + 128 * c:2048 + 128 * c + 128] = oc[256:384]
        out_full[3072 + 128 * c:3072 + 128 * c + 128] = oc[384:512]
    return out_full.reshape(B, S, E)



# revision 3
# speedup vs baseline: 1.0904x; 1.0904x over previous
"""Trainium2 Bass kernel for fused LoRA-attention block (nn_Attention_18846316494887).

Reference computation:
  qkv = y @ Wqkv.T + bqkv (+ LoRA deltas y @ (B@A) per Q/K/V)  -> Q,K,V [B,H,S,D]
  attn = softmax(Q K^T / sqrt(D)); o = attn @ V -> [B,S,E]
  msa = o @ Wmsa.T + o @ (Bo@Ao); res = msa + y; out = LayerNorm(res)*gamma + beta

Sharding: tensor-parallel over heads (2 heads/core, 8 cores), AllToAll to
reshard head-dim -> token-dim before the output projection, token-parallel
msa + LayerNorm, host-side gather of per-core token shards.

Precision plan (error budget: attention path contributes only ~2.2% of the
LN'd output norm, so a few-% relative error there is invisible):
  - y, Wqkv (x32), V, exp(scores) all in fp8e4m3; f32 PSUM accumulation
  - Q/K projection matmuls in DoubleRow mode (2 fp8 k-subtiles per pass)
  - AV matmuls in DoubleRow mode over kt-pairs (halves the ex stream time)
  - the x32*x32 weight scaling and 1/sqrt(D) fold into the exp's free
    affine scale (exp(x * 1/8192)); V-scale folds into msa weights (/32)

Host-side prep (exact algebra, no approximation):
  - LoRA folded into Wqkv / Wmsa (y@W.T + y@(B@A) == y@(W.T + B@A))
  - V bias applied post-softmax on o (exact since attn rows sum to 1)
  - y pre-transposed to [E, T] for the QKV matmuls
"""
import functools
import numpy as np
import ml_dtypes

import concourse.mybir as mybir
import concourse.tile as tile
from concourse import bacc
from concourse import bass_utils
from concourse.bass import _add_dep_helper

# problem shapes (hardcoded per harness contract)
E = 1024
H = 16
D = 64
B = 2
S = 2048
T = B * S          # 4096 tokens
N_CORES = 8
EPS = 1e-6

BF16 = mybir.dt.bfloat16
F32 = mybir.dt.float32
F8 = mybir.dt.float8e4
NP_F8 = ml_dtypes.float8_e4m3
AF = mybir.ActivationFunctionType
ALU = mybir.AluOpType
DR = mybir.MatmulPerfMode.DoubleRow

# per-core worksizes
TOK = T // N_CORES          # 512 tokens per core for msa/LN
QC = 512                    # attention q-chunk
N_QC = S // QC              # 4 q-chunks per (b, head-pair)
N_KT = S // 128             # 16 k-tiles
N_KP = N_KT // 2            # 8 kt-pairs (DoubleRow AV granularity)
VW = 80                     # padded V row (64 d + 1 ones + pad to 16B mult)
WSC = 32.0                  # fp8 weight pre-scale
S_ACT = 1.0 / (WSC * WSC * 8.0)   # exp affine scale: /32^2 (w-scales) /sqrt(D)


def _build(use_gamma: bool, use_beta: bool):
    nc = bacc.Bacc("TRN2", target_bir_lowering=False, debug=False, num_devices=N_CORES)

    # ---- DRAM parameters -------------------------------------------------
    yT = nc.dram_tensor("yT", [E, T], F8, kind="ExternalInput")
    wqT = nc.dram_tensor("wqT", [E, 128], F8, kind="ExternalInput")
    wkT = nc.dram_tensor("wkT", [E, 128], F8, kind="ExternalInput")
    wvT = nc.dram_tensor("wvT", [E, 128], F8, kind="ExternalInput")
    bq = nc.dram_tensor("bq", [128, 1], F32, kind="ExternalInput")
    bk = nc.dram_tensor("bk", [128, 1], F32, kind="ExternalInput")
    bva = nc.dram_tensor("bva", [64, 1], F32, kind="ExternalInput")
    bvb = nc.dram_tensor("bvb", [64, 1], F32, kind="ExternalInput")
    msa_w = nc.dram_tensor("msa_w", [E, E], BF16, kind="ExternalInput")
    y_shard = nc.dram_tensor("y_shard", [TOK, E], F32, kind="ExternalInput")
    if use_gamma:
        gamma_b = nc.dram_tensor("gamma_b", [128, E], F32, kind="ExternalInput")
    if use_beta:
        beta_b = nc.dram_tensor("beta_b", [128, E], F32, kind="ExternalInput")
    out = nc.dram_tensor("out", [TOK, E], F32, kind="ExternalOutput")

    # internal DRAM: A2A bounce buffers (shard k: (b, q-half) -> 128 tok/core)
    a2a_in = [nc.dram_tensor(f"a2a_in{k}", [N_CORES, 128, 128], BF16) for k in range(4)]
    a2a_out = [nc.dram_tensor(f"a2a_out{k}", [N_CORES, 128, 128], BF16) for k in range(4)]

    with tile.TileContext(nc) as tc:
        with (
            tc.tile_pool(name="const", bufs=1) as cpool,
            tc.tile_pool(name="yt", bufs=5) as ytp,
            tc.tile_pool(name="qk", bufs=1) as qkp,
            tc.tile_pool(name="exp", bufs=3) as expp,
            tc.tile_pool(name="stage", bufs=1) as stp,
            tc.tile_pool(name="fin", bufs=2) as finp,
            tc.tile_pool(name="a2asb", bufs=4) as a2ap,
            tc.tile_pool(name="ps_acc", bufs=2, space="PSUM") as ps_acc,
            tc.tile_pool(name="ps_sc", bufs=2, space="PSUM") as ps_sc,
            tc.tile_pool(name="ps_av", bufs=2, space="PSUM") as ps_av,
        ):
            # ---- constants -------------------------------------------------
            wqT_sb = cpool.tile([128, 8, 128], F8)
            wkT_sb = cpool.tile([128, 8, 128], F8)
            wvT_sb = cpool.tile([128, 8, 128], F8)
            nc.sync.dma_start(wqT_sb[:], wqT[:, :].rearrange("(a p) n -> p a n", p=128))
            nc.sync.dma_start(wkT_sb[:], wkT[:, :].rearrange("(a p) n -> p a n", p=128))
            nc.sync.dma_start(wvT_sb[:], wvT[:, :].rearrange("(a p) n -> p a n", p=128))
            bq_sb = cpool.tile([128, 1], F32)
            bk_sb = cpool.tile([128, 1], F32)
            bva_sb = cpool.tile([64, 1], F32)
            bvb_sb = cpool.tile([64, 1], F32)
            nc.sync.dma_start(bq_sb[:], bq[:, :])
            nc.sync.dma_start(bk_sb[:], bk[:, :])
            nc.sync.dma_start(bva_sb[:], bva[:, :])
            nc.sync.dma_start(bvb_sb[:], bvb[:, :])
            # (msa weights / LN consts are DMA'd later, after the attention
            # loops are issued, so startup DMA bandwidth goes to yT tiles)
            msa_w_sb = cpool.tile([128, 8, E], BF16)
            y_shard_sb = cpool.tile([128, 4, E], F32)
            if use_gamma:
                gamma_sb = cpool.tile([128, E], F32)
            if use_beta:
                beta_sb = cpool.tile([128, E], F32)

            # V tiles, padded: [k-part, b, head, kt, VW]; col 64 = ones
            v_sb = cpool.tile([128, B, 2, N_KT, VW], F8)
            nc.vector.memset(v_sb[:, :, :, :, 64:VW], 0.0)
            nc.vector.memset(v_sb[:, :, :, :, 64:65], 1.0)

            # Q^T/K^T: [d-part(2 heads), b, q]
            qT_sb = qkp.tile([128, B, S], BF16)
            kT_sb = qkp.tile([128, B, S], BF16)
            # o^T staging for A2A: [d-part, b, q]
            stage = stp.tile([128, B, S], BF16)

            # ============== main per-batch pipeline ==============
            def make_qkv_steps(b):
                """QKV projection for batch b as a list of small closures so the
                PE work can be interleaved into the other batch's attention
                (fills the in-order PE stream's exp-wait slots)."""
                loads, qs, ks, vs = [], [], [], []
                for tc8 in range(4):
                    st8 = {}

                    def load(b=b, tc8=tc8, st8=st8):
                        yt = ytp.tile([128, 8, 512], F8, tag="yt")
                        st8["yt"] = yt
                        for et in range(8):
                            nc.sync.dma_start(
                                yt[:, et, :], yT[128 * et:128 * (et + 1),
                                                 b * S + 512 * tc8: b * S + 512 * (tc8 + 1)])
                    loads.append(load)
                    qs.append([])
                    ks.append([])
                    vs.append([])

                    # Q/K: 4 DoubleRow matmuls (et-pairs), K=1024 contraction.
                    # DR forbids column tile_position offsets, so each mm is
                    # full-width [128, 2, 128] -> out [128, 512].
                    for eg in range(4):
                        def qstep(b=b, tc8=tc8, eg=eg, st8=st8):
                            if eg == 0:
                                st8["ps_q"] = ps_acc.tile([128, 512], F32, tag="acc", name="ps_q")
                            ps_q, yt = st8["ps_q"], st8["yt"]
                            st, sp = (eg == 0), (eg == 3)
                            nc.tensor.matmul(ps_q[:], wqT_sb[:, 2 * eg:2 * eg + 2, :],
                                             yt[:, 2 * eg:2 * eg + 2, :], start=st, stop=sp,
                                             perf_mode=DR)
                            if eg == 3:
                                nc.vector.tensor_scalar(
                                    qT_sb[:, b, 512 * tc8:512 * (tc8 + 1)], ps_q[:],
                                    bq_sb[:], None, ALU.add)
                        qs[tc8].append(qstep)

                    for eg in range(4):
                        def kstep(b=b, tc8=tc8, eg=eg, st8=st8):
                            if eg == 0:
                                st8["ps_k"] = ps_acc.tile([128, 512], F32, tag="acc", name="ps_k")
                            ps_k, yt = st8["ps_k"], st8["yt"]
                            st, sp = (eg == 0), (eg == 3)
                            nc.tensor.matmul(ps_k[:], wkT_sb[:, 2 * eg:2 * eg + 2, :],
                                             yt[:, 2 * eg:2 * eg + 2, :], start=st, stop=sp,
                                             perf_mode=DR)
                            if eg == 3:
                                nc.vector.tensor_scalar(
                                    kT_sb[:, b, 512 * tc8:512 * (tc8 + 1)], ps_k[:],
                                    bk_sb[:], None, ALU.add)
                        ks[tc8].append(kstep)

                    # V: [tok, vdim] layout, fp8 operands (no DoubleRow: the
                    # stationary operand changes every matmul)
                    for eg in range(4):
                        def vstep(b=b, tc8=tc8, eg=eg, st8=st8):
                            if eg == 0:
                                st8["ps_v"] = ps_acc.tile([128, 512], F32, tag="acc", name="ps_v")
                            ps_v, yt = st8["ps_v"], st8["yt"]
                            for et in (2 * eg, 2 * eg + 1):
                                st, sp = (et == 0), (et == 7)
                                for s4 in range(4):
                                    nc.tensor.matmul(ps_v[:, 128 * s4:128 * (s4 + 1)],
                                                     yt[:, et, 128 * s4:128 * (s4 + 1)],
                                                     wvT_sb[:, et, :], start=st, stop=sp)
                            if eg == 3:
                                for h in range(2):
                                    src = ps_v[:, :].rearrange(
                                        "p (s n) -> p s n", s=4)[:, :, 64 * h:64 * (h + 1)]
                                    nc.vector.tensor_copy(
                                        v_sb[:, b, h, 4 * tc8:4 * (tc8 + 1), 0:64], src)
                        vs[tc8].append(vstep)
                return loads, qs, ks, vs

            def attention(b, bg, av_last=None, qcs=range(N_QC), pops=2):
                # software-pipelined ACROSS kt steps: qk/exp runs OV steps
                # ahead of av, so the ACT engine never drains at qc
                # boundaries; bg closures (other QKV work) fill PE wait slots.
                # AV runs per kt-PAIR in fp8 DoubleRow mode.
                if av_last is None:
                    av_last = []
                OV = 4
                states = {}

                def qk_exp(qc, kt):
                    stq = states[qc]
                    if kt % 2 == 0:
                        stq["exs"][kt // 2] = expp.tile([128, 2, 1024], F8, name="ex")
                    sc = ps_sc.tile([128, 1024], F32, tag="sc", name="sc")
                    nc.tensor.matmul(sc[:, 0:512],
                                     kT_sb[0:64, b, 128 * kt:128 * (kt + 1)],
                                     qT_sb[0:64, b, QC * qc:QC * (qc + 1)],
                                     start=True, stop=True, tile_position=(0, 0))
                    nc.tensor.matmul(sc[:, 512:1024],
                                     kT_sb[64:128, b, 128 * kt:128 * (kt + 1)],
                                     qT_sb[64:128, b, QC * qc:QC * (qc + 1)],
                                     start=True, stop=True, tile_position=(64, 0))
                    ex = stq["exs"][kt // 2]
                    nc.scalar.activation(ex[:, kt % 2, :], sc[:], AF.Exp, scale=S_ACT)

                def av_a(qc, kp):
                    stq = states[qc]
                    if kp == 0:
                        stq["av_a"] = ps_av.tile([128, 512], F32, tag="av", name="av_a")
                        stq["av_b"] = ps_av.tile([128, 512], F32, tag="av", name="av_b")
                    ex = stq["exs"][kp]
                    nc.tensor.matmul(stq["av_a"][0:65, :],
                                     v_sb[:, b, 0, 2 * kp:2 * kp + 2, 0:65],
                                     ex[:, :, 0:512],
                                     start=(kp == 0), stop=(kp == N_KP - 1), perf_mode=DR)

                def av_b(qc, kp):
                    stq = states[qc]
                    ex = stq["exs"][kp]
                    i2 = nc.tensor.matmul(stq["av_b"][0:65, :],
                                          v_sb[:, b, 1, 2 * kp:2 * kp + 2, 0:65],
                                          ex[:, :, 512:1024],
                                          start=(kp == 0), stop=(kp == N_KP - 1), perf_mode=DR)
                    if kp == N_KP - 1:
                        av_last.append(i2)

                def finalize(qc):
                    av_a, av_b = states[qc]["av_a"], states[qc]["av_b"]
                    # drain AV psum to SBUF fast (releases psum for next q-chunk)
                    af = finp.tile([128, 1024], F32, tag="af", name="af")
                    nc.vector.tensor_copy(af[0:65, 0:512], av_a[0:65, :])
                    nc.vector.tensor_copy(af[0:65, 512:1024], av_b[0:65, :])
                    # denominator row -> partition 0 (DMA shifts partitions),
                    # fast reciprocal there, then gpsimd broadcast to all lanes
                    rc = finp.tile([128, 1024], F32, tag="rc", name="rc")
                    nc.gpsimd.dma_start(rc[0:1, :], af[64:65, :])
                    rc2 = finp.tile([128, 1024], F32, tag="rc2", name="rc2")
                    nc.vector.reciprocal_approx_fast(rc2[0:1, :], rc[0:1, :])
                    rb = finp.tile([128, 1024], F32, tag="rb", name="rb")
                    nc.gpsimd.partition_broadcast(rb[:, :], rc2[0:1, :])
                    # o^T = o_raw^T * recip + bv; all on partitions 0..63, then
                    # head B is partition-shifted into the stage via DMA
                    osc = stage[:, b, QC * qc:QC * (qc + 1)]
                    nc.vector.tensor_tensor(osc[0:64, :], af[0:64, 0:512], rb[0:64, 0:512], ALU.mult)
                    nc.vector.tensor_scalar(osc[0:64, :], osc[0:64, :], bva_sb[:], None, ALU.add)
                    tb = finp.tile([64, 512], BF16, tag="tb", name="tb")
                    nc.vector.tensor_tensor(tb[:], af[0:64, 512:1024], rb[0:64, 512:1024], ALU.mult)
                    nc.vector.tensor_scalar(tb[:], tb[:], bvb_sb[:], None, ALU.add)
                    nc.gpsimd.dma_start(osc[64:128, :], tb[:])
                    # A2A per q-half: upload each qc's blocks as soon as
                    # staged; issue the collective after the odd qc
                    hf = qc // 2
                    k = 2 * b + hf
                    half = a2a_in[k].ap().rearrange("j p n -> p j n")
                    if qc % 2 == 0:
                        nc.gpsimd.dma_start(
                            half[:, 0:4, :],
                            stage[:, b, 1024 * hf:1024 * hf + 512].rearrange(
                                "p (j n) -> p j n", j=4))
                    else:
                        nc.gpsimd.dma_start(
                            half[:, 4:8, :],
                            stage[:, b, 1024 * hf + 512:1024 * (hf + 1)].rearrange(
                                "p (j n) -> p j n", j=4))
                        nc.gpsimd.collective_compute(
                            "AllToAll", ALU.bypass,
                            replica_groups=[list(range(N_CORES))],
                            ins=[a2a_in[k].ap().opt()],
                            outs=[a2a_out[k].ap().opt()],
                        )

                seq = [(qc, kt) for qc in qcs for kt in range(N_KT)]
                for i, (qc, kt) in enumerate(seq):
                    states.setdefault(qc, {"exs": [None] * N_KP})
                    qk_exp(qc, kt)
                    for _ in range(pops):
                        if bg:
                            bg.pop(0)()
                    j = i - OV
                    if j >= 0 and seq[j][1] % 2 == 1:
                        jqc, jkt = seq[j]
                        av_a(jqc, jkt // 2)
                        av_b(jqc, jkt // 2)
                        if jkt == N_KT - 1:
                            finalize(jqc)
                for j in range(max(0, len(seq) - OV), len(seq)):
                    if seq[j][1] % 2 == 1:
                        jqc, jkt = seq[j]
                        av_a(jqc, jkt // 2)
                        av_b(jqc, jkt // 2)
                        if jkt == N_KT - 1:
                            finalize(jqc)
                return av_last

            # drive: emit only chunk 0 of b0's QKV up front, then start
            # attention qc0 with chunks 1-3 interleaved as background steps
            # (order [K,V] per chunk matches the kt windows that consume them);
            # b1's QKV interleaves into b0's qc1-3.
            l0, q0, k0, v0 = make_qkv_steps(0)
            for step in l0:
                step()
            for s in k0[0]:
                s()
            for s in q0[0]:
                s()
            for s in v0[0]:
                s()
            bgA = []
            for tc8 in (1, 2, 3):
                bgA.extend(k0[tc8])
                bgA.extend(v0[tc8])
            attention(0, bgA, qcs=[0])
            while bgA:
                bgA.pop(0)()
            for tc8 in (1, 2, 3):
                for s in q0[tc8]:
                    s()
            l1, q1, k1, v1 = make_qkv_steps(1)
            bg1 = []
            for tc8 in range(4):
                bg1.append(l1[tc8])
                bg1.extend(q1[tc8])
                bg1.extend(k1[tc8])
                bg1.extend(v1[tc8])
            attention(0, bg1, qcs=[1, 2, 3], pops=1)
            while bg1:
                bg1.pop(0)()
            av_anchors = attention(1, [])

            # deferred bulk const loads (issued after attention DMAs in queue order)
            nc.sync.dma_start(msa_w_sb[:], msa_w[:, :].rearrange("(a p) n -> p a n", p=128))
            nc.sync.dma_start(y_shard_sb[:], y_shard[:, :].rearrange("(a p) n -> p a n", p=128))
            if use_gamma:
                nc.sync.dma_start(gamma_sb[:], gamma_b[:, :])
            if use_beta:
                nc.sync.dma_start(beta_sb[:], beta_b[:, :])

            # ============== msa + residual + LayerNorm per shard ==============
            sum_sb = cpool.tile([128, 8], F32)
            ssq_sb = cpool.tile([128, 8], F32)
            res_sb = stp.tile([128, 4, E], F32)
            inv_e = 1.0 / E
            for k in range(4):
                lhs = a2ap.tile([128, 8, 128], BF16, tag="lhs")
                nc.sync.dma_start(lhs[:], a2a_out[k].ap().rearrange("j p n -> p j n"))
                # i-major so consecutive matmuls share lhs weights (LDW dedup);
                # both e-halves accumulate concurrently in two psum tiles
                ps_m0 = ps_acc.tile([128, 512], F32, tag="acc", name="ps_m0")
                ps_m1 = ps_acc.tile([128, 512], F32, tag="acc", name="ps_m1")
                for i in range(8):
                    for ec, ps_m in ((0, ps_m0), (1, ps_m1)):
                        mi = nc.tensor.matmul(ps_m[:], lhs[:, i, :],
                                              msa_w_sb[:, i, 512 * ec:512 * (ec + 1)],
                                              start=(i == 0), stop=(i == 7))
                        if ec == 0 and i == 0:
                            # keep msa out of the PE stream until b1 attention
                            # has progressed past qc k+1 (the A2A data won't be
                            # there earlier; an early msa blocks the in-order PE)
                            _add_dep_helper(
                                mi.ins, av_anchors[min(k + 1, 3)].ins, sync=False,
                                reason="msa gated behind b1 attention progress")
                for ec, ps_m in ((0, ps_m0), (1, ps_m1)):
                    # residual add + per-half stats, all on DVE (no ACT table switch)
                    rhalf = res_sb[:, k, 512 * ec:512 * (ec + 1)]
                    nc.vector.tensor_tensor(
                        rhalf, ps_m[:],
                        y_shard_sb[:, k, 512 * ec:512 * (ec + 1)], ALU.add)
                    dump = finp.tile([128, 512], F32, tag="dump")
                    dump2 = finp.tile([128, 512], BF16, tag="dump2")
                    nc.vector.tensor_scalar(dump2[:], rhalf, 0.0, 0.0, ALU.add,
                                            ALU.add, accum_out=sum_sb[:, 2 * k + ec:2 * k + ec + 1])
                    nc.vector.tensor_tensor(dump[:], rhalf, rhalf, ALU.mult)
                    nc.vector.tensor_scalar(dump2[:], dump[:], 0.0, 0.0, ALU.add,
                                            ALU.add, accum_out=ssq_sb[:, 2 * k + ec:2 * k + ec + 1])
                # per-shard LN scalars [128, 1]
                mu = cpool.tile([128, 4], F32, name=f"mu{k}")
                nc.vector.tensor_tensor(mu[:, 0:1], sum_sb[:, 2 * k:2 * k + 1],
                                        sum_sb[:, 2 * k + 1:2 * k + 2], ALU.add)
                nc.vector.tensor_scalar(mu[:, 0:1], mu[:, 0:1], inv_e, None, ALU.mult)
                nc.vector.tensor_tensor(mu[:, 1:2], ssq_sb[:, 2 * k:2 * k + 1],
                                        ssq_sb[:, 2 * k + 1:2 * k + 2], ALU.add)
                nc.vector.tensor_scalar(mu[:, 1:2], mu[:, 1:2], inv_e, None, ALU.mult)
                nc.vector.tensor_tensor(mu[:, 2:3], mu[:, 0:1], mu[:, 0:1], ALU.mult)
                nc.vector.tensor_tensor(mu[:, 1:2], mu[:, 1:2], mu[:, 2:3], ALU.subtract)
                nc.vector.tensor_scalar(mu[:, 1:2], mu[:, 1:2], EPS, None, ALU.add)
                # rstd = sqrt(1/var) + one Newton polish (keeps msa-era ACT
                # in the sqrt table set only -> a single table load, no thrash)
                nc.vector.reciprocal_approx_fast(mu[:, 2:3], mu[:, 1:2])
                nc.scalar.activation(mu[:, 3:4], mu[:, 2:3], AF.Sqrt)
                nc.vector.tensor_tensor(mu[:, 2:3], mu[:, 3:4], mu[:, 3:4], ALU.mult)
                nc.vector.tensor_tensor(mu[:, 2:3], mu[:, 2:3], mu[:, 1:2], ALU.mult)
                nc.vector.tensor_scalar(mu[:, 2:3], mu[:, 2:3], -0.5, 1.5, ALU.mult, ALU.add)
                nc.vector.tensor_tensor(mu[:, 3:4], mu[:, 3:4], mu[:, 2:3], ALU.mult)
                # -mu * rstd
                nc.vector.tensor_scalar(mu[:, 0:1], mu[:, 0:1], -1.0, None, ALU.mult)
                nc.vector.tensor_tensor(mu[:, 0:1], mu[:, 0:1], mu[:, 3:4], ALU.mult)
                # normalize on DVE (fused scale+bias) so ACT runs only Ln/Exp
                # in this phase -- avoids per-shard activation-table reloads
                o1 = finp.tile([128, E], F32, tag="o1")
                nc.vector.tensor_scalar(o1[:], res_sb[:, k, :], mu[:, 3:4],
                                        mu[:, 0:1], ALU.mult, ALU.add)
                if use_gamma:
                    nc.vector.tensor_tensor(o1[:], o1[:], gamma_sb[:], ALU.mult)
                if use_beta:
                    nc.vector.tensor_tensor(o1[:], o1[:], beta_sb[:], ALU.add)
                nc.sync.dma_start(out[128 * k:128 * (k + 1), :], o1[:])

    nc.compile()
    return nc


@functools.lru_cache(maxsize=4)
def _get_nc(use_gamma: bool, use_beta: bool):
    return _build(use_gamma, use_beta)


def kernel(**inputs) -> np.ndarray:
    y = np.asarray(inputs["y"], np.float32)
    Wqkv = np.asarray(inputs["Wqkv"], np.float32)
    bqkv = np.asarray(inputs["bqkv"], np.float32)
    Wmsa = np.asarray(inputs["Wmsa"], np.float32)
    Bq_, Aq_ = np.asarray(inputs["Bq"], np.float32), np.asarray(inputs["Aq"], np.float32)
    Bk_, Ak_ = np.asarray(inputs["Bk"], np.float32), np.asarray(inputs["Ak"], np.float32)
    Bv_, Av_ = np.asarray(inputs["Bv"], np.float32), np.asarray(inputs["Av"], np.float32)
    Bo_, Ao_ = np.asarray(inputs["Bo"], np.float32), np.asarray(inputs["Ao"], np.float32)
    gamma = np.asarray(inputs["gamma"], np.float32)
    beta = np.asarray(inputs["beta"], np.float32)

    # effective weights: qkv = y @ (Wqkv.T + blockdiag-ish LoRA) + bqkv
    # y @ W.T: W rows are output dims. LoRA adds y @ (B@A): effective W += (B@A).T
    W_eff = Wqkv.copy()
    W_eff[0:E] += (Bq_ @ Aq_).T
    W_eff[E:2 * E] += (Bk_ @ Ak_).T
    W_eff[2 * E:3 * E] += (Bv_ @ Av_).T
    # fp8 pre-scale: weights x32 (1/sqrt(D) and the scale unwind live in
    # the exp affine scale and the /32 on the msa weights)
    W_eff *= WSC
    bq_eff = bqkv[0:E] * WSC
    bk_eff = bqkv[E:2 * E] * WSC
    bv_eff = bqkv[2 * E:3 * E] * WSC
    # msa: o @ Wmsa.T + o @ (Bo@Ao) = o @ M with M = Wmsa.T + Bo@Ao  [E(d), E(out)]
    M = (Wmsa.T + Bo_ @ Ao_) * (1.0 / WSC)

    y_flat = y.reshape(T, E)
    yT_f8 = np.ascontiguousarray(y_flat.T).astype(NP_F8)
    M_bf = np.ascontiguousarray(M).astype(ml_dtypes.bfloat16)

    use_gamma = not np.allclose(gamma, 1.0)
    use_beta = not np.allclose(beta, 0.0)
    nc = _get_nc(use_gamma, use_beta)

    in_maps = []
    for c in range(N_CORES):
        r0 = c * 128
        r1 = r0 + 128
        wq_c = np.ascontiguousarray(W_eff[0:E][r0:r1].T).astype(NP_F8)
        wk_c = np.ascontiguousarray(W_eff[E:2 * E][r0:r1].T).astype(NP_F8)
        wv_c = np.ascontiguousarray(W_eff[2 * E:3 * E][r0:r1].T).astype(NP_F8)
        tok = np.concatenate([
            np.arange(128 * c, 128 * c + 128),
            np.arange(1024 + 128 * c, 1024 + 128 * c + 128),
            np.arange(2048 + 128 * c, 2048 + 128 * c + 128),
            np.arange(3072 + 128 * c, 3072 + 128 * c + 128),
        ])
        m = {
            "yT": yT_f8,
            "wqT": wq_c,
            "wkT": wk_c,
            "wvT": wv_c,
            "bq": bq_eff[r0:r1].reshape(128, 1).copy(),
            "bk": bk_eff[r0:r1].reshape(128, 1).copy(),
            "bva": bv_eff[r0:r0 + 64].reshape(64, 1).copy(),
            "bvb": bv_eff[r0 + 64:r1].reshape(64, 1).copy(),
            "msa_w": M_bf,
            "y_shard": np.ascontiguousarray(y_flat[tok]),
        }
        if use_gamma:
            m["gamma_b"] = np.broadcast_to(gamma, (128, E)).copy()
        if use_beta:
            m["beta_b"] = np.broadcast_to(beta, (128, E)).copy()
        in_maps.append(m)

    res = bass_utils.run_bass_kernel_spmd(nc, in_maps, core_ids=list(range(N_CORES)))

    out_full = np.empty((T, E), np.float32)
    for c in range(N_CORES):
        oc = res.results[c]["out"]
        out_full[128 * c:128 * c + 128] = oc[0:128]
        out_full[1024 + 128 * c:1024 + 128 * c + 128] = oc[128:256]
        out_full[2048 + 128 * c:2048 + 128 * c + 128] = oc[256:384]
        out_full[3072 + 128 * c:3072 + 128 * c + 128] = oc[384:512]
    return out_full.reshape(B, S, E)


# revision 18
# speedup vs baseline: 1.1342x; 1.0402x over previous
"""Trainium2 Bass kernel for fused LoRA-attention block (nn_Attention_18846316494887).

Reference computation:
  qkv = y @ Wqkv.T + bqkv (+ LoRA deltas y @ (B@A) per Q/K/V)  -> Q,K,V [B,H,S,D]
  attn = softmax(Q K^T / sqrt(D)); o = attn @ V -> [B,S,E]
  msa = o @ Wmsa.T + o @ (Bo@Ao); res = msa + y; out = LayerNorm(res)*gamma + beta

Sharding: tensor-parallel over heads (2 heads/core, 8 cores), AllToAll to
reshard head-dim -> token-dim before the output projection, token-parallel
msa + LayerNorm, host-side gather of per-core token shards.

Precision plan (error budget: attention path contributes only ~2.2% of the
LN'd output norm, so a few-% relative error there is invisible):
  - y, Wqkv (x32), V, exp(scores) all in fp8e4m3; f32 PSUM accumulation
  - Q/K projection matmuls in DoubleRow mode (2 fp8 k-subtiles per pass)
  - AV matmuls in DoubleRow mode over kt-pairs (halves the ex stream time)
  - the x32*x32 weight scaling and 1/sqrt(D) fold into the exp's free
    affine scale (exp(x * 1/8192)); V-scale folds into msa weights (/32)

Host-side prep (exact algebra, no approximation):
  - LoRA folded into Wqkv / Wmsa (y@W.T + y@(B@A) == y@(W.T + B@A))
  - V bias applied post-softmax on o (exact since attn rows sum to 1)
  - y pre-transposed to [E, T] for the QKV matmuls
"""
import functools
import numpy as np
import ml_dtypes

import concourse.mybir as mybir
import concourse.tile as tile
from concourse import bacc
from concourse import bass_utils
from concourse.bass import _add_dep_helper

# problem shapes (hardcoded per harness contract)
E = 1024
H = 16
D = 64
B = 2
S = 2048
T = B * S          # 4096 tokens
N_CORES = 8
EPS = 1e-6

BF16 = mybir.dt.bfloat16
F32 = mybir.dt.float32
F8 = mybir.dt.float8e4
NP_F8 = ml_dtypes.float8_e4m3
AF = mybir.ActivationFunctionType
ALU = mybir.AluOpType
DR = mybir.MatmulPerfMode.DoubleRow

# per-core worksizes
TOK = T // N_CORES          # 512 tokens per core for msa/LN
QC = 512                    # attention q-chunk
N_QC = S // QC              # 4 q-chunks per (b, head-pair)
N_KT = S // 128             # 16 k-tiles
N_KP = N_KT // 2            # 8 kt-pairs (DoubleRow AV granularity)
VW = 80                     # padded V row (64 d + 1 ones + pad to 16B mult)
WSC = 32.0                  # fp8 weight pre-scale
S_ACT = 1.0 / (WSC * WSC * 8.0)   # exp affine scale: /32^2 (w-scales) /sqrt(D)


def _build(use_gamma: bool, use_beta: bool):
    nc = bacc.Bacc("TRN2", target_bir_lowering=False, debug=False, num_devices=N_CORES)

    # ---- DRAM parameters -------------------------------------------------
    yT = nc.dram_tensor("yT", [E, T], F8, kind="ExternalInput")
    wqT = nc.dram_tensor("wqT", [E, 128], F8, kind="ExternalInput")
    wkT = nc.dram_tensor("wkT", [E, 128], F8, kind="ExternalInput")
    wvT = nc.dram_tensor("wvT", [E, 128], F8, kind="ExternalInput")
    bq = nc.dram_tensor("bq", [128, 1], F32, kind="ExternalInput")
    bk = nc.dram_tensor("bk", [128, 1], F32, kind="ExternalInput")
    bva = nc.dram_tensor("bva", [64, 1], F32, kind="ExternalInput")
    bvb = nc.dram_tensor("bvb", [64, 1], F32, kind="ExternalInput")
    msa_w = nc.dram_tensor("msa_w", [E, E], F8, kind="ExternalInput")
    y_shard = nc.dram_tensor("y_shard", [TOK, E], F32, kind="ExternalInput")
    if use_gamma:
        gamma_b = nc.dram_tensor("gamma_b", [128, E], F32, kind="ExternalInput")
    if use_beta:
        beta_b = nc.dram_tensor("beta_b", [128, E], F32, kind="ExternalInput")
    out = nc.dram_tensor("out", [TOK, E], F32, kind="ExternalOutput")

    # internal DRAM: A2A bounce buffers (shard k: (b, q-half) -> 128 tok/core)
    a2a_in = [nc.dram_tensor(f"a2a_in{k}", [N_CORES, 128, 128], F8) for k in range(4)]
    a2a_out = [nc.dram_tensor(f"a2a_out{k}", [N_CORES, 128, 128], F8) for k in range(4)]

    with tile.TileContext(nc) as tc:
        with (
            tc.tile_pool(name="const", bufs=1) as cpool,
            tc.tile_pool(name="yt", bufs=5) as ytp,
            tc.tile_pool(name="qk", bufs=1) as qkp,
            tc.tile_pool(name="exp", bufs=3) as expp,
            tc.tile_pool(name="stage", bufs=1) as stp,
            tc.tile_pool(name="fin", bufs=2) as finp,
            tc.tile_pool(name="a2asb", bufs=4) as a2ap,
            tc.tile_pool(name="ps_acc", bufs=2, space="PSUM") as ps_acc,
            tc.tile_pool(name="ps_sc", bufs=2, space="PSUM") as ps_sc,
            tc.tile_pool(name="ps_av", bufs=2, space="PSUM") as ps_av,
        ):
            # ---- constants -------------------------------------------------
            wqT_sb = cpool.tile([128, 8, 128], F8)
            wkT_sb = cpool.tile([128, 8, 128], F8)
            wvT_sb = cpool.tile([128, 8, 128], F8)
            nc.sync.dma_start(wqT_sb[:], wqT[:, :].rearrange("(a p) n -> p a n", p=128))
            nc.sync.dma_start(wkT_sb[:], wkT[:, :].rearrange("(a p) n -> p a n", p=128))
            nc.sync.dma_start(wvT_sb[:], wvT[:, :].rearrange("(a p) n -> p a n", p=128))
            bq_sb = cpool.tile([128, 1], F32)
            bk_sb = cpool.tile([128, 1], F32)
            bva_sb = cpool.tile([64, 1], F32)
            bvb_sb = cpool.tile([64, 1], F32)
            nc.sync.dma_start(bq_sb[:], bq[:, :])
            nc.sync.dma_start(bk_sb[:], bk[:, :])
            nc.sync.dma_start(bva_sb[:], bva[:, :])
            nc.sync.dma_start(bvb_sb[:], bvb[:, :])
            # (msa weights / LN consts are DMA'd later, after the attention
            # loops are issued, so startup DMA bandwidth goes to yT tiles)
            msa_w_sb = cpool.tile([128, 8, E], F8)
            y_shard_sb = cpool.tile([128, 4, E], F32)
            if use_gamma:
                gamma_sb = cpool.tile([128, E], F32)
            if use_beta:
                beta_sb = cpool.tile([128, E], F32)

            # V tiles, padded: [k-part, b, head, kt, VW]; col 64 = ones
            v_sb = cpool.tile([128, B, 2, N_KT, VW], F8)
            nc.vector.memset(v_sb[:, :, :, :, 64:VW], 0.0)
            nc.vector.memset(v_sb[:, :, :, :, 64:65], 1.0)

            # Q^T/K^T: [d-part(2 heads), b, q]
            qT_sb = qkp.tile([128, B, S], BF16)
            kT_sb = qkp.tile([128, B, S], BF16)
            # o^T staging for A2A: [d-part, b, q] (fp8: carries 32*(o+bv))
            stage = stp.tile([128, B, S], F8)

            # ============== main per-batch pipeline ==============
            def make_qkv_steps(b):
                """QKV projection for batch b as a list of small closures so the
                PE work can be interleaved into the other batch's attention
                (fills the in-order PE stream's exp-wait slots)."""
                loads, qs, ks, vs = [], [], [], []
                for tc8 in range(4):
                    st8 = {}

                    def load(b=b, tc8=tc8, st8=st8):
                        yt = ytp.tile([128, 8, 512], F8, tag="yt")
                        st8["yt"] = yt
                        for et in range(8):
                            nc.sync.dma_start(
                                yt[:, et, :], yT[128 * et:128 * (et + 1),
                                                 b * S + 512 * tc8: b * S + 512 * (tc8 + 1)])
                    loads.append(load)
                    qs.append([])
                    ks.append([])
                    vs.append([])

                    # Q/K: 4 DoubleRow matmuls (et-pairs), K=1024 contraction.
                    # DR forbids column tile_position offsets, so each mm is
                    # full-width [128, 2, 128] -> out [128, 512].
                    for eg in range(4):
                        def qstep(b=b, tc8=tc8, eg=eg, st8=st8):
                            if eg == 0:
                                st8["ps_q"] = ps_acc.tile([128, 512], F32, tag="acc", name="ps_q")
                            ps_q, yt = st8["ps_q"], st8["yt"]
                            st, sp = (eg == 0), (eg == 3)
                            nc.tensor.matmul(ps_q[:], wqT_sb[:, 2 * eg:2 * eg + 2, :],
                                             yt[:, 2 * eg:2 * eg + 2, :], start=st, stop=sp,
                                             perf_mode=DR)
                            if eg == 3:
                                nc.vector.tensor_scalar(
                                    qT_sb[:, b, 512 * tc8:512 * (tc8 + 1)], ps_q[:],
                                    bq_sb[:], None, ALU.add)
                        qs[tc8].append(qstep)

                    for eg in range(4):
                        def kstep(b=b, tc8=tc8, eg=eg, st8=st8):
                            if eg == 0:
                                st8["ps_k"] = ps_acc.tile([128, 512], F32, tag="acc", name="ps_k")
                            ps_k, yt = st8["ps_k"], st8["yt"]
                            st, sp = (eg == 0), (eg == 3)
                            nc.tensor.matmul(ps_k[:], wkT_sb[:, 2 * eg:2 * eg + 2, :],
                                             yt[:, 2 * eg:2 * eg + 2, :], start=st, stop=sp,
                                             perf_mode=DR)
                            if eg == 3:
                                nc.vector.tensor_scalar(
                                    kT_sb[:, b, 512 * tc8:512 * (tc8 + 1)], ps_k[:],
                                    bk_sb[:], None, ALU.add)
                        ks[tc8].append(kstep)

                    # V: [tok, vdim] layout, fp8 operands (no DoubleRow: the
                    # stationary operand changes every matmul)
                    for eg in range(4):
                        def vstep(b=b, tc8=tc8, eg=eg, st8=st8):
                            if eg == 0:
                                st8["ps_v"] = ps_acc.tile([128, 512], F32, tag="acc", name="ps_v")
                            ps_v, yt = st8["ps_v"], st8["yt"]
                            for et in (2 * eg, 2 * eg + 1):
                                st, sp = (et == 0), (et == 7)
                                for s4 in range(4):
                                    nc.tensor.matmul(ps_v[:, 128 * s4:128 * (s4 + 1)],
                                                     yt[:, et, 128 * s4:128 * (s4 + 1)],
                                                     wvT_sb[:, et, :], start=st, stop=sp)
                            if eg == 3:
                                for h in range(2):
                                    src = ps_v[:, :].rearrange(
                                        "p (s n) -> p s n", s=4)[:, :, 64 * h:64 * (h + 1)]
                                    nc.vector.tensor_copy(
                                        v_sb[:, b, h, 4 * tc8:4 * (tc8 + 1), 0:64], src)
                        vs[tc8].append(vstep)
                return loads, qs, ks, vs

            trigs = []  # collective trigger instrs, k-order

            def attention(b, bg, av_last=None, qcs=range(N_QC), pops=2):
                # software-pipelined ACROSS kt steps: qk/exp runs OV steps
                # ahead of av, so the ACT engine never drains at qc
                # boundaries; bg closures (other QKV work) fill PE wait slots.
                # AV runs per kt-PAIR in fp8 DoubleRow mode.
                if av_last is None:
                    av_last = []
                OV = 4
                states = {}

                def qk_exp(qc, kt):
                    stq = states[qc]
                    if kt % 2 == 0:
                        stq["exs"][kt // 2] = expp.tile([128, 2, 1024], F8, name="ex")
                    sc = ps_sc.tile([128, 1024], F32, tag="sc", name="sc")
                    nc.tensor.matmul(sc[:, 0:512],
                                     kT_sb[0:64, b, 128 * kt:128 * (kt + 1)],
                                     qT_sb[0:64, b, QC * qc:QC * (qc + 1)],
                                     start=True, stop=True, tile_position=(0, 0))
                    nc.tensor.matmul(sc[:, 512:1024],
                                     kT_sb[64:128, b, 128 * kt:128 * (kt + 1)],
                                     qT_sb[64:128, b, QC * qc:QC * (qc + 1)],
                                     start=True, stop=True, tile_position=(64, 0))
                    ex = stq["exs"][kt // 2]
                    nc.scalar.activation(ex[:, kt % 2, :], sc[:], AF.Exp, scale=S_ACT)

                def av_a(qc, kp):
                    stq = states[qc]
                    if kp == 0:
                        stq["av_a"] = ps_av.tile([128, 512], F32, tag="av", name="av_a")
                        stq["av_b"] = ps_av.tile([128, 512], F32, tag="av", name="av_b")
                    ex = stq["exs"][kp]
                    nc.tensor.matmul(stq["av_a"][0:65, :],
                                     v_sb[:, b, 0, 2 * kp:2 * kp + 2, 0:65],
                                     ex[:, :, 0:512],
                                     start=(kp == 0), stop=(kp == N_KP - 1), perf_mode=DR)

                def av_b(qc, kp):
                    stq = states[qc]
                    ex = stq["exs"][kp]
                    i2 = nc.tensor.matmul(stq["av_b"][0:65, :],
                                          v_sb[:, b, 1, 2 * kp:2 * kp + 2, 0:65],
                                          ex[:, :, 512:1024],
                                          start=(kp == 0), stop=(kp == N_KP - 1), perf_mode=DR)
                    if kp == N_KP - 1:
                        av_last.append(i2)

                def finalize(qc):
                    av_a, av_b = states[qc]["av_a"], states[qc]["av_b"]
                    # drain AV psum to SBUF fast (releases psum for next q-chunk)
                    af = finp.tile([128, 1024], F32, tag="af", name="af")
                    nc.vector.tensor_copy(af[0:65, 0:512], av_a[0:65, :])
                    nc.vector.tensor_copy(af[0:65, 512:1024], av_b[0:65, :])
                    # denominator row -> partition 0 (DMA shifts partitions),
                    # fast reciprocal there, then gpsimd broadcast to all lanes
                    rc = finp.tile([128, 1024], F32, tag="rc", name="rc")
                    nc.gpsimd.dma_start(rc[0:1, :], af[64:65, :])
                    rc2 = finp.tile([128, 1024], F32, tag="rc2", name="rc2")
                    nc.vector.reciprocal_approx_fast(rc2[0:1, :], rc[0:1, :])
                    rb = finp.tile([128, 1024], F32, tag="rb", name="rb")
                    nc.gpsimd.partition_broadcast(rb[:, :], rc2[0:1, :])
                    # o^T = o_raw^T * recip + bv; all on partitions 0..63, then
                    # head B is partition-shifted into the stage via DMA.
                    # (fp8 tiles are write-only for the DVE: mult lands in an
                    # f32 scratch, the bias-add writes the fp8 copy once)
                    osc = stage[:, b, QC * qc:QC * (qc + 1)]
                    om = finp.tile([64, 1024], F32, tag="om", name="om")
                    nc.vector.tensor_tensor(om[:, 0:512], af[0:64, 0:512], rb[0:64, 0:512], ALU.mult)
                    nc.vector.tensor_scalar(om[:, 0:512], om[:, 0:512], bva_sb[:], None, ALU.add)
                    nc.vector.tensor_copy(osc[0:64, :], om[:, 0:512])
                    tb = finp.tile([64, 512], F8, tag="tb", name="tb")
                    nc.vector.tensor_tensor(om[:, 512:1024], af[0:64, 512:1024], rb[0:64, 512:1024], ALU.mult)
                    nc.vector.tensor_scalar(om[:, 512:1024], om[:, 512:1024], bvb_sb[:], None, ALU.add)
                    nc.vector.tensor_copy(tb[:], om[:, 512:1024])
                    nc.gpsimd.dma_start(osc[64:128, :], tb[:])
                    # A2A per q-half: upload each qc's blocks as soon as
                    # staged; issue the collective after the odd qc
                    hf = qc // 2
                    k = 2 * b + hf
                    half = a2a_in[k].ap().rearrange("j p n -> p j n")
                    if qc % 2 == 0:
                        nc.gpsimd.dma_start(
                            half[:, 0:4, :],
                            stage[:, b, 1024 * hf:1024 * hf + 512].rearrange(
                                "p (j n) -> p j n", j=4))
                    else:
                        nc.gpsimd.dma_start(
                            half[:, 4:8, :],
                            stage[:, b, 1024 * hf + 512:1024 * (hf + 1)].rearrange(
                                "p (j n) -> p j n", j=4))
                        trigs.append(nc.gpsimd.collective_compute(
                            "AllToAll", ALU.bypass,
                            replica_groups=[list(range(N_CORES))],
                            ins=[a2a_in[k].ap().opt()],
                            outs=[a2a_out[k].ap().opt()],
                        ))

                seq = [(qc, kt) for qc in qcs for kt in range(N_KT)]
                for i, (qc, kt) in enumerate(seq):
                    states.setdefault(qc, {"exs": [None] * N_KP})
                    qk_exp(qc, kt)
                    for _ in range(pops):
                        if bg:
                            bg.pop(0)()
                    j = i - OV
                    if j >= 0 and seq[j][1] % 2 == 1:
                        jqc, jkt = seq[j]
                        av_a(jqc, jkt // 2)
                        av_b(jqc, jkt // 2)
                        if jkt == N_KT - 1:
                            finalize(jqc)
                for j in range(max(0, len(seq) - OV), len(seq)):
                    if seq[j][1] % 2 == 1:
                        jqc, jkt = seq[j]
                        av_a(jqc, jkt // 2)
                        av_b(jqc, jkt // 2)
                        if jkt == N_KT - 1:
                            finalize(jqc)
                return av_last

            # drive: emit only chunk 0 of b0's QKV up front, then start
            # attention qc0 with chunks 1-3 interleaved as background steps
            # (order [K,V] per chunk matches the kt windows that consume them);
            # b1's QKV interleaves into b0's qc1-3.
            l0, q0, k0, v0 = make_qkv_steps(0)
            for step in l0:
                step()
            for s in k0[0]:
                s()
            for s in q0[0]:
                s()
            for s in v0[0]:
                s()
            bgA = []
            for tc8 in (1, 2, 3):
                bgA.extend(k0[tc8])
                bgA.extend(v0[tc8])
            attention(0, bgA, qcs=[0])
            while bgA:
                bgA.pop(0)()
            for tc8 in (1, 2, 3):
                for s in q0[tc8]:
                    s()
            l1, q1, k1, v1 = make_qkv_steps(1)
            bg1 = []
            for tc8 in range(4):
                bg1.append(l1[tc8])
                bg1.extend(q1[tc8])
                bg1.extend(k1[tc8])
                bg1.extend(v1[tc8])
            attention(0, bg1, qcs=[1, 2, 3], pops=1)
            while bg1:
                bg1.pop(0)()
            av_anchors = attention(1, [])

            # deferred bulk const loads (issued after attention DMAs in queue order)
            nc.sync.dma_start(msa_w_sb[:], msa_w[:, :].rearrange("(a p) n -> p a n", p=128))
            nc.sync.dma_start(y_shard_sb[:], y_shard[:, :].rearrange("(a p) n -> p a n", p=128))
            if use_gamma:
                nc.sync.dma_start(gamma_sb[:], gamma_b[:, :])
            if use_beta:
                nc.sync.dma_start(beta_sb[:], beta_b[:, :])

            # ============== msa + residual + LayerNorm per shard ==============
            sum_sb = cpool.tile([128, 8], F32)
            ssq_sb = cpool.tile([128, 8], F32)
            res_sb = stp.tile([128, 4, E], F32)
            inv_e = 1.0 / E
            for k in range(4):
                lhs = a2ap.tile([128, 8, 128], F8, tag="lhs")
                nc.sync.dma_start(lhs[:], a2a_out[k].ap().rearrange("j p n -> p j n"))
                # i-major so consecutive matmuls share lhs weights (LDW dedup);
                # both e-halves accumulate concurrently in two psum tiles.
                # fp8 DoubleRow: i-pairs, contraction 1024 in 4 passes.
                ps_m0 = ps_acc.tile([128, 512], F32, tag="acc", name="ps_m0")
                ps_m1 = ps_acc.tile([128, 512], F32, tag="acc", name="ps_m1")
                for i in range(4):
                    for ec, ps_m in ((0, ps_m0), (1, ps_m1)):
                        mi = nc.tensor.matmul(ps_m[:], lhs[:, 2 * i:2 * i + 2, :],
                                              msa_w_sb[:, 2 * i:2 * i + 2,
                                                       512 * ec:512 * (ec + 1)],
                                              start=(i == 0), stop=(i == 3),
                                              perf_mode=DR)
                        if ec == 0 and i == 0:
                            # keep msa out of the PE stream until b1 attention
                            # has progressed past qc k+1 (the A2A data won't be
                            # there earlier; an early msa blocks the in-order PE)
                            _add_dep_helper(
                                mi.ins, av_anchors[min(k + 1, 3)].ins, sync=False,
                                reason="msa gated behind b1 attention progress")
                for ec, ps_m in ((0, ps_m0), (1, ps_m1)):
                    # residual add + per-half stats, all on DVE (no ACT table switch)
                    rhalf = res_sb[:, k, 512 * ec:512 * (ec + 1)]
                    ri = nc.vector.tensor_tensor(
                        rhalf, ps_m[:],
                        y_shard_sb[:, k, 512 * ec:512 * (ec + 1)], ALU.add)
                    if ec == 0:
                        # keep this shard's LN work behind the (k+1)-th
                        # collective TRIGGER on the DVE queue: the trigger path
                        # of the last q-chunk must not queue behind LN ops
                        _add_dep_helper(
                            ri.ins, trigs[min(k + 1, 3)].ins, sync=False,
                            reason="LN deprioritized behind collective trigger")
                    dump = finp.tile([128, 512], F32, tag="dump")
                    dump2 = finp.tile([128, 512], BF16, tag="dump2")
                    nc.vector.tensor_scalar(dump2[:], rhalf, 0.0, 0.0, ALU.add,
                                            ALU.add, accum_out=sum_sb[:, 2 * k + ec:2 * k + ec + 1])
                    nc.vector.tensor_tensor(dump[:], rhalf, rhalf, ALU.mult)
                    nc.vector.tensor_scalar(dump2[:], dump[:], 0.0, 0.0, ALU.add,
                                            ALU.add, accum_out=ssq_sb[:, 2 * k + ec:2 * k + ec + 1])
                # per-shard LN scalars [128, 1]
                mu = cpool.tile([128, 4], F32, name=f"mu{k}")
                nc.vector.tensor_tensor(mu[:, 0:1], sum_sb[:, 2 * k:2 * k + 1],
                                        sum_sb[:, 2 * k + 1:2 * k + 2], ALU.add)
                nc.vector.tensor_scalar(mu[:, 0:1], mu[:, 0:1], inv_e, None, ALU.mult)
                nc.vector.tensor_tensor(mu[:, 1:2], ssq_sb[:, 2 * k:2 * k + 1],
                                        ssq_sb[:, 2 * k + 1:2 * k + 2], ALU.add)
                nc.vector.tensor_scalar(mu[:, 1:2], mu[:, 1:2], inv_e, None, ALU.mult)
                nc.vector.tensor_tensor(mu[:, 2:3], mu[:, 0:1], mu[:, 0:1], ALU.mult)
                nc.vector.tensor_tensor(mu[:, 1:2], mu[:, 1:2], mu[:, 2:3], ALU.subtract)
                nc.vector.tensor_scalar(mu[:, 1:2], mu[:, 1:2], EPS, None, ALU.add)
                # rstd = sqrt(1/var) + one Newton polish (keeps msa-era ACT
                # in the sqrt table set only -> a single table load, no thrash)
                nc.vector.reciprocal_approx_fast(mu[:, 2:3], mu[:, 1:2])
                nc.scalar.activation(mu[:, 3:4], mu[:, 2:3], AF.Sqrt)
                nc.vector.tensor_tensor(mu[:, 2:3], mu[:, 3:4], mu[:, 3:4], ALU.mult)
                nc.vector.tensor_tensor(mu[:, 2:3], mu[:, 2:3], mu[:, 1:2], ALU.mult)
                nc.vector.tensor_scalar(mu[:, 2:3], mu[:, 2:3], -0.5, 1.5, ALU.mult, ALU.add)
                nc.vector.tensor_tensor(mu[:, 3:4], mu[:, 3:4], mu[:, 2:3], ALU.mult)
                # -mu * rstd
                nc.vector.tensor_scalar(mu[:, 0:1], mu[:, 0:1], -1.0, None, ALU.mult)
                nc.vector.tensor_tensor(mu[:, 0:1], mu[:, 0:1], mu[:, 3:4], ALU.mult)
                # normalize on DVE (fused scale+bias) so ACT runs only Ln/Exp
                # in this phase -- avoids per-shard activation-table reloads
                o1 = finp.tile([128, E], F32, tag="o1")
                nc.vector.tensor_scalar(o1[:], res_sb[:, k, :], mu[:, 3:4],
                                        mu[:, 0:1], ALU.mult, ALU.add)
                if use_gamma:
                    nc.vector.tensor_tensor(o1[:], o1[:], gamma_sb[:], ALU.mult)
                if use_beta:
                    nc.vector.tensor_tensor(o1[:], o1[:], beta_sb[:], ALU.add)
                nc.sync.dma_start(out[128 * k:128 * (k + 1), :], o1[:])

    nc.compile()
    return nc


@functools.lru_cache(maxsize=4)
def _get_nc(use_gamma: bool, use_beta: bool):
    return _build(use_gamma, use_beta)


def kernel(**inputs) -> np.ndarray:
    y = np.asarray(inputs["y"], np.float32)
    Wqkv = np.asarray(inputs["Wqkv"], np.float32)
    bqkv = np.asarray(inputs["bqkv"], np.float32)
    Wmsa = np.asarray(inputs["Wmsa"], np.float32)
    Bq_, Aq_ = np.asarray(inputs["Bq"], np.float32), np.asarray(inputs["Aq"], np.float32)
    Bk_, Ak_ = np.asarray(inputs["Bk"], np.float32), np.asarray(inputs["Ak"], np.float32)
    Bv_, Av_ = np.asarray(inputs["Bv"], np.float32), np.asarray(inputs["Av"], np.float32)
    Bo_, Ao_ = np.asarray(inputs["Bo"], np.float32), np.asarray(inputs["Ao"], np.float32)
    gamma = np.asarray(inputs["gamma"], np.float32)
    beta = np.asarray(inputs["beta"], np.float32)

    # effective weights: qkv = y @ (Wqkv.T + blockdiag-ish LoRA) + bqkv
    # y @ W.T: W rows are output dims. LoRA adds y @ (B@A): effective W += (B@A).T
    W_eff = Wqkv.copy()
    W_eff[0:E] += (Bq_ @ Aq_).T
    W_eff[E:2 * E] += (Bk_ @ Ak_).T
    W_eff[2 * E:3 * E] += (Bv_ @ Av_).T
    # fp8 pre-scale: weights x32 (1/sqrt(D) and the scale unwind live in
    # the exp affine scale and the /32 on the msa weights)
    W_eff *= WSC
    bq_eff = bqkv[0:E] * WSC
    bk_eff = bqkv[E:2 * E] * WSC
    bv_eff = bqkv[2 * E:3 * E] * WSC
    # msa: o @ Wmsa.T + o @ (Bo@Ao) = o @ M with M = Wmsa.T + Bo@Ao  [E(d), E(out)].
    # stage carries 32*(o+bv); store 32*M in fp8 (good dynamic range) so the
    # msa psum is 1024*msa, and scale the residual y by 1024 to match --
    # LayerNorm is scale-invariant, so the final output is unchanged.
    M = (Wmsa.T + Bo_ @ Ao_) * WSC

    y_flat = y.reshape(T, E)
    yT_f8 = np.ascontiguousarray(y_flat.T).astype(NP_F8)
    M_f8 = np.ascontiguousarray(M).astype(NP_F8)

    use_gamma = not np.allclose(gamma, 1.0)
    use_beta = not np.allclose(beta, 0.0)
    nc = _get_nc(use_gamma, use_beta)

    in_maps = []
    for c in range(N_CORES):
        r0 = c * 128
        r1 = r0 + 128
        wq_c = np.ascontiguousarray(W_eff[0:E][r0:r1].T).astype(NP_F8)
        wk_c = np.ascontiguousarray(W_eff[E:2 * E][r0:r1].T).astype(NP_F8)
        wv_c = np.ascontiguousarray(W_eff[2 * E:3 * E][r0:r1].T).astype(NP_F8)
        tok = np.concatenate([
            np.arange(128 * c, 128 * c + 128),
            np.arange(1024 + 128 * c, 1024 + 128 * c + 128),
            np.arange(2048 + 128 * c, 2048 + 128 * c + 128),
            np.arange(3072 + 128 * c, 3072 + 128 * c + 128),
        ])
        m = {
            "yT": yT_f8,
            "wqT": wq_c,
            "wkT": wk_c,
            "wvT": wv_c,
            "bq": bq_eff[r0:r1].reshape(128, 1).copy(),
            "bk": bk_eff[r0:r1].reshape(128, 1).copy(),
            "bva": bv_eff[r0:r0 + 64].reshape(64, 1).copy(),
            "bvb": bv_eff[r0 + 64:r1].reshape(64, 1).copy(),
            "msa_w": M_f8,
            "y_shard": np.ascontiguousarray(y_flat[tok]) * (WSC * WSC),
        }
        if use_gamma:
            m["gamma_b"] = np.broadcast_to(gamma, (128, E)).copy()
        if use_beta:
            m["beta_b"] = np.broadcast_to(beta, (128, E)).copy()
        in_maps.append(m)

    res = bass_utils.run_bass_kernel_spmd(nc, in_maps, core_ids=list(range(N_CORES)))

    out_full = np.empty((T, E), np.float32)
    for c in range(N_CORES):
        oc = res.results[c]["out"]
        out_full[128 * c:128 * c + 128] = oc[0:128]
        out_full[1024 + 128 * c:1024 + 128 * c + 128] = oc[128:256]
        out_full[2048 + 128 * c:2048 + 128 * c + 128] = oc[256:384]
        out_full[3072 + 128 * c:3072 + 128 * c + 128] = oc[384:512]
    return out_full.reshape(B, S, E)


# revision 21
# speedup vs baseline: 1.1500x; 1.0139x over previous
"""Trainium2 Bass kernel for fused LoRA-attention block (nn_Attention_18846316494887).

Reference computation:
  qkv = y @ Wqkv.T + bqkv (+ LoRA deltas y @ (B@A) per Q/K/V)  -> Q,K,V [B,H,S,D]
  attn = softmax(Q K^T / sqrt(D)); o = attn @ V -> [B,S,E]
  msa = o @ Wmsa.T + o @ (Bo@Ao); res = msa + y; out = LayerNorm(res)*gamma + beta

Sharding: tensor-parallel over heads (2 heads/core, 8 cores), AllToAll to
reshard head-dim -> token-dim before the output projection, token-parallel
msa + LayerNorm, host-side gather of per-core token shards.

Precision plan (error budget: attention path contributes only ~2.2% of the
LN'd output norm, so a few-% relative error there is invisible):
  - y, Wqkv (x32), V, exp(scores) all in fp8e4m3; f32 PSUM accumulation
  - Q/K projection matmuls in DoubleRow mode (2 fp8 k-subtiles per pass)
  - AV matmuls in DoubleRow mode over kt-pairs (halves the ex stream time)
  - the x32*x32 weight scaling and 1/sqrt(D) fold into the exp's free
    affine scale (exp(x * 1/8192)); V-scale folds into msa weights (/32)

Host-side prep (exact algebra, no approximation):
  - LoRA folded into Wqkv / Wmsa (y@W.T + y@(B@A) == y@(W.T + B@A))
  - V bias applied post-softmax on o (exact since attn rows sum to 1)
  - y pre-transposed to [E, T] for the QKV matmuls
"""
import functools
import numpy as np
import ml_dtypes

import concourse.mybir as mybir
import concourse.tile as tile
from concourse import bacc
from concourse import bass_utils
from concourse.bass import _add_dep_helper

# problem shapes (hardcoded per harness contract)
E = 1024
H = 16
D = 64
B = 2
S = 2048
T = B * S          # 4096 tokens
N_CORES = 8
EPS = 1e-6

BF16 = mybir.dt.bfloat16
F32 = mybir.dt.float32
F8 = mybir.dt.float8e4
NP_F8 = ml_dtypes.float8_e4m3
AF = mybir.ActivationFunctionType
ALU = mybir.AluOpType
DR = mybir.MatmulPerfMode.DoubleRow

# per-core worksizes
TOK = T // N_CORES          # 512 tokens per core for msa/LN
QC = 512                    # attention q-chunk
N_QC = S // QC              # 4 q-chunks per (b, head-pair)
N_KT = S // 128             # 16 k-tiles
N_KP = N_KT // 2            # 8 kt-pairs (DoubleRow AV granularity)
VW = 80                     # padded V row (64 d + 1 ones + pad to 16B mult)
WSC = 32.0                  # fp8 weight pre-scale
S_ACT = 1.0 / (WSC * WSC * 8.0)   # exp affine scale: /32^2 (w-scales) /sqrt(D)


def _build(use_gamma: bool, use_beta: bool):
    nc = bacc.Bacc("TRN2", target_bir_lowering=False, debug=False, num_devices=N_CORES)

    # ---- DRAM parameters -------------------------------------------------
    yT = nc.dram_tensor("yT", [E, T], F8, kind="ExternalInput")
    wqT = nc.dram_tensor("wqT", [E, 128], F8, kind="ExternalInput")
    wkT = nc.dram_tensor("wkT", [E, 128], F8, kind="ExternalInput")
    wvT = nc.dram_tensor("wvT", [E, 128], F8, kind="ExternalInput")
    bq = nc.dram_tensor("bq", [128, 1], F32, kind="ExternalInput")
    bk = nc.dram_tensor("bk", [128, 1], F32, kind="ExternalInput")
    bva = nc.dram_tensor("bva", [64, 1], F32, kind="ExternalInput")
    bvb = nc.dram_tensor("bvb", [64, 1], F32, kind="ExternalInput")
    msa_w = nc.dram_tensor("msa_w", [E, E], F8, kind="ExternalInput")
    y_shard = nc.dram_tensor("y_shard", [TOK, E], F32, kind="ExternalInput")
    if use_gamma:
        gamma_b = nc.dram_tensor("gamma_b", [128, E], F32, kind="ExternalInput")
    if use_beta:
        beta_b = nc.dram_tensor("beta_b", [128, E], F32, kind="ExternalInput")
    out = nc.dram_tensor("out", [TOK, E], F32, kind="ExternalOutput")

    # internal DRAM: A2A bounce buffers (shard k: (b, q-half) -> 128 tok/core)
    a2a_in = [nc.dram_tensor(f"a2a_in{k}", [N_CORES, 128, 128], F8) for k in range(4)]
    a2a_out = [nc.dram_tensor(f"a2a_out{k}", [N_CORES, 128, 128], F8) for k in range(4)]

    with tile.TileContext(nc) as tc:
        with (
            tc.tile_pool(name="const", bufs=1) as cpool,
            tc.tile_pool(name="yt", bufs=5) as ytp,
            tc.tile_pool(name="qk", bufs=1) as qkp,
            tc.tile_pool(name="exp", bufs=3) as expp,
            tc.tile_pool(name="stage", bufs=1) as stp,
            tc.tile_pool(name="fin", bufs=2) as finp,
            tc.tile_pool(name="a2asb", bufs=4) as a2ap,
            tc.tile_pool(name="ps_acc", bufs=2, space="PSUM") as ps_acc,
            tc.tile_pool(name="ps_sc", bufs=2, space="PSUM") as ps_sc,
            tc.tile_pool(name="ps_av", bufs=2, space="PSUM") as ps_av,
        ):
            # ---- constants -------------------------------------------------
            wqT_sb = cpool.tile([128, 8, 128], F8)
            wkT_sb = cpool.tile([128, 8, 128], F8)
            wvT_sb = cpool.tile([128, 8, 128], F8)
            nc.sync.dma_start(wqT_sb[:], wqT[:, :].rearrange("(a p) n -> p a n", p=128))
            nc.sync.dma_start(wkT_sb[:], wkT[:, :].rearrange("(a p) n -> p a n", p=128))
            nc.sync.dma_start(wvT_sb[:], wvT[:, :].rearrange("(a p) n -> p a n", p=128))
            bq_sb = cpool.tile([128, 1], F32)
            bk_sb = cpool.tile([128, 1], F32)
            bva_sb = cpool.tile([64, 1], F32)
            bvb_sb = cpool.tile([64, 1], F32)
            nc.sync.dma_start(bq_sb[:], bq[:, :])
            nc.sync.dma_start(bk_sb[:], bk[:, :])
            nc.sync.dma_start(bva_sb[:], bva[:, :])
            nc.sync.dma_start(bvb_sb[:], bvb[:, :])
            # (msa weights / LN consts are DMA'd later, after the attention
            # loops are issued, so startup DMA bandwidth goes to yT tiles)
            msa_w_sb = cpool.tile([128, 8, E], F8)
            y_shard_sb = cpool.tile([128, 4, E], F32)
            if use_gamma:
                gamma_sb = cpool.tile([128, E], F32)
            if use_beta:
                beta_sb = cpool.tile([128, E], F32)

            # V tiles, padded: [k-part, b, head, kt, VW]; col 64 = ones
            v_sb = cpool.tile([128, B, 2, N_KT, VW], F8)
            nc.vector.memset(v_sb[:, :, :, :, 64:VW], 0.0)
            nc.vector.memset(v_sb[:, :, :, :, 64:65], 1.0)

            # Q^T/K^T: [d-part(2 heads), b, q]
            qT_sb = qkp.tile([128, B, S], BF16)
            kT_sb = qkp.tile([128, B, S], BF16)
            # o^T staging for A2A: [d-part, b, q] (fp8: carries 32*(o+bv))
            stage = stp.tile([128, B, S], F8)

            # ============== main per-batch pipeline ==============
            def make_qkv_steps(b):
                """QKV projection for batch b as a list of small closures so the
                PE work can be interleaved into the other batch's attention
                (fills the in-order PE stream's exp-wait slots)."""
                loads, qs, ks, vs = [], [], [], []
                for tc8 in range(4):
                    st8 = {}

                    def load(b=b, tc8=tc8, st8=st8):
                        yt = ytp.tile([128, 8, 512], F8, tag="yt")
                        st8["yt"] = yt
                        for et in range(8):
                            nc.sync.dma_start(
                                yt[:, et, :], yT[128 * et:128 * (et + 1),
                                                 b * S + 512 * tc8: b * S + 512 * (tc8 + 1)])
                    loads.append(load)
                    qs.append([])
                    ks.append([])
                    vs.append([])

                    # Q/K: 4 DoubleRow matmuls (et-pairs), K=1024 contraction.
                    # DR forbids column tile_position offsets, so each mm is
                    # full-width [128, 2, 128] -> out [128, 512].
                    for eg in range(4):
                        def qstep(b=b, tc8=tc8, eg=eg, st8=st8):
                            if eg == 0:
                                st8["ps_q"] = ps_acc.tile([128, 512], F32, tag="acc", name="ps_q")
                            ps_q, yt = st8["ps_q"], st8["yt"]
                            st, sp = (eg == 0), (eg == 3)
                            nc.tensor.matmul(ps_q[:], wqT_sb[:, 2 * eg:2 * eg + 2, :],
                                             yt[:, 2 * eg:2 * eg + 2, :], start=st, stop=sp,
                                             perf_mode=DR)
                            if eg == 3:
                                nc.vector.tensor_scalar(
                                    qT_sb[:, b, 512 * tc8:512 * (tc8 + 1)], ps_q[:],
                                    bq_sb[:], None, ALU.add)
                        qs[tc8].append(qstep)

                    for eg in range(4):
                        def kstep(b=b, tc8=tc8, eg=eg, st8=st8):
                            if eg == 0:
                                st8["ps_k"] = ps_acc.tile([128, 512], F32, tag="acc", name="ps_k")
                            ps_k, yt = st8["ps_k"], st8["yt"]
                            st, sp = (eg == 0), (eg == 3)
                            nc.tensor.matmul(ps_k[:], wkT_sb[:, 2 * eg:2 * eg + 2, :],
                                             yt[:, 2 * eg:2 * eg + 2, :], start=st, stop=sp,
                                             perf_mode=DR)
                            if eg == 3:
                                nc.vector.tensor_scalar(
                                    kT_sb[:, b, 512 * tc8:512 * (tc8 + 1)], ps_k[:],
                                    bk_sb[:], None, ALU.add)
                        ks[tc8].append(kstep)

                    # V: [tok, vdim] layout, fp8 operands (no DoubleRow: the
                    # stationary operand changes every matmul)
                    for eg in range(4):
                        def vstep(b=b, tc8=tc8, eg=eg, st8=st8):
                            if eg == 0:
                                st8["ps_v"] = ps_acc.tile([128, 512], F32, tag="acc", name="ps_v")
                            ps_v, yt = st8["ps_v"], st8["yt"]
                            for et in (2 * eg, 2 * eg + 1):
                                st, sp = (et == 0), (et == 7)
                                for s4 in range(4):
                                    nc.tensor.matmul(ps_v[:, 128 * s4:128 * (s4 + 1)],
                                                     yt[:, et, 128 * s4:128 * (s4 + 1)],
                                                     wvT_sb[:, et, :], start=st, stop=sp)
                            if eg == 3:
                                for h in range(2):
                                    src = ps_v[:, :].rearrange(
                                        "p (s n) -> p s n", s=4)[:, :, 64 * h:64 * (h + 1)]
                                    nc.vector.tensor_copy(
                                        v_sb[:, b, h, 4 * tc8:4 * (tc8 + 1), 0:64], src)
                        vs[tc8].append(vstep)
                return loads, qs, ks, vs

            trigs = []  # collective trigger instrs, k-order

            def attention(b, bg, av_last=None, qcs=range(N_QC), pops=2):
                # software-pipelined ACROSS kt steps: qk/exp runs OV steps
                # ahead of av, so the ACT engine never drains at qc
                # boundaries; bg closures (other QKV work) fill PE wait slots.
                # AV runs per kt-PAIR in fp8 DoubleRow mode.
                if av_last is None:
                    av_last = []
                OV = 4
                states = {}

                def qk_exp(qc, kt):
                    stq = states[qc]
                    if kt % 2 == 0:
                        stq["exs"][kt // 2] = expp.tile([128, 2, 1024], F8, name="ex")
                    sc = ps_sc.tile([128, 1024], F32, tag="sc", name="sc")
                    nc.tensor.matmul(sc[:, 0:512],
                                     kT_sb[0:64, b, 128 * kt:128 * (kt + 1)],
                                     qT_sb[0:64, b, QC * qc:QC * (qc + 1)],
                                     start=True, stop=True, tile_position=(0, 0))
                    nc.tensor.matmul(sc[:, 512:1024],
                                     kT_sb[64:128, b, 128 * kt:128 * (kt + 1)],
                                     qT_sb[64:128, b, QC * qc:QC * (qc + 1)],
                                     start=True, stop=True, tile_position=(64, 0))
                    ex = stq["exs"][kt // 2]
                    nc.scalar.activation(ex[:, kt % 2, :], sc[:], AF.Exp, scale=S_ACT)

                def av_a(qc, kp):
                    stq = states[qc]
                    if kp == 0:
                        stq["av_a"] = ps_av.tile([128, 512], F32, tag="av", name="av_a")
                        stq["av_b"] = ps_av.tile([128, 512], F32, tag="av", name="av_b")
                    ex = stq["exs"][kp]
                    nc.tensor.matmul(stq["av_a"][0:65, :],
                                     v_sb[:, b, 0, 2 * kp:2 * kp + 2, 0:65],
                                     ex[:, :, 0:512],
                                     start=(kp == 0), stop=(kp == N_KP - 1), perf_mode=DR)

                def av_b(qc, kp):
                    stq = states[qc]
                    ex = stq["exs"][kp]
                    i2 = nc.tensor.matmul(stq["av_b"][0:65, :],
                                          v_sb[:, b, 1, 2 * kp:2 * kp + 2, 0:65],
                                          ex[:, :, 512:1024],
                                          start=(kp == 0), stop=(kp == N_KP - 1), perf_mode=DR)
                    if kp == N_KP - 1:
                        av_last.append(i2)

                def finalize(qc):
                    av_a, av_b = states[qc]["av_a"], states[qc]["av_b"]
                    # drain AV psum to SBUF fast (releases psum for next q-chunk)
                    af = finp.tile([128, 1024], F32, tag="af", name="af")
                    nc.vector.tensor_copy(af[0:65, 0:512], av_a[0:65, :])
                    nc.vector.tensor_copy(af[0:65, 512:1024], av_b[0:65, :])
                    # denominator row -> partition 0 (DMA shifts partitions),
                    # fast reciprocal there, then gpsimd broadcast to all lanes
                    rc = finp.tile([128, 1024], F32, tag="rc", name="rc")
                    nc.gpsimd.dma_start(rc[0:1, :], af[64:65, :])
                    rc2 = finp.tile([128, 1024], F32, tag="rc2", name="rc2")
                    nc.vector.reciprocal_approx_fast(rc2[0:1, :], rc[0:1, :])
                    rb = finp.tile([128, 1024], F32, tag="rb", name="rb")
                    nc.gpsimd.partition_broadcast(rb[:, :], rc2[0:1, :])
                    # o^T = o_raw^T * recip + bv; all on partitions 0..63, then
                    # head B is partition-shifted into the stage via DMA.
                    # (fp8 tiles are write-only for the DVE: mult lands in an
                    # f32 scratch, the bias-add writes the fp8 copy once)
                    osc = stage[:, b, QC * qc:QC * (qc + 1)]
                    om = finp.tile([64, 1024], F32, tag="om", name="om")
                    nc.vector.tensor_tensor(om[:, 0:512], af[0:64, 0:512], rb[0:64, 0:512], ALU.mult)
                    nc.vector.tensor_scalar(om[:, 0:512], om[:, 0:512], bva_sb[:], None, ALU.add)
                    nc.vector.tensor_copy(osc[0:64, :], om[:, 0:512])
                    tb = finp.tile([64, 512], F8, tag="tb", name="tb")
                    nc.vector.tensor_tensor(om[:, 512:1024], af[0:64, 512:1024], rb[0:64, 512:1024], ALU.mult)
                    nc.vector.tensor_scalar(om[:, 512:1024], om[:, 512:1024], bvb_sb[:], None, ALU.add)
                    nc.vector.tensor_copy(tb[:], om[:, 512:1024])
                    nc.gpsimd.dma_start(osc[64:128, :], tb[:])
                    # A2A per q-half: upload each qc's blocks as soon as
                    # staged; issue the collective after the odd qc
                    hf = qc // 2
                    k = 2 * b + hf
                    half = a2a_in[k].ap().rearrange("j p n -> p j n")
                    if qc % 2 == 0:
                        nc.gpsimd.dma_start(
                            half[:, 0:4, :],
                            stage[:, b, 1024 * hf:1024 * hf + 512].rearrange(
                                "p (j n) -> p j n", j=4))
                    else:
                        nc.gpsimd.dma_start(
                            half[:, 4:8, :],
                            stage[:, b, 1024 * hf + 512:1024 * (hf + 1)].rearrange(
                                "p (j n) -> p j n", j=4))
                        trigs.append(nc.gpsimd.collective_compute(
                            "AllToAll", ALU.bypass,
                            replica_groups=[list(range(N_CORES))],
                            ins=[a2a_in[k].ap().opt()],
                            outs=[a2a_out[k].ap().opt()],
                        ))

                seq = [(qc, kt) for qc in qcs for kt in range(N_KT)]
                for i, (qc, kt) in enumerate(seq):
                    states.setdefault(qc, {"exs": [None] * N_KP})
                    qk_exp(qc, kt)
                    for _ in range(pops):
                        if bg:
                            bg.pop(0)()
                    j = i - OV
                    if j >= 0 and seq[j][1] % 2 == 1:
                        jqc, jkt = seq[j]
                        av_a(jqc, jkt // 2)
                        av_b(jqc, jkt // 2)
                        if jkt == N_KT - 1:
                            finalize(jqc)
                for j in range(max(0, len(seq) - OV), len(seq)):
                    if seq[j][1] % 2 == 1:
                        jqc, jkt = seq[j]
                        av_a(jqc, jkt // 2)
                        av_b(jqc, jkt // 2)
                        if jkt == N_KT - 1:
                            finalize(jqc)
                return av_last

            # drive: emit only chunk 0 of b0's QKV up front, then start
            # attention qc0 with chunks 1-3 interleaved as background steps
            # (order [K,V] per chunk matches the kt windows that consume them);
            # b1's QKV interleaves into b0's qc1-3.
            l0, q0, k0, v0 = make_qkv_steps(0)
            for step in l0:
                step()
            for s in k0[0]:
                s()
            for s in q0[0]:
                s()
            for s in v0[0]:
                s()
            l1, q1, k1, v1 = make_qkv_steps(1)
            # bgA feeds b0 qc0: kT chunk c needed by step 4c, v chunk c by the
            # av of its kt-pairs, q0[1] before qc1 starts; 28 closures, 32 slots
            bgA = []
            for tc8 in (1, 2, 3):
                bgA.extend(k0[tc8])
                bgA.extend(v0[tc8])
            bgA.extend(q0[1])
            attention(0, bgA, qcs=[0])
            while bgA:
                bgA.pop(0)()
            # qc1: rest of b0's Q + b1's loads and kT (25 closures, 32 slots)
            bgB = list(q0[2]) + list(q0[3]) + list(l1)
            for tc8 in range(4):
                bgB.extend(k1[tc8])
            attention(0, bgB, qcs=[1])
            while bgB:
                bgB.pop(0)()
            # qc2+qc3: b1's Q and V (32 closures, 32 slots)
            bgC = []
            for tc8 in range(4):
                bgC.extend(q1[tc8])
                bgC.extend(v1[tc8])
            attention(0, bgC, qcs=[2, 3], pops=1)
            while bgC:
                bgC.pop(0)()
            av_anchors = attention(1, [])

            # deferred bulk const loads (issued after attention DMAs in queue order)
            nc.sync.dma_start(msa_w_sb[:], msa_w[:, :].rearrange("(a p) n -> p a n", p=128))
            nc.sync.dma_start(y_shard_sb[:], y_shard[:, :].rearrange("(a p) n -> p a n", p=128))
            if use_gamma:
                nc.sync.dma_start(gamma_sb[:], gamma_b[:, :])
            if use_beta:
                nc.sync.dma_start(beta_sb[:], beta_b[:, :])

            # ============== msa + residual + LayerNorm per shard ==============
            res_sb = stp.tile([128, 4, E], F32)
            for k in range(4):
                lhs = a2ap.tile([128, 8, 128], F8, tag="lhs")
                nc.sync.dma_start(lhs[:], a2a_out[k].ap().rearrange("j p n -> p j n"))
                # i-major so consecutive matmuls share lhs weights (LDW dedup);
                # both e-halves accumulate concurrently in two psum tiles.
                # fp8 DoubleRow: i-pairs, contraction 1024 in 4 passes.
                ps_m0 = ps_acc.tile([128, 512], F32, tag="acc", name="ps_m0")
                ps_m1 = ps_acc.tile([128, 512], F32, tag="acc", name="ps_m1")
                for i in range(4):
                    for ec, ps_m in ((0, ps_m0), (1, ps_m1)):
                        mi = nc.tensor.matmul(ps_m[:], lhs[:, 2 * i:2 * i + 2, :],
                                              msa_w_sb[:, 2 * i:2 * i + 2,
                                                       512 * ec:512 * (ec + 1)],
                                              start=(i == 0), stop=(i == 3),
                                              perf_mode=DR)
                        if ec == 0 and i == 0:
                            # keep msa out of the PE stream until b1 attention
                            # has progressed past qc k+1 (the A2A data won't be
                            # there earlier; an early msa blocks the in-order PE)
                            _add_dep_helper(
                                mi.ins, av_anchors[min(k + 1, 3)].ins, sync=False,
                                reason="msa gated behind b1 attention progress")
                for ec, ps_m in ((0, ps_m0), (1, ps_m1)):
                    # residual add, on DVE (no ACT table switch)
                    rhalf = res_sb[:, k, 512 * ec:512 * (ec + 1)]
                    ri = nc.vector.tensor_tensor(
                        rhalf, ps_m[:],
                        y_shard_sb[:, k, 512 * ec:512 * (ec + 1)], ALU.add)
                    if ec == 0:
                        # keep this shard's LN work behind the (k+1)-th
                        # collective TRIGGER on the DVE queue: the trigger path
                        # of the last q-chunk must not queue behind LN ops
                        _add_dep_helper(
                            ri.ins, trigs[min(k + 1, 3)].ins, sync=False,
                            reason="LN deprioritized behind collective trigger")
                # fused mean/var via bn_stats halves + one aggregate
                stats = finp.tile([128, 2, 6], F32, tag="stats")
                nc.vector.bn_stats(stats[:, 0, :], res_sb[:, k, 0:512])
                nc.vector.bn_stats(stats[:, 1, :], res_sb[:, k, 512:1024])
                mu = cpool.tile([128, 4], F32, name=f"mu{k}")
                nc.vector.bn_aggr(mu[:, 0:2], stats[:])
                # rstd = sqrt(1/(var+eps)); 51-ULP reciprocal is plenty here
                nc.vector.tensor_scalar(mu[:, 1:2], mu[:, 1:2], EPS, None, ALU.add)
                nc.vector.reciprocal_approx_fast(mu[:, 2:3], mu[:, 1:2])
                nc.scalar.activation(mu[:, 3:4], mu[:, 2:3], AF.Sqrt)
                nc.vector.tensor_scalar(mu[:, 0:1], mu[:, 0:1], -1.0, None, ALU.mult)
                # o1 = (res - mu) * rstd, fused on DVE
                o1 = finp.tile([128, E], F32, tag="o1")
                nc.vector.tensor_scalar(o1[:], res_sb[:, k, :], mu[:, 0:1],
                                        mu[:, 3:4], ALU.add, ALU.mult)
                if use_gamma:
                    nc.vector.tensor_tensor(o1[:], o1[:], gamma_sb[:], ALU.mult)
                if use_beta:
                    nc.vector.tensor_tensor(o1[:], o1[:], beta_sb[:], ALU.add)
                nc.sync.dma_start(out[128 * k:128 * (k + 1), :], o1[:])

    nc.compile()
    return nc


@functools.lru_cache(maxsize=4)
def _get_nc(use_gamma: bool, use_beta: bool):
    return _build(use_gamma, use_beta)


def kernel(**inputs) -> np.ndarray:
    y = np.asarray(inputs["y"], np.float32)
    Wqkv = np.asarray(inputs["Wqkv"], np.float32)
    bqkv = np.asarray(inputs["bqkv"], np.float32)
    Wmsa = np.asarray(inputs["Wmsa"], np.float32)
    Bq_, Aq_ = np.asarray(inputs["Bq"], np.float32), np.asarray(inputs["Aq"], np.float32)
    Bk_, Ak_ = np.asarray(inputs["Bk"], np.float32), np.asarray(inputs["Ak"], np.float32)
    Bv_, Av_ = np.asarray(inputs["Bv"], np.float32), np.asarray(inputs["Av"], np.float32)
    Bo_, Ao_ = np.asarray(inputs["Bo"], np.float32), np.asarray(inputs["Ao"], np.float32)
    gamma = np.asarray(inputs["gamma"], np.float32)
    beta = np.asarray(inputs["beta"], np.float32)

    # effective weights: qkv = y @ (Wqkv.T + blockdiag-ish LoRA) + bqkv
    # y @ W.T: W rows are output dims. LoRA adds y @ (B@A): effective W += (B@A).T
    W_eff = Wqkv.copy()
    W_eff[0:E] += (Bq_ @ Aq_).T
    W_eff[E:2 * E] += (Bk_ @ Ak_).T
    W_eff[2 * E:3 * E] += (Bv_ @ Av_).T
    # fp8 pre-scale: weights x32 (1/sqrt(D) and the scale unwind live in
    # the exp affine scale and the /32 on the msa weights)
    W_eff *= WSC
    bq_eff = bqkv[0:E] * WSC
    bk_eff = bqkv[E:2 * E] * WSC
    bv_eff = bqkv[2 * E:3 * E] * WSC
    # msa: o @ Wmsa.T + o @ (Bo@Ao) = o @ M with M = Wmsa.T + Bo@Ao  [E(d), E(out)].
    # stage carries 32*(o+bv); store 32*M in fp8 (good dynamic range) so the
    # msa psum is 1024*msa, and scale the residual y by 1024 to match --
    # LayerNorm is scale-invariant, so the final output is unchanged.
    M = (Wmsa.T + Bo_ @ Ao_) * WSC

    y_flat = y.reshape(T, E)
    yT_f8 = np.ascontiguousarray(y_flat.T).astype(NP_F8)
    M_f8 = np.ascontiguousarray(M).astype(NP_F8)

    use_gamma = not np.allclose(gamma, 1.0)
    use_beta = not np.allclose(beta, 0.0)
    nc = _get_nc(use_gamma, use_beta)

    in_maps = []
    for c in range(N_CORES):
        r0 = c * 128
        r1 = r0 + 128
        wq_c = np.ascontiguousarray(W_eff[0:E][r0:r1].T).astype(NP_F8)
        wk_c = np.ascontiguousarray(W_eff[E:2 * E][r0:r1].T).astype(NP_F8)
        wv_c = np.ascontiguousarray(W_eff[2 * E:3 * E][r0:r1].T).astype(NP_F8)
        tok = np.concatenate([
            np.arange(128 * c, 128 * c + 128),
            np.arange(1024 + 128 * c, 1024 + 128 * c + 128),
            np.arange(2048 + 128 * c, 2048 + 128 * c + 128),
            np.arange(3072 + 128 * c, 3072 + 128 * c + 128),
        ])
        m = {
            "yT": yT_f8,
            "wqT": wq_c,
            "wkT": wk_c,
            "wvT": wv_c,
            "bq": bq_eff[r0:r1].reshape(128, 1).copy(),
            "bk": bk_eff[r0:r1].reshape(128, 1).copy(),
            "bva": bv_eff[r0:r0 + 64].reshape(64, 1).copy(),
            "bvb": bv_eff[r0 + 64:r1].reshape(64, 1).copy(),
            "msa_w": M_f8,
            "y_shard": np.ascontiguousarray(y_flat[tok]) * (WSC * WSC),
        }
        if use_gamma:
            m["gamma_b"] = np.broadcast_to(gamma, (128, E)).copy()
        if use_beta:
            m["beta_b"] = np.broadcast_to(beta, (128, E)).copy()
        in_maps.append(m)

    res = bass_utils.run_bass_kernel_spmd(nc, in_maps, core_ids=list(range(N_CORES)))

    out_full = np.empty((T, E), np.float32)
    for c in range(N_CORES):
        oc = res.results[c]["out"]
        out_full[128 * c:128 * c + 128] = oc[0:128]
        out_full[1024 + 128 * c:1024 + 128 * c + 128] = oc[128:256]
        out_full[2048 + 128 * c:2048 + 128 * c + 128] = oc[256:384]
        out_full[3072 + 128 * c:3072 + 128 * c + 128] = oc[384:512]
    return out_full.reshape(B, S, E)


# revision 25
# speedup vs baseline: 1.1513x; 1.0011x over previous
"""Trainium2 Bass kernel for fused LoRA-attention block (nn_Attention_18846316494887).

Reference computation:
  qkv = y @ Wqkv.T + bqkv (+ LoRA deltas y @ (B@A) per Q/K/V)  -> Q,K,V [B,H,S,D]
  attn = softmax(Q K^T / sqrt(D)); o = attn @ V -> [B,S,E]
  msa = o @ Wmsa.T + o @ (Bo@Ao); res = msa + y; out = LayerNorm(res)*gamma + beta

Sharding: tensor-parallel over heads (2 heads/core, 8 cores), AllToAll to
reshard head-dim -> token-dim before the output projection, token-parallel
msa + LayerNorm, host-side gather of per-core token shards.

Precision plan (error budget: attention path contributes only ~2.2% of the
LN'd output norm, so a few-% relative error there is invisible):
  - y, Wqkv (x32), V, exp(scores) all in fp8e4m3; f32 PSUM accumulation
  - Q/K projection matmuls in DoubleRow mode (2 fp8 k-subtiles per pass)
  - AV matmuls in DoubleRow mode over kt-pairs (halves the ex stream time)
  - the x32*x32 weight scaling and 1/sqrt(D) fold into the exp's free
    affine scale (exp(x * 1/8192)); V-scale folds into msa weights (/32)

Host-side prep (exact algebra, no approximation):
  - LoRA folded into Wqkv / Wmsa (y@W.T + y@(B@A) == y@(W.T + B@A))
  - V bias applied post-softmax on o (exact since attn rows sum to 1)
  - y pre-transposed to [E, T] for the QKV matmuls
"""
import functools
import numpy as np
import ml_dtypes

import concourse.mybir as mybir
import concourse.tile as tile
from concourse import bacc
from concourse import bass_utils
from concourse.bass import _add_dep_helper

# problem shapes (hardcoded per harness contract)
E = 1024
H = 16
D = 64
B = 2
S = 2048
T = B * S          # 4096 tokens
N_CORES = 8
EPS = 1e-6

BF16 = mybir.dt.bfloat16
F32 = mybir.dt.float32
F8 = mybir.dt.float8e4
NP_F8 = ml_dtypes.float8_e4m3
AF = mybir.ActivationFunctionType
ALU = mybir.AluOpType
DR = mybir.MatmulPerfMode.DoubleRow

# per-core worksizes
TOK = T // N_CORES          # 512 tokens per core for msa/LN
QC = 512                    # attention q-chunk
N_QC = S // QC              # 4 q-chunks per (b, head-pair)
N_KT = S // 128             # 16 k-tiles
N_KP = N_KT // 2            # 8 kt-pairs (DoubleRow AV granularity)
VW = 80                     # padded V row (64 d + 1 ones + pad to 16B mult)
WSC = 32.0                  # fp8 weight pre-scale
S_ACT = 1.0 / (WSC * WSC * 8.0)   # exp affine scale: /32^2 (w-scales) /sqrt(D)


def _build(use_gamma: bool, use_beta: bool):
    nc = bacc.Bacc("TRN2", target_bir_lowering=False, debug=False, num_devices=N_CORES)

    # ---- DRAM parameters -------------------------------------------------
    yT = nc.dram_tensor("yT", [E, T], F8, kind="ExternalInput")
    wqT = nc.dram_tensor("wqT", [E, 128], F8, kind="ExternalInput")
    wkT = nc.dram_tensor("wkT", [E, 128], F8, kind="ExternalInput")
    wvT = nc.dram_tensor("wvT", [E, 128], F8, kind="ExternalInput")
    bq = nc.dram_tensor("bq", [128, 1], F32, kind="ExternalInput")
    bk = nc.dram_tensor("bk", [128, 1], F32, kind="ExternalInput")
    bva = nc.dram_tensor("bva", [64, 1], F32, kind="ExternalInput")
    bvb = nc.dram_tensor("bvb", [64, 1], F32, kind="ExternalInput")
    msa_w = nc.dram_tensor("msa_w", [E, E], F8, kind="ExternalInput")
    y_shard = nc.dram_tensor("y_shard", [TOK, E], F32, kind="ExternalInput")
    if use_gamma:
        gamma_b = nc.dram_tensor("gamma_b", [128, E], F32, kind="ExternalInput")
    if use_beta:
        beta_b = nc.dram_tensor("beta_b", [128, E], F32, kind="ExternalInput")
    out = nc.dram_tensor("out", [TOK, E], F32, kind="ExternalOutput")

    # internal DRAM: A2A bounce buffers (shard k: (b, q-half) -> 128 tok/core)
    a2a_in = [nc.dram_tensor(f"a2a_in{k}", [N_CORES, 128, 128], F8) for k in range(4)]
    a2a_out = [nc.dram_tensor(f"a2a_out{k}", [N_CORES, 128, 128], F8) for k in range(4)]

    with tile.TileContext(nc) as tc:
        with (
            tc.tile_pool(name="const", bufs=1) as cpool,
            tc.tile_pool(name="yt", bufs=5) as ytp,
            tc.tile_pool(name="qk", bufs=1) as qkp,
            tc.tile_pool(name="exp", bufs=3) as expp,
            tc.tile_pool(name="stage", bufs=1) as stp,
            tc.tile_pool(name="fin", bufs=2) as finp,
            tc.tile_pool(name="a2asb", bufs=4) as a2ap,
            tc.tile_pool(name="ps_acc", bufs=2, space="PSUM") as ps_acc,
            tc.tile_pool(name="ps_sc", bufs=2, space="PSUM") as ps_sc,
            tc.tile_pool(name="ps_av", bufs=2, space="PSUM") as ps_av,
        ):
            # ---- constants -------------------------------------------------
            wqT_sb = cpool.tile([128, 8, 128], F8)
            wkT_sb = cpool.tile([128, 8, 128], F8)
            wvT_sb = cpool.tile([128, 8, 128], F8)
            nc.sync.dma_start(wqT_sb[:], wqT[:, :].rearrange("(a p) n -> p a n", p=128))
            nc.sync.dma_start(wkT_sb[:], wkT[:, :].rearrange("(a p) n -> p a n", p=128))
            nc.sync.dma_start(wvT_sb[:], wvT[:, :].rearrange("(a p) n -> p a n", p=128))
            bq_sb = cpool.tile([128, 1], F32)
            bk_sb = cpool.tile([128, 1], F32)
            bva_sb = cpool.tile([64, 1], F32)
            bvb_sb = cpool.tile([64, 1], F32)
            nc.sync.dma_start(bq_sb[:], bq[:, :])
            nc.sync.dma_start(bk_sb[:], bk[:, :])
            nc.sync.dma_start(bva_sb[:], bva[:, :])
            nc.sync.dma_start(bvb_sb[:], bvb[:, :])
            # (msa weights / LN consts are DMA'd later, after the attention
            # loops are issued, so startup DMA bandwidth goes to yT tiles)
            msa_w_sb = cpool.tile([128, 8, E], F8)
            y_shard_sb = cpool.tile([128, 4, E], F32)
            if use_gamma:
                gamma_sb = cpool.tile([128, E], F32)
            if use_beta:
                beta_sb = cpool.tile([128, E], F32)

            # V tiles, padded: [k-part, b, head, kt, VW]; col 64 = ones
            v_sb = cpool.tile([128, B, 2, N_KT, VW], F8)
            nc.vector.memset(v_sb[:, :, :, :, 64:VW], 0.0)
            nc.vector.memset(v_sb[:, :, :, :, 64:65], 1.0)

            # Q^T/K^T: [d-part(2 heads), b, q]
            qT_sb = qkp.tile([128, B, S], BF16)
            kT_sb = qkp.tile([128, B, S], BF16)
            # o^T staging for A2A: [d-part, b, q] (fp8: carries 32*(o+bv))
            stage = stp.tile([128, B, S], F8)

            # ============== main per-batch pipeline ==============
            def make_qkv_steps(b):
                """QKV projection for batch b as a list of small closures so the
                PE work can be interleaved into the other batch's attention
                (fills the in-order PE stream's exp-wait slots)."""
                loads, qs, ks, vs = [], [], [], []
                for tc8 in range(4):
                    st8 = {}

                    def load(b=b, tc8=tc8, st8=st8):
                        yt = ytp.tile([128, 8, 512], F8, tag="yt")
                        st8["yt"] = yt
                        for et in range(8):
                            nc.sync.dma_start(
                                yt[:, et, :], yT[128 * et:128 * (et + 1),
                                                 b * S + 512 * tc8: b * S + 512 * (tc8 + 1)])
                    loads.append(load)
                    qs.append([])
                    ks.append([])
                    vs.append([])

                    # Q/K: 4 DoubleRow matmuls (et-pairs), K=1024 contraction.
                    # DR forbids column tile_position offsets, so each mm is
                    # full-width [128, 2, 128] -> out [128, 512].
                    for eg in range(4):
                        def qstep(b=b, tc8=tc8, eg=eg, st8=st8):
                            if eg == 0:
                                st8["ps_q"] = ps_acc.tile([128, 512], F32, tag="acc", name="ps_q")
                            ps_q, yt = st8["ps_q"], st8["yt"]
                            st, sp = (eg == 0), (eg == 3)
                            nc.tensor.matmul(ps_q[:], wqT_sb[:, 2 * eg:2 * eg + 2, :],
                                             yt[:, 2 * eg:2 * eg + 2, :], start=st, stop=sp,
                                             perf_mode=DR)
                            if eg == 3:
                                nc.vector.tensor_scalar(
                                    qT_sb[:, b, 512 * tc8:512 * (tc8 + 1)], ps_q[:],
                                    bq_sb[:], None, ALU.add)
                        qs[tc8].append(qstep)

                    for eg in range(4):
                        def kstep(b=b, tc8=tc8, eg=eg, st8=st8):
                            if eg == 0:
                                st8["ps_k"] = ps_acc.tile([128, 512], F32, tag="acc", name="ps_k")
                            ps_k, yt = st8["ps_k"], st8["yt"]
                            st, sp = (eg == 0), (eg == 3)
                            nc.tensor.matmul(ps_k[:], wkT_sb[:, 2 * eg:2 * eg + 2, :],
                                             yt[:, 2 * eg:2 * eg + 2, :], start=st, stop=sp,
                                             perf_mode=DR)
                            if eg == 3:
                                nc.vector.tensor_scalar(
                                    kT_sb[:, b, 512 * tc8:512 * (tc8 + 1)], ps_k[:],
                                    bk_sb[:], None, ALU.add)
                        ks[tc8].append(kstep)

                    # V: [tok, vdim] layout, fp8 operands (no DoubleRow: the
                    # stationary operand changes every matmul)
                    for eg in range(4):
                        def vstep(b=b, tc8=tc8, eg=eg, st8=st8):
                            if eg == 0:
                                st8["ps_v"] = ps_acc.tile([128, 512], F32, tag="acc", name="ps_v")
                            ps_v, yt = st8["ps_v"], st8["yt"]
                            for et in (2 * eg, 2 * eg + 1):
                                st, sp = (et == 0), (et == 7)
                                for s4 in range(4):
                                    nc.tensor.matmul(ps_v[:, 128 * s4:128 * (s4 + 1)],
                                                     yt[:, et, 128 * s4:128 * (s4 + 1)],
                                                     wvT_sb[:, et, :], start=st, stop=sp)
                            if eg == 3:
                                for h in range(2):
                                    src = ps_v[:, :].rearrange(
                                        "p (s n) -> p s n", s=4)[:, :, 64 * h:64 * (h + 1)]
                                    nc.vector.tensor_copy(
                                        v_sb[:, b, h, 4 * tc8:4 * (tc8 + 1), 0:64], src)
                        vs[tc8].append(vstep)
                return loads, qs, ks, vs

            trigs = []  # collective trigger instrs, k-order

            def attention(b, bg, av_last=None, qcs=range(N_QC), pops=2):
                # software-pipelined ACROSS kt steps: qk/exp runs OV steps
                # ahead of av, so the ACT engine never drains at qc
                # boundaries; bg closures (other QKV work) fill PE wait slots.
                # AV runs per kt-PAIR in fp8 DoubleRow mode.
                if av_last is None:
                    av_last = []
                OV = 4
                states = {}

                def qk_exp(qc, kt):
                    stq = states[qc]
                    if kt % 2 == 0:
                        stq["exs"][kt // 2] = expp.tile([128, 2, 1024], F8, name="ex")
                    sc = ps_sc.tile([128, 1024], F32, tag="sc", name="sc")
                    nc.tensor.matmul(sc[:, 0:512],
                                     kT_sb[0:64, b, 128 * kt:128 * (kt + 1)],
                                     qT_sb[0:64, b, QC * qc:QC * (qc + 1)],
                                     start=True, stop=True, tile_position=(0, 0))
                    nc.tensor.matmul(sc[:, 512:1024],
                                     kT_sb[64:128, b, 128 * kt:128 * (kt + 1)],
                                     qT_sb[64:128, b, QC * qc:QC * (qc + 1)],
                                     start=True, stop=True, tile_position=(64, 0))
                    ex = stq["exs"][kt // 2]
                    nc.scalar.activation(ex[:, kt % 2, :], sc[:], AF.Exp, scale=S_ACT)

                def av_a(qc, kp):
                    stq = states[qc]
                    if kp == 0:
                        stq["av_a"] = ps_av.tile([128, 512], F32, tag="av", name="av_a")
                        stq["av_b"] = ps_av.tile([128, 512], F32, tag="av", name="av_b")
                    ex = stq["exs"][kp]
                    nc.tensor.matmul(stq["av_a"][0:65, :],
                                     v_sb[:, b, 0, 2 * kp:2 * kp + 2, 0:65],
                                     ex[:, :, 0:512],
                                     start=(kp == 0), stop=(kp == N_KP - 1), perf_mode=DR)

                def av_b(qc, kp):
                    stq = states[qc]
                    ex = stq["exs"][kp]
                    i2 = nc.tensor.matmul(stq["av_b"][0:65, :],
                                          v_sb[:, b, 1, 2 * kp:2 * kp + 2, 0:65],
                                          ex[:, :, 512:1024],
                                          start=(kp == 0), stop=(kp == N_KP - 1), perf_mode=DR)
                    if kp == N_KP - 1:
                        av_last.append(i2)

                def finalize(qc):
                    av_a, av_b = states[qc]["av_a"], states[qc]["av_b"]
                    # drain AV psum to SBUF fast (releases psum for next q-chunk)
                    af = finp.tile([128, 1024], F32, tag="af", name="af")
                    nc.vector.tensor_copy(af[0:65, 0:512], av_a[0:65, :])
                    nc.vector.tensor_copy(af[0:65, 512:1024], av_b[0:65, :])
                    # denominator row -> partition 0 (DMA shifts partitions;
                    # reciprocal_approx_fast corrupts on non-zero base
                    # partitions, so the recip must run at partition 0),
                    # then gpsimd broadcast to all lanes
                    rc = finp.tile([128, 1024], F32, tag="rc", name="rc")
                    nc.gpsimd.dma_start(rc[0:1, :], af[64:65, :])
                    rc2 = finp.tile([128, 1024], F32, tag="rc2", name="rc2")
                    nc.vector.reciprocal_approx_fast(rc2[0:1, :], rc[0:1, :])
                    rb = finp.tile([128, 1024], F32, tag="rb", name="rb")
                    nc.gpsimd.partition_broadcast(rb[:, :], rc2[0:1, :])
                    # o^T = o_raw^T * recip + bv; all on partitions 0..63, then
                    # head B is partition-shifted into the stage via DMA.
                    # (fp8 tiles are write-only for the DVE: mult lands in an
                    # f32 scratch, the bias-add writes the fp8 copy once)
                    osc = stage[:, b, QC * qc:QC * (qc + 1)]
                    om = finp.tile([64, 1024], F32, tag="om", name="om")
                    nc.vector.tensor_tensor(om[:, 0:512], af[0:64, 0:512], rb[0:64, 0:512], ALU.mult)
                    nc.vector.tensor_scalar(om[:, 0:512], om[:, 0:512], bva_sb[:], None, ALU.add)
                    nc.vector.tensor_copy(osc[0:64, :], om[:, 0:512])
                    tb = finp.tile([64, 512], F8, tag="tb", name="tb")
                    nc.vector.tensor_tensor(om[:, 512:1024], af[0:64, 512:1024], rb[0:64, 512:1024], ALU.mult)
                    nc.vector.tensor_scalar(om[:, 512:1024], om[:, 512:1024], bvb_sb[:], None, ALU.add)
                    nc.vector.tensor_copy(tb[:], om[:, 512:1024])
                    nc.gpsimd.dma_start(osc[64:128, :], tb[:])
                    # A2A per q-half: upload each qc's blocks as soon as
                    # staged; issue the collective after the odd qc
                    hf = qc // 2
                    k = 2 * b + hf
                    half = a2a_in[k].ap().rearrange("j p n -> p j n")
                    if qc % 2 == 0:
                        nc.gpsimd.dma_start(
                            half[:, 0:4, :],
                            stage[:, b, 1024 * hf:1024 * hf + 512].rearrange(
                                "p (j n) -> p j n", j=4))
                    else:
                        nc.gpsimd.dma_start(
                            half[:, 4:8, :],
                            stage[:, b, 1024 * hf + 512:1024 * (hf + 1)].rearrange(
                                "p (j n) -> p j n", j=4))
                        trigs.append(nc.gpsimd.collective_compute(
                            "AllToAll", ALU.bypass,
                            replica_groups=[list(range(N_CORES))],
                            ins=[a2a_in[k].ap().opt()],
                            outs=[a2a_out[k].ap().opt()],
                        ))

                seq = [(qc, kt) for qc in qcs for kt in range(N_KT)]
                for i, (qc, kt) in enumerate(seq):
                    states.setdefault(qc, {"exs": [None] * N_KP})
                    qk_exp(qc, kt)
                    for _ in range(pops):
                        if bg:
                            bg.pop(0)()
                    j = i - OV
                    if j >= 0 and seq[j][1] % 2 == 1:
                        jqc, jkt = seq[j]
                        av_a(jqc, jkt // 2)
                        av_b(jqc, jkt // 2)
                        if jkt == N_KT - 1:
                            finalize(jqc)
                for j in range(max(0, len(seq) - OV), len(seq)):
                    if seq[j][1] % 2 == 1:
                        jqc, jkt = seq[j]
                        av_a(jqc, jkt // 2)
                        av_b(jqc, jkt // 2)
                        if jkt == N_KT - 1:
                            finalize(jqc)
                return av_last

            # drive: emit only chunk 0 of b0's QKV up front, then start
            # attention qc0 with chunks 1-3 interleaved as background steps
            # (order [K,V] per chunk matches the kt windows that consume them);
            # b1's QKV interleaves into b0's qc1-3.
            l0, q0, k0, v0 = make_qkv_steps(0)
            for step in l0:
                step()
            for s in k0[0]:
                s()
            for s in q0[0]:
                s()
            for s in v0[0]:
                s()
            l1, q1, k1, v1 = make_qkv_steps(1)
            # bgA feeds b0 qc0: kT chunk c needed by step 4c, v chunk c by the
            # av of its kt-pairs, q0[1] before qc1 starts; 28 closures, 32 slots
            bgA = []
            for tc8 in (1, 2, 3):
                bgA.extend(k0[tc8])
                bgA.extend(v0[tc8])
            bgA.extend(q0[1])
            attention(0, bgA, qcs=[0])
            while bgA:
                bgA.pop(0)()
            # qc1: rest of b0's Q + b1's loads and kT (25 closures, 32 slots)
            bgB = list(q0[2]) + list(q0[3]) + list(l1)
            for tc8 in range(4):
                bgB.extend(k1[tc8])
            attention(0, bgB, qcs=[1])
            while bgB:
                bgB.pop(0)()
            # qc2+qc3: b1's Q and V (32 closures, 32 slots)
            bgC = []
            for tc8 in range(4):
                bgC.extend(q1[tc8])
                bgC.extend(v1[tc8])
            attention(0, bgC, qcs=[2, 3], pops=1)
            while bgC:
                bgC.pop(0)()
            av_anchors = attention(1, [])

            # deferred bulk const loads (issued after attention DMAs in queue order)
            nc.sync.dma_start(msa_w_sb[:], msa_w[:, :].rearrange("(a p) n -> p a n", p=128))
            nc.sync.dma_start(y_shard_sb[:], y_shard[:, :].rearrange("(a p) n -> p a n", p=128))
            if use_gamma:
                nc.sync.dma_start(gamma_sb[:], gamma_b[:, :])
            if use_beta:
                nc.sync.dma_start(beta_sb[:], beta_b[:, :])

            # ============== msa + residual + LayerNorm per shard ==============
            res_sb = stp.tile([128, 4, E], F32)
            for k in range(4):
                lhs = a2ap.tile([128, 8, 128], F8, tag="lhs")
                nc.sync.dma_start(lhs[:], a2a_out[k].ap().rearrange("j p n -> p j n"))
                # i-major so consecutive matmuls share lhs weights (LDW dedup);
                # both e-halves accumulate concurrently in two psum tiles.
                # fp8 DoubleRow: i-pairs, contraction 1024 in 4 passes.
                ps_m0 = ps_acc.tile([128, 512], F32, tag="acc", name="ps_m0")
                ps_m1 = ps_acc.tile([128, 512], F32, tag="acc", name="ps_m1")
                for i in range(4):
                    for ec, ps_m in ((0, ps_m0), (1, ps_m1)):
                        mi = nc.tensor.matmul(ps_m[:], lhs[:, 2 * i:2 * i + 2, :],
                                              msa_w_sb[:, 2 * i:2 * i + 2,
                                                       512 * ec:512 * (ec + 1)],
                                              start=(i == 0), stop=(i == 3),
                                              perf_mode=DR)
                        if ec == 0 and i == 0:
                            # keep msa out of the PE stream until b1 attention
                            # has progressed past qc k+1 (the A2A data won't be
                            # there earlier; an early msa blocks the in-order PE)
                            _add_dep_helper(
                                mi.ins, av_anchors[min(k + 1, 3)].ins, sync=False,
                                reason="msa gated behind b1 attention progress")
                for ec, ps_m in ((0, ps_m0), (1, ps_m1)):
                    # residual add, on DVE (no ACT table switch)
                    rhalf = res_sb[:, k, 512 * ec:512 * (ec + 1)]
                    ri = nc.vector.tensor_tensor(
                        rhalf, ps_m[:],
                        y_shard_sb[:, k, 512 * ec:512 * (ec + 1)], ALU.add)
                    if ec == 0:
                        # keep this shard's LN work behind the (k+1)-th
                        # collective TRIGGER on the DVE queue: the trigger path
                        # of the last q-chunk must not queue behind LN ops.
                        # NOTE: sync=False on purpose -- a sync=True semaphore
                        # edge here deadlocks the device (cross-engine cycle)
                        _add_dep_helper(
                            ri.ins, trigs[min(k + 1, 3)].ins, sync=False,
                            reason="LN deprioritized behind collective trigger")
                # fused mean/var via bn_stats halves + one aggregate
                stats = finp.tile([128, 2, 6], F32, tag="stats")
                nc.vector.bn_stats(stats[:, 0, :], res_sb[:, k, 0:512])
                nc.vector.bn_stats(stats[:, 1, :], res_sb[:, k, 512:1024])
                mu = cpool.tile([128, 4], F32, name=f"mu{k}")
                nc.vector.bn_aggr(mu[:, 0:2], stats[:])
                # rstd = sqrt(1/(var+eps)); 51-ULP reciprocal is plenty here
                nc.vector.tensor_scalar(mu[:, 1:2], mu[:, 1:2], EPS, None, ALU.add)
                nc.vector.reciprocal_approx_fast(mu[:, 2:3], mu[:, 1:2])
                nc.scalar.activation(mu[:, 3:4], mu[:, 2:3], AF.Sqrt)
                nc.vector.tensor_scalar(mu[:, 0:1], mu[:, 0:1], -1.0, None, ALU.mult)
                # o1 = (res - mu) * rstd, fused on DVE
                o1 = finp.tile([128, E], F32, tag="o1")
                nc.vector.tensor_scalar(o1[:], res_sb[:, k, :], mu[:, 0:1],
                                        mu[:, 3:4], ALU.add, ALU.mult)
                if use_gamma:
                    nc.vector.tensor_tensor(o1[:], o1[:], gamma_sb[:], ALU.mult)
                if use_beta:
                    nc.vector.tensor_tensor(o1[:], o1[:], beta_sb[:], ALU.add)
                nc.sync.dma_start(out[128 * k:128 * (k + 1), :], o1[:])

    nc.compile()
    return nc


@functools.lru_cache(maxsize=4)
def _get_nc(use_gamma: bool, use_beta: bool):
    return _build(use_gamma, use_beta)


def kernel(**inputs) -> np.ndarray:
    y = np.asarray(inputs["y"], np.float32)
    Wqkv = np.asarray(inputs["Wqkv"], np.float32)
    bqkv = np.asarray(inputs["bqkv"], np.float32)
    Wmsa = np.asarray(inputs["Wmsa"], np.float32)
    Bq_, Aq_ = np.asarray(inputs["Bq"], np.float32), np.asarray(inputs["Aq"], np.float32)
    Bk_, Ak_ = np.asarray(inputs["Bk"], np.float32), np.asarray(inputs["Ak"], np.float32)
    Bv_, Av_ = np.asarray(inputs["Bv"], np.float32), np.asarray(inputs["Av"], np.float32)
    Bo_, Ao_ = np.asarray(inputs["Bo"], np.float32), np.asarray(inputs["Ao"], np.float32)
    gamma = np.asarray(inputs["gamma"], np.float32)
    beta = np.asarray(inputs["beta"], np.float32)

    # effective weights: qkv = y @ (Wqkv.T + blockdiag-ish LoRA) + bqkv
    # y @ W.T: W rows are output dims. LoRA adds y @ (B@A): effective W += (B@A).T
    W_eff = Wqkv.copy()
    W_eff[0:E] += (Bq_ @ Aq_).T
    W_eff[E:2 * E] += (Bk_ @ Ak_).T
    W_eff[2 * E:3 * E] += (Bv_ @ Av_).T
    # fp8 pre-scale: weights x32 (1/sqrt(D) and the scale unwind live in
    # the exp affine scale and the /32 on the msa weights)
    W_eff *= WSC
    bq_eff = bqkv[0:E] * WSC
    bk_eff = bqkv[E:2 * E] * WSC
    bv_eff = bqkv[2 * E:3 * E] * WSC
    # msa: o @ Wmsa.T + o @ (Bo@Ao) = o @ M with M = Wmsa.T + Bo@Ao  [E(d), E(out)].
    # stage carries 32*(o+bv); store 32*M in fp8 (good dynamic range) so the
    # msa psum is 1024*msa, and scale the residual y by 1024 to match --
    # LayerNorm is scale-invariant, so the final output is unchanged.
    M = (Wmsa.T + Bo_ @ Ao_) * WSC

    y_flat = y.reshape(T, E)
    yT_f8 = np.ascontiguousarray(y_flat.T).astype(NP_F8)
    M_f8 = np.ascontiguousarray(M).astype(NP_F8)

    use_gamma = not np.allclose(gamma, 1.0)
    use_beta = not np.allclose(beta, 0.0)
    nc = _get_nc(use_gamma, use_beta)

    in_maps = []
    for c in range(N_CORES):
        r0 = c * 128
        r1 = r0 + 128
        wq_c = np.ascontiguousarray(W_eff[0:E][r0:r1].T).astype(NP_F8)
        wk_c = np.ascontiguousarray(W_eff[E:2 * E][r0:r1].T).astype(NP_F8)
        wv_c = np.ascontiguousarray(W_eff[2 * E:3 * E][r0:r1].T).astype(NP_F8)
        tok = np.concatenate([
            np.arange(128 * c, 128 * c + 128),
            np.arange(1024 + 128 * c, 1024 + 128 * c + 128),
            np.arange(2048 + 128 * c, 2048 + 128 * c + 128),
            np.arange(3072 + 128 * c, 3072 + 128 * c + 128),
        ])
        m = {
            "yT": yT_f8,
            "wqT": wq_c,
            "wkT": wk_c,
            "wvT": wv_c,
            "bq": bq_eff[r0:r1].reshape(128, 1).copy(),
            "bk": bk_eff[r0:r1].reshape(128, 1).copy(),
            "bva": bv_eff[r0:r0 + 64].reshape(64, 1).copy(),
            "bvb": bv_eff[r0 + 64:r1].reshape(64, 1).copy(),
            "msa_w": M_f8,
            "y_shard": np.ascontiguousarray(y_flat[tok]) * (WSC * WSC),
        }
        if use_gamma:
            m["gamma_b"] = np.broadcast_to(gamma, (128, E)).copy()
        if use_beta:
            m["beta_b"] = np.broadcast_to(beta, (128, E)).copy()
        in_maps.append(m)

    res = bass_utils.run_bass_kernel_spmd(nc, in_maps, core_ids=list(range(N_CORES)))

    out_full = np.empty((T, E), np.float32)
    for c in range(N_CORES):
        oc = res.results[c]["out"]
        out_full[128 * c:128 * c + 128] = oc[0:128]
        out_full[1024 + 128 * c:1024 + 128 * c + 128] = oc[128:256]
        out_full[2048 + 128 * c:2048 + 128 * c + 128] = oc[256:384]
        out_full[3072 + 128 * c:3072 + 128 * c + 128] = oc[384:512]
    return out_full.reshape(B, S, E)


# revision 26
# speedup vs baseline: 1.1796x; 1.0246x over previous
"""Trainium2 Bass kernel for fused LoRA-attention block (nn_Attention_18846316494887).

Reference computation:
  qkv = y @ Wqkv.T + bqkv (+ LoRA deltas y @ (B@A) per Q/K/V)  -> Q,K,V [B,H,S,D]
  attn = softmax(Q K^T / sqrt(D)); o = attn @ V -> [B,S,E]
  msa = o @ Wmsa.T + o @ (Bo@Ao); res = msa + y; out = LayerNorm(res)*gamma + beta

Sharding: tensor-parallel over heads (2 heads/core, 8 cores), AllToAll to
reshard head-dim -> token-dim before the output projection, token-parallel
msa + LayerNorm, host-side gather of per-core token shards.

Precision plan (error budget: attention path contributes only ~2.2% of the
LN'd output norm, so a few-% relative error there is invisible):
  - y, Wqkv (x32), V, exp(scores) all in fp8e4m3; f32 PSUM accumulation
  - Q/K projection matmuls in DoubleRow mode (2 fp8 k-subtiles per pass)
  - AV matmuls in DoubleRow mode over kt-pairs (halves the ex stream time)
  - the x32*x32 weight scaling and 1/sqrt(D) fold into the exp's free
    affine scale (exp(x * 1/8192)); V-scale folds into msa weights (/32)

Host-side prep (exact algebra, no approximation):
  - LoRA folded into Wqkv / Wmsa (y@W.T + y@(B@A) == y@(W.T + B@A))
  - V bias applied post-softmax on o (exact since attn rows sum to 1)
  - y pre-transposed to [E, T] for the QKV matmuls
"""
import functools
import numpy as np
import ml_dtypes

import concourse.mybir as mybir
import concourse.tile as tile
from concourse import bacc
from concourse import bass_utils
from concourse.bass import _add_dep_helper

# problem shapes (hardcoded per harness contract)
E = 1024
H = 16
D = 64
B = 2
S = 2048
T = B * S          # 4096 tokens
N_CORES = 8
EPS = 1e-6

BF16 = mybir.dt.bfloat16
F32 = mybir.dt.float32
F8 = mybir.dt.float8e4
NP_F8 = ml_dtypes.float8_e4m3
AF = mybir.ActivationFunctionType
ALU = mybir.AluOpType
DR = mybir.MatmulPerfMode.DoubleRow

# per-core worksizes
TOK = T // N_CORES          # 512 tokens per core for msa/LN
QC = 512                    # attention q-chunk
N_QC = S // QC              # 4 q-chunks per (b, head-pair)
N_KT = S // 128             # 16 k-tiles
N_KP = N_KT // 2            # 8 kt-pairs (DoubleRow AV granularity)
VW = 80                     # padded V row (64 d + 1 ones + pad to 16B mult)
WSC = 32.0                  # fp8 weight pre-scale
S_ACT = 1.0 / (WSC * WSC * 8.0)   # exp affine scale: /32^2 (w-scales) /sqrt(D)


def _build(use_gamma: bool, use_beta: bool):
    nc = bacc.Bacc("TRN2", target_bir_lowering=False, debug=False, num_devices=N_CORES)

    # ---- DRAM parameters -------------------------------------------------
    yT = nc.dram_tensor("yT", [E, T], F8, kind="ExternalInput")
    wqT = nc.dram_tensor("wqT", [E, 128], F8, kind="ExternalInput")
    wkT = nc.dram_tensor("wkT", [E, 128], F8, kind="ExternalInput")
    wvT = nc.dram_tensor("wvT", [E, 128], F8, kind="ExternalInput")
    bq = nc.dram_tensor("bq", [128, 1], F32, kind="ExternalInput")
    bk = nc.dram_tensor("bk", [128, 1], F32, kind="ExternalInput")
    bva = nc.dram_tensor("bva", [64, 1], F32, kind="ExternalInput")
    bvb = nc.dram_tensor("bvb", [64, 1], F32, kind="ExternalInput")
    msa_w = nc.dram_tensor("msa_w", [E, E], F8, kind="ExternalInput")
    y_shard = nc.dram_tensor("y_shard", [TOK, E], F32, kind="ExternalInput")
    if use_gamma:
        gamma_b = nc.dram_tensor("gamma_b", [128, E], F32, kind="ExternalInput")
    if use_beta:
        beta_b = nc.dram_tensor("beta_b", [128, E], F32, kind="ExternalInput")
    out = nc.dram_tensor("out", [TOK, E], F32, kind="ExternalOutput")

    # internal DRAM: A2A bounce buffers (shard k: (b, q-half) -> 128 tok/core)
    a2a_in = [nc.dram_tensor(f"a2a_in{k}", [N_CORES, 128, 128], F8) for k in range(4)]
    a2a_out = [nc.dram_tensor(f"a2a_out{k}", [N_CORES, 128, 128], F8) for k in range(4)]

    with tile.TileContext(nc) as tc:
        with (
            tc.tile_pool(name="const", bufs=1) as cpool,
            tc.tile_pool(name="yt", bufs=5) as ytp,
            tc.tile_pool(name="qk", bufs=1) as qkp,
            tc.tile_pool(name="exp", bufs=3) as expp,
            tc.tile_pool(name="stage", bufs=1) as stp,
            tc.tile_pool(name="fin", bufs=2) as finp,
            tc.tile_pool(name="a2asb", bufs=4) as a2ap,
            tc.tile_pool(name="ps_acc", bufs=2, space="PSUM") as ps_acc,
            tc.tile_pool(name="ps_sc", bufs=2, space="PSUM") as ps_sc,
            tc.tile_pool(name="ps_av", bufs=2, space="PSUM") as ps_av,
        ):
            # ---- constants -------------------------------------------------
            wqT_sb = cpool.tile([128, 8, 128], F8)
            wkT_sb = cpool.tile([128, 8, 128], F8)
            wvT_sb = cpool.tile([128, 8, 128], F8)
            nc.sync.dma_start(wqT_sb[:], wqT[:, :].rearrange("(a p) n -> p a n", p=128))
            nc.sync.dma_start(wkT_sb[:], wkT[:, :].rearrange("(a p) n -> p a n", p=128))
            nc.sync.dma_start(wvT_sb[:], wvT[:, :].rearrange("(a p) n -> p a n", p=128))
            bq_sb = cpool.tile([128, 1], F32)
            bk_sb = cpool.tile([128, 1], F32)
            bva_sb = cpool.tile([64, 1], F32)
            bvb_sb = cpool.tile([64, 1], F32)
            nc.sync.dma_start(bq_sb[:], bq[:, :])
            nc.sync.dma_start(bk_sb[:], bk[:, :])
            nc.sync.dma_start(bva_sb[:], bva[:, :])
            nc.sync.dma_start(bvb_sb[:], bvb[:, :])
            # (msa weights / LN consts are DMA'd later, after the attention
            # loops are issued, so startup DMA bandwidth goes to yT tiles)
            msa_w_sb = cpool.tile([128, 8, E], F8)
            y_shard_sb = cpool.tile([128, 4, E], F32)
            if use_gamma:
                gamma_sb = cpool.tile([128, E], F32)
            if use_beta:
                beta_sb = cpool.tile([128, E], F32)

            # V tiles, padded: [k-part, b, head, kt, VW]; col 64 = ones
            v_sb = cpool.tile([128, B, 2, N_KT, VW], F8)
            nc.vector.memset(v_sb[:, :, :, :, 64:VW], 0.0)
            nc.vector.memset(v_sb[:, :, :, :, 64:65], 1.0)

            # Q^T/K^T: [d-part(2 heads), b, q]
            qT_sb = qkp.tile([128, B, S], BF16)
            kT_sb = qkp.tile([128, B, S], BF16)
            # o^T staging for A2A: [d-part, b, q] (fp8: carries 32*(o+bv))
            stage = stp.tile([128, B, S], F8)

            # ============== main per-batch pipeline ==============
            def make_qkv_steps(b):
                """QKV projection for batch b as a list of small closures so the
                PE work can be interleaved into the other batch's attention
                (fills the in-order PE stream's exp-wait slots)."""
                loads, qs, ks, vs = [], [], [], []
                for tc8 in range(4):
                    st8 = {}

                    def load(b=b, tc8=tc8, st8=st8):
                        yt = ytp.tile([128, 8, 512], F8, tag="yt")
                        st8["yt"] = yt
                        for et in range(8):
                            # b0 startup is DMA-latency-bound: stripe across
                            # the sync + gpsimd queues for 2x issue bandwidth.
                            # b1 loads run mid-kernel when sync is idle but
                            # gpsimd carries the finalize/A2A path: sync only.
                            eng = nc.gpsimd if (b == 0 and et % 2 == 1) else nc.sync
                            eng.dma_start(
                                yt[:, et, :], yT[128 * et:128 * (et + 1),
                                                 b * S + 512 * tc8: b * S + 512 * (tc8 + 1)])
                    loads.append(load)
                    qs.append([])
                    ks.append([])
                    vs.append([])

                    # Q/K: 4 DoubleRow matmuls (et-pairs), K=1024 contraction.
                    # DR forbids column tile_position offsets, so each mm is
                    # full-width [128, 2, 128] -> out [128, 512].
                    for eg in range(4):
                        def qstep(b=b, tc8=tc8, eg=eg, st8=st8):
                            if eg == 0:
                                st8["ps_q"] = ps_acc.tile([128, 512], F32, tag="acc", name="ps_q")
                            ps_q, yt = st8["ps_q"], st8["yt"]
                            st, sp = (eg == 0), (eg == 3)
                            nc.tensor.matmul(ps_q[:], wqT_sb[:, 2 * eg:2 * eg + 2, :],
                                             yt[:, 2 * eg:2 * eg + 2, :], start=st, stop=sp,
                                             perf_mode=DR)
                            if eg == 3:
                                nc.vector.tensor_scalar(
                                    qT_sb[:, b, 512 * tc8:512 * (tc8 + 1)], ps_q[:],
                                    bq_sb[:], None, ALU.add)
                        qs[tc8].append(qstep)

                    for eg in range(4):
                        def kstep(b=b, tc8=tc8, eg=eg, st8=st8):
                            if eg == 0:
                                st8["ps_k"] = ps_acc.tile([128, 512], F32, tag="acc", name="ps_k")
                            ps_k, yt = st8["ps_k"], st8["yt"]
                            st, sp = (eg == 0), (eg == 3)
                            nc.tensor.matmul(ps_k[:], wkT_sb[:, 2 * eg:2 * eg + 2, :],
                                             yt[:, 2 * eg:2 * eg + 2, :], start=st, stop=sp,
                                             perf_mode=DR)
                            if eg == 3:
                                nc.vector.tensor_scalar(
                                    kT_sb[:, b, 512 * tc8:512 * (tc8 + 1)], ps_k[:],
                                    bk_sb[:], None, ALU.add)
                        ks[tc8].append(kstep)

                    # V: [tok, vdim] layout, fp8 operands (no DoubleRow: the
                    # stationary operand changes every matmul)
                    for eg in range(4):
                        def vstep(b=b, tc8=tc8, eg=eg, st8=st8):
                            if eg == 0:
                                st8["ps_v"] = ps_acc.tile([128, 512], F32, tag="acc", name="ps_v")
                            ps_v, yt = st8["ps_v"], st8["yt"]
                            for et in (2 * eg, 2 * eg + 1):
                                st, sp = (et == 0), (et == 7)
                                for s4 in range(4):
                                    nc.tensor.matmul(ps_v[:, 128 * s4:128 * (s4 + 1)],
                                                     yt[:, et, 128 * s4:128 * (s4 + 1)],
                                                     wvT_sb[:, et, :], start=st, stop=sp)
                            if eg == 3:
                                for h in range(2):
                                    src = ps_v[:, :].rearrange(
                                        "p (s n) -> p s n", s=4)[:, :, 64 * h:64 * (h + 1)]
                                    nc.vector.tensor_copy(
                                        v_sb[:, b, h, 4 * tc8:4 * (tc8 + 1), 0:64], src)
                        vs[tc8].append(vstep)
                return loads, qs, ks, vs

            trigs = []  # collective trigger instrs, k-order

            def attention(b, bg, av_last=None, qcs=range(N_QC), pops=2):
                # software-pipelined ACROSS kt steps: qk/exp runs OV steps
                # ahead of av, so the ACT engine never drains at qc
                # boundaries; bg closures (other QKV work) fill PE wait slots.
                # AV runs per kt-PAIR in fp8 DoubleRow mode.
                if av_last is None:
                    av_last = []
                OV = 4
                states = {}

                def qk_exp(qc, kt):
                    stq = states[qc]
                    if kt % 2 == 0:
                        stq["exs"][kt // 2] = expp.tile([128, 2, 1024], F8, name="ex")
                    sc = ps_sc.tile([128, 1024], F32, tag="sc", name="sc")
                    nc.tensor.matmul(sc[:, 0:512],
                                     kT_sb[0:64, b, 128 * kt:128 * (kt + 1)],
                                     qT_sb[0:64, b, QC * qc:QC * (qc + 1)],
                                     start=True, stop=True, tile_position=(0, 0))
                    nc.tensor.matmul(sc[:, 512:1024],
                                     kT_sb[64:128, b, 128 * kt:128 * (kt + 1)],
                                     qT_sb[64:128, b, QC * qc:QC * (qc + 1)],
                                     start=True, stop=True, tile_position=(64, 0))
                    ex = stq["exs"][kt // 2]
                    nc.scalar.activation(ex[:, kt % 2, :], sc[:], AF.Exp, scale=S_ACT)

                def av_a(qc, kp):
                    stq = states[qc]
                    if kp == 0:
                        stq["av_a"] = ps_av.tile([128, 512], F32, tag="av", name="av_a")
                        stq["av_b"] = ps_av.tile([128, 512], F32, tag="av", name="av_b")
                    ex = stq["exs"][kp]
                    nc.tensor.matmul(stq["av_a"][0:65, :],
                                     v_sb[:, b, 0, 2 * kp:2 * kp + 2, 0:65],
                                     ex[:, :, 0:512],
                                     start=(kp == 0), stop=(kp == N_KP - 1), perf_mode=DR)

                def av_b(qc, kp):
                    stq = states[qc]
                    ex = stq["exs"][kp]
                    i2 = nc.tensor.matmul(stq["av_b"][0:65, :],
                                          v_sb[:, b, 1, 2 * kp:2 * kp + 2, 0:65],
                                          ex[:, :, 512:1024],
                                          start=(kp == 0), stop=(kp == N_KP - 1), perf_mode=DR)
                    if kp == N_KP - 1:
                        av_last.append(i2)

                def finalize(qc):
                    av_a, av_b = states[qc]["av_a"], states[qc]["av_b"]
                    # drain AV psum to SBUF fast (releases psum for next q-chunk)
                    af = finp.tile([128, 1024], F32, tag="af", name="af")
                    nc.vector.tensor_copy(af[0:65, 0:512], av_a[0:65, :])
                    nc.vector.tensor_copy(af[0:65, 512:1024], av_b[0:65, :])
                    # denominator row -> partition 0 (DMA shifts partitions;
                    # reciprocal_approx_fast corrupts on non-zero base
                    # partitions, so the recip must run at partition 0),
                    # then gpsimd broadcast to all lanes
                    rc = finp.tile([128, 1024], F32, tag="rc", name="rc")
                    nc.gpsimd.dma_start(rc[0:1, :], af[64:65, :])
                    rc2 = finp.tile([128, 1024], F32, tag="rc2", name="rc2")
                    nc.vector.reciprocal_approx_fast(rc2[0:1, :], rc[0:1, :])
                    rb = finp.tile([128, 1024], F32, tag="rb", name="rb")
                    nc.gpsimd.partition_broadcast(rb[:, :], rc2[0:1, :])
                    # o^T = o_raw^T * recip + bv; all on partitions 0..63, then
                    # head B is partition-shifted into the stage via DMA.
                    # (fp8 tiles are write-only for the DVE: mult lands in an
                    # f32 scratch, the bias-add writes the fp8 copy once)
                    osc = stage[:, b, QC * qc:QC * (qc + 1)]
                    om = finp.tile([64, 1024], F32, tag="om", name="om")
                    nc.vector.tensor_tensor(om[:, 0:512], af[0:64, 0:512], rb[0:64, 0:512], ALU.mult)
                    nc.vector.tensor_scalar(om[:, 0:512], om[:, 0:512], bva_sb[:], None, ALU.add)
                    nc.vector.tensor_copy(osc[0:64, :], om[:, 0:512])
                    tb = finp.tile([64, 512], F8, tag="tb", name="tb")
                    nc.vector.tensor_tensor(om[:, 512:1024], af[0:64, 512:1024], rb[0:64, 512:1024], ALU.mult)
                    nc.vector.tensor_scalar(om[:, 512:1024], om[:, 512:1024], bvb_sb[:], None, ALU.add)
                    nc.vector.tensor_copy(tb[:], om[:, 512:1024])
                    nc.gpsimd.dma_start(osc[64:128, :], tb[:])
                    # A2A per q-half: upload each qc's blocks as soon as
                    # staged; issue the collective after the odd qc
                    hf = qc // 2
                    k = 2 * b + hf
                    half = a2a_in[k].ap().rearrange("j p n -> p j n")
                    if qc % 2 == 0:
                        nc.gpsimd.dma_start(
                            half[:, 0:4, :],
                            stage[:, b, 1024 * hf:1024 * hf + 512].rearrange(
                                "p (j n) -> p j n", j=4))
                    else:
                        nc.gpsimd.dma_start(
                            half[:, 4:8, :],
                            stage[:, b, 1024 * hf + 512:1024 * (hf + 1)].rearrange(
                                "p (j n) -> p j n", j=4))
                        trigs.append(nc.gpsimd.collective_compute(
                            "AllToAll", ALU.bypass,
                            replica_groups=[list(range(N_CORES))],
                            ins=[a2a_in[k].ap().opt()],
                            outs=[a2a_out[k].ap().opt()],
                        ))

                seq = [(qc, kt) for qc in qcs for kt in range(N_KT)]
                for i, (qc, kt) in enumerate(seq):
                    states.setdefault(qc, {"exs": [None] * N_KP})
                    qk_exp(qc, kt)
                    for _ in range(pops):
                        if bg:
                            bg.pop(0)()
                    j = i - OV
                    if j >= 0 and seq[j][1] % 2 == 1:
                        jqc, jkt = seq[j]
                        av_a(jqc, jkt // 2)
                        av_b(jqc, jkt // 2)
                        if jkt == N_KT - 1:
                            finalize(jqc)
                for j in range(max(0, len(seq) - OV), len(seq)):
                    if seq[j][1] % 2 == 1:
                        jqc, jkt = seq[j]
                        av_a(jqc, jkt // 2)
                        av_b(jqc, jkt // 2)
                        if jkt == N_KT - 1:
                            finalize(jqc)
                return av_last

            # drive: emit only chunk 0 of b0's QKV up front, then start
            # attention qc0 with chunks 1-3 interleaved as background steps
            # (order [K,V] per chunk matches the kt windows that consume them);
            # b1's QKV interleaves into b0's qc1-3.
            l0, q0, k0, v0 = make_qkv_steps(0)
            for step in l0:
                step()
            for s in k0[0]:
                s()
            for s in q0[0]:
                s()
            for s in v0[0]:
                s()
            l1, q1, k1, v1 = make_qkv_steps(1)
            # bgA feeds b0 qc0: kT chunk c needed by step 4c, v chunk c by the
            # av of its kt-pairs, q0[1] before qc1 starts; 28 closures, 32 slots
            bgA = []
            for tc8 in (1, 2, 3):
                bgA.extend(k0[tc8])
                bgA.extend(v0[tc8])
            bgA.extend(q0[1])
            attention(0, bgA, qcs=[0])
            while bgA:
                bgA.pop(0)()
            # qc1: rest of b0's Q + b1's loads and kT (25 closures, 32 slots)
            bgB = list(q0[2]) + list(q0[3]) + list(l1)
            for tc8 in range(4):
                bgB.extend(k1[tc8])
            attention(0, bgB, qcs=[1])
            while bgB:
                bgB.pop(0)()
            # qc2+qc3: b1's Q and V (32 closures, 32 slots)
            bgC = []
            for tc8 in range(4):
                bgC.extend(q1[tc8])
                bgC.extend(v1[tc8])
            attention(0, bgC, qcs=[2, 3], pops=1)
            while bgC:
                bgC.pop(0)()
            av_anchors = attention(1, [])

            # deferred bulk const loads (issued after attention DMAs in queue order)
            nc.sync.dma_start(msa_w_sb[:], msa_w[:, :].rearrange("(a p) n -> p a n", p=128))
            nc.sync.dma_start(y_shard_sb[:], y_shard[:, :].rearrange("(a p) n -> p a n", p=128))
            if use_gamma:
                nc.sync.dma_start(gamma_sb[:], gamma_b[:, :])
            if use_beta:
                nc.sync.dma_start(beta_sb[:], beta_b[:, :])

            # ============== msa + residual + LayerNorm per shard ==============
            res_sb = stp.tile([128, 4, E], F32)
            for k in range(4):
                lhs = a2ap.tile([128, 8, 128], F8, tag="lhs")
                nc.sync.dma_start(lhs[:], a2a_out[k].ap().rearrange("j p n -> p j n"))
                # i-major so consecutive matmuls share lhs weights (LDW dedup);
                # both e-halves accumulate concurrently in two psum tiles.
                # fp8 DoubleRow: i-pairs, contraction 1024 in 4 passes.
                ps_m0 = ps_acc.tile([128, 512], F32, tag="acc", name="ps_m0")
                ps_m1 = ps_acc.tile([128, 512], F32, tag="acc", name="ps_m1")
                for i in range(4):
                    for ec, ps_m in ((0, ps_m0), (1, ps_m1)):
                        mi = nc.tensor.matmul(ps_m[:], lhs[:, 2 * i:2 * i + 2, :],
                                              msa_w_sb[:, 2 * i:2 * i + 2,
                                                       512 * ec:512 * (ec + 1)],
                                              start=(i == 0), stop=(i == 3),
                                              perf_mode=DR)
                        if ec == 0 and i == 0:
                            # keep msa out of the PE stream until b1 attention
                            # has progressed past qc k+1 (the A2A data won't be
                            # there earlier; an early msa blocks the in-order PE)
                            _add_dep_helper(
                                mi.ins, av_anchors[min(k + 1, 3)].ins, sync=False,
                                reason="msa gated behind b1 attention progress")
                for ec, ps_m in ((0, ps_m0), (1, ps_m1)):
                    # residual add, on DVE (no ACT table switch)
                    rhalf = res_sb[:, k, 512 * ec:512 * (ec + 1)]
                    ri = nc.vector.tensor_tensor(
                        rhalf, ps_m[:],
                        y_shard_sb[:, k, 512 * ec:512 * (ec + 1)], ALU.add)
                    if ec == 0:
                        # keep this shard's LN work behind the (k+1)-th
                        # collective TRIGGER on the DVE queue: the trigger path
                        # of the last q-chunk must not queue behind LN ops.
                        # NOTE: sync=False on purpose -- a sync=True semaphore
                        # edge here deadlocks the device (cross-engine cycle)
                        _add_dep_helper(
                            ri.ins, trigs[min(k + 1, 3)].ins, sync=False,
                            reason="LN deprioritized behind collective trigger")
                # fused mean/var via bn_stats halves + one aggregate
                stats = finp.tile([128, 2, 6], F32, tag="stats")
                nc.vector.bn_stats(stats[:, 0, :], res_sb[:, k, 0:512])
                nc.vector.bn_stats(stats[:, 1, :], res_sb[:, k, 512:1024])
                mu = cpool.tile([128, 4], F32, name=f"mu{k}")
                nc.vector.bn_aggr(mu[:, 0:2], stats[:])
                # rstd = sqrt(1/(var+eps)); 51-ULP reciprocal is plenty here
                nc.vector.tensor_scalar(mu[:, 1:2], mu[:, 1:2], EPS, None, ALU.add)
                nc.vector.reciprocal_approx_fast(mu[:, 2:3], mu[:, 1:2])
                nc.scalar.activation(mu[:, 3:4], mu[:, 2:3], AF.Sqrt)
                nc.vector.tensor_scalar(mu[:, 0:1], mu[:, 0:1], -1.0, None, ALU.mult)
                # o1 = (res - mu) * rstd, fused on DVE
                o1 = finp.tile([128, E], F32, tag="o1")
                nc.vector.tensor_scalar(o1[:], res_sb[:, k, :], mu[:, 0:1],
                                        mu[:, 3:4], ALU.add, ALU.mult)
                if use_gamma:
                    nc.vector.tensor_tensor(o1[:], o1[:], gamma_sb[:], ALU.mult)
                if use_beta:
                    nc.vector.tensor_tensor(o1[:], o1[:], beta_sb[:], ALU.add)
                nc.sync.dma_start(out[128 * k:128 * (k + 1), :], o1[:])

    nc.compile()
    return nc


@functools.lru_cache(maxsize=4)
def _get_nc(use_gamma: bool, use_beta: bool):
    return _build(use_gamma, use_beta)


def kernel(**inputs) -> np.ndarray:
    y = np.asarray(inputs["y"], np.float32)
    Wqkv = np.asarray(inputs["Wqkv"], np.float32)
    bqkv = np.asarray(inputs["bqkv"], np.float32)
    Wmsa = np.asarray(inputs["Wmsa"], np.float32)
    Bq_, Aq_ = np.asarray(inputs["Bq"], np.float32), np.asarray(inputs["Aq"], np.float32)
    Bk_, Ak_ = np.asarray(inputs["Bk"], np.float32), np.asarray(inputs["Ak"], np.float32)
    Bv_, Av_ = np.asarray(inputs["Bv"], np.float32), np.asarray(inputs["Av"], np.float32)
    Bo_, Ao_ = np.asarray(inputs["Bo"], np.float32), np.asarray(inputs["Ao"], np.float32)
    gamma = np.asarray(inputs["gamma"], np.float32)
    beta = np.asarray(inputs["beta"], np.float32)

    # effective weights: qkv = y @ (Wqkv.T + blockdiag-ish LoRA) + bqkv
    # y @ W.T: W rows are output dims. LoRA adds y @ (B@A): effective W += (B@A).T
    W_eff = Wqkv.copy()
    W_eff[0:E] += (Bq_ @ Aq_).T
    W_eff[E:2 * E] += (Bk_ @ Ak_).T
    W_eff[2 * E:3 * E] += (Bv_ @ Av_).T
    # fp8 pre-scale: weights x32 (1/sqrt(D) and the scale unwind live in
    # the exp affine scale and the /32 on the msa weights)
    W_eff *= WSC
    bq_eff = bqkv[0:E] * WSC
    bk_eff = bqkv[E:2 * E] * WSC
    bv_eff = bqkv[2 * E:3 * E] * WSC
    # msa: o @ Wmsa.T + o @ (Bo@Ao) = o @ M with M = Wmsa.T + Bo@Ao  [E(d), E(out)].
    # stage carries 32*(o+bv); store 32*M in fp8 (good dynamic range) so the
    # msa psum is 1024*msa, and scale the residual y by 1024 to match --
    # LayerNorm is scale-invariant, so the final output is unchanged.
    M = (Wmsa.T + Bo_ @ Ao_) * WSC

    y_flat = y.reshape(T, E)
    yT_f8 = np.ascontiguousarray(y_flat.T).astype(NP_F8)
    M_f8 = np.ascontiguousarray(M).astype(NP_F8)

    use_gamma = not np.allclose(gamma, 1.0)
    use_beta = not np.allclose(beta, 0.0)
    nc = _get_nc(use_gamma, use_beta)

    in_maps = []
    for c in range(N_CORES):
        r0 = c * 128
        r1 = r0 + 128
        wq_c = np.ascontiguousarray(W_eff[0:E][r0:r1].T).astype(NP_F8)
        wk_c = np.ascontiguousarray(W_eff[E:2 * E][r0:r1].T).astype(NP_F8)
        wv_c = np.ascontiguousarray(W_eff[2 * E:3 * E][r0:r1].T).astype(NP_F8)
        tok = np.concatenate([
            np.arange(128 * c, 128 * c + 128),
            np.arange(1024 + 128 * c, 1024 + 128 * c + 128),
            np.arange(2048 + 128 * c, 2048 + 128 * c + 128),
            np.arange(3072 + 128 * c, 3072 + 128 * c + 128),
        ])
        m = {
            "yT": yT_f8,
            "wqT": wq_c,
            "wkT": wk_c,
            "wvT": wv_c,
            "bq": bq_eff[r0:r1].reshape(128, 1).copy(),
            "bk": bk_eff[r0:r1].reshape(128, 1).copy(),
            "bva": bv_eff[r0:r0 + 64].reshape(64, 1).copy(),
            "bvb": bv_eff[r0 + 64:r1].reshape(64, 1).copy(),
            "msa_w": M_f8,
            "y_shard": np.ascontiguousarray(y_flat[tok]) * (WSC * WSC),
        }
        if use_gamma:
            m["gamma_b"] = np.broadcast_to(gamma, (128, E)).copy()
        if use_beta:
            m["beta_b"] = np.broadcast_to(beta, (128, E)).copy()
        in_maps.append(m)

    res = bass_utils.run_bass_kernel_spmd(nc, in_maps, core_ids=list(range(N_CORES)))

    out_full = np.empty((T, E), np.float32)
    for c in range(N_CORES):
        oc = res.results[c]["out"]
        out_full[128 * c:128 * c + 128] = oc[0:128]
        out_full[1024 + 128 * c:1024 + 128 * c + 128] = oc[128:256]
        out_full[2048 + 128 * c:2048 + 128 * c + 128] = oc[256:384]
        out_full[3072 + 128 * c:3072 + 128 * c + 128] = oc[384:512]
    return out_full.reshape(B, S, E)


# revision 27
# speedup vs baseline: 1.2261x; 1.0394x over previous
"""Trainium2 Bass kernel for fused LoRA-attention block (nn_Attention_18846316494887).

Reference computation:
  qkv = y @ Wqkv.T + bqkv (+ LoRA deltas y @ (B@A) per Q/K/V)  -> Q,K,V [B,H,S,D]
  attn = softmax(Q K^T / sqrt(D)); o = attn @ V -> [B,S,E]
  msa = o @ Wmsa.T + o @ (Bo@Ao); res = msa + y; out = LayerNorm(res)*gamma + beta

Sharding: tensor-parallel over heads (2 heads/core, 8 cores), AllToAll to
reshard head-dim -> token-dim before the output projection, token-parallel
msa + LayerNorm, host-side gather of per-core token shards.

Precision plan (error budget: attention path contributes only ~2.2% of the
LN'd output norm, so a few-% relative error there is invisible):
  - y, Wqkv (x32), V, exp(scores) all in fp8e4m3; f32 PSUM accumulation
  - Q/K projection matmuls in DoubleRow mode (2 fp8 k-subtiles per pass)
  - AV matmuls in DoubleRow mode over kt-pairs (halves the ex stream time)
  - the x32*x32 weight scaling and 1/sqrt(D) fold into the exp's free
    affine scale (exp(x * 1/8192)); V-scale folds into msa weights (/32)

Host-side prep (exact algebra, no approximation):
  - LoRA folded into Wqkv / Wmsa (y@W.T + y@(B@A) == y@(W.T + B@A))
  - V bias applied post-softmax on o (exact since attn rows sum to 1)
  - y pre-transposed to [E, T] for the QKV matmuls
"""
import functools
import numpy as np
import ml_dtypes

import concourse.mybir as mybir
import concourse.tile as tile
from concourse import bacc
from concourse import bass_utils
from concourse.bass import _add_dep_helper

# problem shapes (hardcoded per harness contract)
E = 1024
H = 16
D = 64
B = 2
S = 2048
T = B * S          # 4096 tokens
N_CORES = 8
EPS = 1e-6

BF16 = mybir.dt.bfloat16
F32 = mybir.dt.float32
F8 = mybir.dt.float8e4
NP_F8 = ml_dtypes.float8_e4m3
AF = mybir.ActivationFunctionType
ALU = mybir.AluOpType
DR = mybir.MatmulPerfMode.DoubleRow

# per-core worksizes
TOK = T // N_CORES          # 512 tokens per core for msa/LN
QC = 512                    # attention q-chunk
N_QC = S // QC              # 4 q-chunks per (b, head-pair)
N_KT = S // 128             # 16 k-tiles
N_KP = N_KT // 2            # 8 kt-pairs (DoubleRow AV granularity)
VW = 80                     # padded V row (64 d + 1 ones + pad to 16B mult)
WSC = 32.0                  # fp8 weight pre-scale
S_ACT = 1.0 / (WSC * WSC * 8.0)   # exp affine scale: /32^2 (w-scales) /sqrt(D)


def _build(use_gamma: bool, use_beta: bool):
    nc = bacc.Bacc("TRN2", target_bir_lowering=False, debug=False, num_devices=N_CORES)

    # ---- DRAM parameters -------------------------------------------------
    yT = nc.dram_tensor("yT", [E, T], F8, kind="ExternalInput")
    wqT = nc.dram_tensor("wqT", [E, 128], F8, kind="ExternalInput")
    wkT = nc.dram_tensor("wkT", [E, 128], F8, kind="ExternalInput")
    wvT = nc.dram_tensor("wvT", [E, 128], F8, kind="ExternalInput")
    bq = nc.dram_tensor("bq", [128, 1], F32, kind="ExternalInput")
    bk = nc.dram_tensor("bk", [128, 1], F32, kind="ExternalInput")
    bva = nc.dram_tensor("bva", [64, 1], F32, kind="ExternalInput")
    bvb = nc.dram_tensor("bvb", [64, 1], F32, kind="ExternalInput")
    msa_w = nc.dram_tensor("msa_w", [E, E], F8, kind="ExternalInput")
    y_shard = nc.dram_tensor("y_shard", [TOK, E], F32, kind="ExternalInput")
    if use_gamma:
        gamma_b = nc.dram_tensor("gamma_b", [128, E], F32, kind="ExternalInput")
    if use_beta:
        beta_b = nc.dram_tensor("beta_b", [128, E], F32, kind="ExternalInput")
    out = nc.dram_tensor("out", [TOK, E], F32, kind="ExternalOutput")

    # internal DRAM: A2A bounce buffers (shard k: (b, q-half) -> 128 tok/core)
    a2a_in = [nc.dram_tensor(f"a2a_in{k}", [N_CORES, 128, 128], F8) for k in range(4)]
    a2a_out = [nc.dram_tensor(f"a2a_out{k}", [N_CORES, 128, 128], F8) for k in range(4)]

    with tile.TileContext(nc) as tc:
        with (
            tc.tile_pool(name="const", bufs=1) as cpool,
            tc.tile_pool(name="yt", bufs=5) as ytp,
            tc.tile_pool(name="qk", bufs=1) as qkp,
            tc.tile_pool(name="exp", bufs=3) as expp,
            tc.tile_pool(name="stage", bufs=1) as stp,
            tc.tile_pool(name="fin", bufs=2) as finp,
            tc.tile_pool(name="a2asb", bufs=4) as a2ap,
            tc.tile_pool(name="ps_acc", bufs=2, space="PSUM") as ps_acc,
            tc.tile_pool(name="ps_sc", bufs=2, space="PSUM") as ps_sc,
            tc.tile_pool(name="ps_av", bufs=2, space="PSUM") as ps_av,
        ):
            # ---- constants -------------------------------------------------
            wqT_sb = cpool.tile([128, 8, 128], F8)
            wkT_sb = cpool.tile([128, 8, 128], F8)
            wvT_sb = cpool.tile([128, 8, 128], F8)
            # stripe the startup weight loads across both DMA queues so the
            # first yT tiles (behind them in queue order) land sooner
            nc.sync.dma_start(wqT_sb[:], wqT[:, :].rearrange("(a p) n -> p a n", p=128))
            nc.gpsimd.dma_start(wkT_sb[:], wkT[:, :].rearrange("(a p) n -> p a n", p=128))
            nc.sync.dma_start(wvT_sb[:], wvT[:, :].rearrange("(a p) n -> p a n", p=128))
            bq_sb = cpool.tile([128, 1], F32)
            bk_sb = cpool.tile([128, 1], F32)
            bva_sb = cpool.tile([64, 1], F32)
            bvb_sb = cpool.tile([64, 1], F32)
            nc.sync.dma_start(bq_sb[:], bq[:, :])
            nc.sync.dma_start(bk_sb[:], bk[:, :])
            nc.sync.dma_start(bva_sb[:], bva[:, :])
            nc.sync.dma_start(bvb_sb[:], bvb[:, :])
            # (msa weights / LN consts are DMA'd later, after the attention
            # loops are issued, so startup DMA bandwidth goes to yT tiles)
            msa_w_sb = cpool.tile([128, 8, E], F8)
            y_shard_sb = cpool.tile([128, 4, E], F32)
            if use_gamma:
                gamma_sb = cpool.tile([128, E], F32)
            if use_beta:
                beta_sb = cpool.tile([128, E], F32)

            # V tiles, padded: [k-part, b, head, kt, VW]; col 64 = ones
            v_sb = cpool.tile([128, B, 2, N_KT, VW], F8)
            nc.vector.memset(v_sb[:, :, :, :, 64:VW], 0.0)
            nc.vector.memset(v_sb[:, :, :, :, 64:65], 1.0)

            # Q^T/K^T: [d-part(2 heads), b, q]
            qT_sb = qkp.tile([128, B, S], BF16)
            kT_sb = qkp.tile([128, B, S], BF16)
            # o^T staging for A2A: [d-part, b, q] (fp8: carries 32*(o+bv))
            stage = stp.tile([128, B, S], F8)

            # ============== main per-batch pipeline ==============
            def make_qkv_steps(b):
                """QKV projection for batch b as a list of small closures so the
                PE work can be interleaved into the other batch's attention
                (fills the in-order PE stream's exp-wait slots)."""
                loads, qs, ks, vs = [], [], [], []
                for tc8 in range(4):
                    st8 = {}

                    def load(b=b, tc8=tc8, st8=st8):
                        yt = ytp.tile([128, 8, 512], F8, tag="yt")
                        st8["yt"] = yt
                        for et in range(8):
                            # b0 startup is DMA-latency-bound: stripe across
                            # the sync + gpsimd queues for 2x issue bandwidth.
                            # b1 loads run mid-kernel when sync is idle but
                            # gpsimd carries the finalize/A2A path: sync only.
                            eng = nc.gpsimd if (b == 0 and et % 2 == 1) else nc.sync
                            eng.dma_start(
                                yt[:, et, :], yT[128 * et:128 * (et + 1),
                                                 b * S + 512 * tc8: b * S + 512 * (tc8 + 1)])
                    loads.append(load)
                    qs.append([])
                    ks.append([])
                    vs.append([])

                    # Q/K: 4 DoubleRow matmuls (et-pairs), K=1024 contraction.
                    # DR forbids column tile_position offsets, so each mm is
                    # full-width [128, 2, 128] -> out [128, 512].
                    for eg in range(4):
                        def qstep(b=b, tc8=tc8, eg=eg, st8=st8):
                            if eg == 0:
                                st8["ps_q"] = ps_acc.tile([128, 512], F32, tag="acc", name="ps_q")
                            ps_q, yt = st8["ps_q"], st8["yt"]
                            st, sp = (eg == 0), (eg == 3)
                            nc.tensor.matmul(ps_q[:], wqT_sb[:, 2 * eg:2 * eg + 2, :],
                                             yt[:, 2 * eg:2 * eg + 2, :], start=st, stop=sp,
                                             perf_mode=DR)
                            if eg == 3:
                                nc.vector.tensor_scalar(
                                    qT_sb[:, b, 512 * tc8:512 * (tc8 + 1)], ps_q[:],
                                    bq_sb[:], None, ALU.add)
                        qs[tc8].append(qstep)

                    for eg in range(4):
                        def kstep(b=b, tc8=tc8, eg=eg, st8=st8):
                            if eg == 0:
                                st8["ps_k"] = ps_acc.tile([128, 512], F32, tag="acc", name="ps_k")
                            ps_k, yt = st8["ps_k"], st8["yt"]
                            st, sp = (eg == 0), (eg == 3)
                            nc.tensor.matmul(ps_k[:], wkT_sb[:, 2 * eg:2 * eg + 2, :],
                                             yt[:, 2 * eg:2 * eg + 2, :], start=st, stop=sp,
                                             perf_mode=DR)
                            if eg == 3:
                                nc.vector.tensor_scalar(
                                    kT_sb[:, b, 512 * tc8:512 * (tc8 + 1)], ps_k[:],
                                    bk_sb[:], None, ALU.add)
                        ks[tc8].append(kstep)

                    # V: [tok, vdim] layout, fp8 operands (no DoubleRow: the
                    # stationary operand changes every matmul)
                    for eg in range(4):
                        def vstep(b=b, tc8=tc8, eg=eg, st8=st8):
                            if eg == 0:
                                st8["ps_v"] = ps_acc.tile([128, 512], F32, tag="acc", name="ps_v")
                            ps_v, yt = st8["ps_v"], st8["yt"]
                            for et in (2 * eg, 2 * eg + 1):
                                st, sp = (et == 0), (et == 7)
                                for s4 in range(4):
                                    nc.tensor.matmul(ps_v[:, 128 * s4:128 * (s4 + 1)],
                                                     yt[:, et, 128 * s4:128 * (s4 + 1)],
                                                     wvT_sb[:, et, :], start=st, stop=sp)
                            if eg == 3:
                                for h in range(2):
                                    src = ps_v[:, :].rearrange(
                                        "p (s n) -> p s n", s=4)[:, :, 64 * h:64 * (h + 1)]
                                    nc.vector.tensor_copy(
                                        v_sb[:, b, h, 4 * tc8:4 * (tc8 + 1), 0:64], src)
                        vs[tc8].append(vstep)
                return loads, qs, ks, vs

            trigs = []  # collective trigger instrs, k-order

            def attention(b, bg, av_last=None, qcs=range(N_QC), pops=2):
                # software-pipelined ACROSS kt steps: qk/exp runs OV steps
                # ahead of av, so the ACT engine never drains at qc
                # boundaries; bg closures (other QKV work) fill PE wait slots.
                # AV runs per kt-PAIR in fp8 DoubleRow mode.
                if av_last is None:
                    av_last = []
                OV = 4
                states = {}

                def qk_exp(qc, kt):
                    stq = states[qc]
                    if kt % 2 == 0:
                        stq["exs"][kt // 2] = expp.tile([128, 2, 1024], F8, name="ex")
                    sc = ps_sc.tile([128, 1024], F32, tag="sc", name="sc")
                    nc.tensor.matmul(sc[:, 0:512],
                                     kT_sb[0:64, b, 128 * kt:128 * (kt + 1)],
                                     qT_sb[0:64, b, QC * qc:QC * (qc + 1)],
                                     start=True, stop=True, tile_position=(0, 0))
                    nc.tensor.matmul(sc[:, 512:1024],
                                     kT_sb[64:128, b, 128 * kt:128 * (kt + 1)],
                                     qT_sb[64:128, b, QC * qc:QC * (qc + 1)],
                                     start=True, stop=True, tile_position=(64, 0))
                    ex = stq["exs"][kt // 2]
                    nc.scalar.activation(ex[:, kt % 2, :], sc[:], AF.Exp, scale=S_ACT)

                def av_a(qc, kp):
                    stq = states[qc]
                    if kp == 0:
                        stq["av_a"] = ps_av.tile([128, 512], F32, tag="av", name="av_a")
                        stq["av_b"] = ps_av.tile([128, 512], F32, tag="av", name="av_b")
                    ex = stq["exs"][kp]
                    nc.tensor.matmul(stq["av_a"][0:65, :],
                                     v_sb[:, b, 0, 2 * kp:2 * kp + 2, 0:65],
                                     ex[:, :, 0:512],
                                     start=(kp == 0), stop=(kp == N_KP - 1), perf_mode=DR)

                def av_b(qc, kp):
                    stq = states[qc]
                    ex = stq["exs"][kp]
                    i2 = nc.tensor.matmul(stq["av_b"][0:65, :],
                                          v_sb[:, b, 1, 2 * kp:2 * kp + 2, 0:65],
                                          ex[:, :, 512:1024],
                                          start=(kp == 0), stop=(kp == N_KP - 1), perf_mode=DR)
                    if kp == N_KP - 1:
                        av_last.append(i2)

                def finalize(qc):
                    av_a, av_b = states[qc]["av_a"], states[qc]["av_b"]
                    # drain AV psum to SBUF fast (releases psum for next q-chunk)
                    af = finp.tile([128, 1024], F32, tag="af", name="af")
                    nc.vector.tensor_copy(af[0:65, 0:512], av_a[0:65, :])
                    nc.vector.tensor_copy(af[0:65, 512:1024], av_b[0:65, :])
                    # denominator row -> partition 0 (DMA shifts partitions;
                    # reciprocal_approx_fast corrupts on non-zero base
                    # partitions, so the recip must run at partition 0),
                    # then gpsimd broadcast to all lanes
                    rc = finp.tile([128, 1024], F32, tag="rc", name="rc")
                    nc.gpsimd.dma_start(rc[0:1, :], af[64:65, :])
                    rc2 = finp.tile([128, 1024], F32, tag="rc2", name="rc2")
                    nc.vector.reciprocal_approx_fast(rc2[0:1, :], rc[0:1, :])
                    rb = finp.tile([128, 1024], F32, tag="rb", name="rb")
                    nc.gpsimd.partition_broadcast(rb[:, :], rc2[0:1, :])
                    # o^T = o_raw^T * recip + bv; all on partitions 0..63, then
                    # head B is partition-shifted into the stage via DMA.
                    # (fp8 tiles are write-only for the DVE: mult lands in an
                    # f32 scratch, the bias-add writes the fp8 copy once)
                    osc = stage[:, b, QC * qc:QC * (qc + 1)]
                    om = finp.tile([64, 1024], F32, tag="om", name="om")
                    nc.vector.tensor_tensor(om[:, 0:512], af[0:64, 0:512], rb[0:64, 0:512], ALU.mult)
                    nc.vector.tensor_scalar(om[:, 0:512], om[:, 0:512], bva_sb[:], None, ALU.add)
                    nc.vector.tensor_copy(osc[0:64, :], om[:, 0:512])
                    tb = finp.tile([64, 512], F8, tag="tb", name="tb")
                    nc.vector.tensor_tensor(om[:, 512:1024], af[0:64, 512:1024], rb[0:64, 512:1024], ALU.mult)
                    nc.vector.tensor_scalar(om[:, 512:1024], om[:, 512:1024], bvb_sb[:], None, ALU.add)
                    nc.vector.tensor_copy(tb[:], om[:, 512:1024])
                    nc.gpsimd.dma_start(osc[64:128, :], tb[:])
                    # A2A per q-half: upload each qc's blocks as soon as
                    # staged; issue the collective after the odd qc
                    hf = qc // 2
                    k = 2 * b + hf
                    half = a2a_in[k].ap().rearrange("j p n -> p j n")
                    if qc % 2 == 0:
                        nc.gpsimd.dma_start(
                            half[:, 0:4, :],
                            stage[:, b, 1024 * hf:1024 * hf + 512].rearrange(
                                "p (j n) -> p j n", j=4))
                    else:
                        nc.gpsimd.dma_start(
                            half[:, 4:8, :],
                            stage[:, b, 1024 * hf + 512:1024 * (hf + 1)].rearrange(
                                "p (j n) -> p j n", j=4))
                        trigs.append(nc.gpsimd.collective_compute(
                            "AllToAll", ALU.bypass,
                            replica_groups=[list(range(N_CORES))],
                            ins=[a2a_in[k].ap().opt()],
                            outs=[a2a_out[k].ap().opt()],
                        ))

                seq = [(qc, kt) for qc in qcs for kt in range(N_KT)]
                for i, (qc, kt) in enumerate(seq):
                    states.setdefault(qc, {"exs": [None] * N_KP})
                    qk_exp(qc, kt)
                    for _ in range(pops):
                        if bg:
                            bg.pop(0)()
                    j = i - OV
                    if j >= 0 and seq[j][1] % 2 == 1:
                        jqc, jkt = seq[j]
                        av_a(jqc, jkt // 2)
                        av_b(jqc, jkt // 2)
                        if jkt == N_KT - 1:
                            finalize(jqc)
                for j in range(max(0, len(seq) - OV), len(seq)):
                    if seq[j][1] % 2 == 1:
                        jqc, jkt = seq[j]
                        av_a(jqc, jkt // 2)
                        av_b(jqc, jkt // 2)
                        if jkt == N_KT - 1:
                            finalize(jqc)
                return av_last

            # drive: emit only chunk 0 of b0's QKV up front, then start
            # attention qc0 with chunks 1-3 interleaved as background steps
            # (order [K,V] per chunk matches the kt windows that consume them);
            # b1's QKV interleaves into b0's qc1-3.
            l0, q0, k0, v0 = make_qkv_steps(0)
            for step in l0:
                step()
            for s in k0[0]:
                s()
            for s in q0[0]:
                s()
            for s in v0[0]:
                s()
            l1, q1, k1, v1 = make_qkv_steps(1)
            # bgA feeds b0 qc0: kT chunk c needed by step 4c, v chunk c by the
            # av of its kt-pairs, q0[1] before qc1 starts; 28 closures, 32 slots
            bgA = []
            for tc8 in (1, 2, 3):
                bgA.extend(k0[tc8])
                bgA.extend(v0[tc8])
            bgA.extend(q0[1])
            attention(0, bgA, qcs=[0])
            while bgA:
                bgA.pop(0)()
            # qc1: rest of b0's Q + b1's loads and kT (25 closures, 32 slots)
            bgB = list(q0[2]) + list(q0[3]) + list(l1)
            for tc8 in range(4):
                bgB.extend(k1[tc8])
            attention(0, bgB, qcs=[1])
            while bgB:
                bgB.pop(0)()
            # qc2+qc3: b1's Q and V (32 closures, 32 slots)
            bgC = []
            for tc8 in range(4):
                bgC.extend(q1[tc8])
                bgC.extend(v1[tc8])
            attention(0, bgC, qcs=[2, 3], pops=1)
            while bgC:
                bgC.pop(0)()
            av_anchors = attention(1, [])

            # deferred bulk const loads (issued after attention DMAs in queue order)
            nc.sync.dma_start(msa_w_sb[:], msa_w[:, :].rearrange("(a p) n -> p a n", p=128))
            nc.sync.dma_start(y_shard_sb[:], y_shard[:, :].rearrange("(a p) n -> p a n", p=128))
            if use_gamma:
                nc.sync.dma_start(gamma_sb[:], gamma_b[:, :])
            if use_beta:
                nc.sync.dma_start(beta_sb[:], beta_b[:, :])

            # ============== msa + residual + LayerNorm per shard ==============
            res_sb = stp.tile([128, 4, E], F32)
            for k in range(4):
                lhs = a2ap.tile([128, 8, 128], F8, tag="lhs")
                nc.sync.dma_start(lhs[:], a2a_out[k].ap().rearrange("j p n -> p j n"))
                # i-major so consecutive matmuls share lhs weights (LDW dedup);
                # both e-halves accumulate concurrently in two psum tiles.
                # fp8 DoubleRow: i-pairs, contraction 1024 in 4 passes.
                ps_m0 = ps_acc.tile([128, 512], F32, tag="acc", name="ps_m0")
                ps_m1 = ps_acc.tile([128, 512], F32, tag="acc", name="ps_m1")
                for i in range(4):
                    for ec, ps_m in ((0, ps_m0), (1, ps_m1)):
                        mi = nc.tensor.matmul(ps_m[:], lhs[:, 2 * i:2 * i + 2, :],
                                              msa_w_sb[:, 2 * i:2 * i + 2,
                                                       512 * ec:512 * (ec + 1)],
                                              start=(i == 0), stop=(i == 3),
                                              perf_mode=DR)
                        if ec == 0 and i == 0:
                            # keep msa out of the PE stream until b1 attention
                            # has progressed past qc k+1 (the A2A data won't be
                            # there earlier; an early msa blocks the in-order PE)
                            _add_dep_helper(
                                mi.ins, av_anchors[min(k + 1, 3)].ins, sync=False,
                                reason="msa gated behind b1 attention progress")
                for ec, ps_m in ((0, ps_m0), (1, ps_m1)):
                    # residual add, on DVE (no ACT table switch)
                    rhalf = res_sb[:, k, 512 * ec:512 * (ec + 1)]
                    ri = nc.vector.tensor_tensor(
                        rhalf, ps_m[:],
                        y_shard_sb[:, k, 512 * ec:512 * (ec + 1)], ALU.add)
                    if ec == 0:
                        # keep this shard's LN work behind the (k+1)-th
                        # collective TRIGGER on the DVE queue: the trigger path
                        # of the last q-chunk must not queue behind LN ops.
                        # NOTE: sync=False on purpose -- a sync=True semaphore
                        # edge here deadlocks the device (cross-engine cycle)
                        _add_dep_helper(
                            ri.ins, trigs[min(k + 1, 3)].ins, sync=False,
                            reason="LN deprioritized behind collective trigger")
                # fused mean/var via bn_stats halves + one aggregate
                stats = finp.tile([128, 2, 6], F32, tag="stats")
                nc.vector.bn_stats(stats[:, 0, :], res_sb[:, k, 0:512])
                nc.vector.bn_stats(stats[:, 1, :], res_sb[:, k, 512:1024])
                mu = cpool.tile([128, 4], F32, name=f"mu{k}")
                nc.vector.bn_aggr(mu[:, 0:2], stats[:])
                # rstd = sqrt(1/(var+eps)); 51-ULP reciprocal is plenty here
                nc.vector.tensor_scalar(mu[:, 1:2], mu[:, 1:2], EPS, None, ALU.add)
                nc.vector.reciprocal_approx_fast(mu[:, 2:3], mu[:, 1:2])
                nc.scalar.activation(mu[:, 3:4], mu[:, 2:3], AF.Sqrt)
                nc.vector.tensor_scalar(mu[:, 0:1], mu[:, 0:1], -1.0, None, ALU.mult)
                # o1 = (res - mu) * rstd, fused on DVE
                o1 = finp.tile([128, E], F32, tag="o1")
                nc.vector.tensor_scalar(o1[:], res_sb[:, k, :], mu[:, 0:1],
                                        mu[:, 3:4], ALU.add, ALU.mult)
                if use_gamma:
                    nc.vector.tensor_tensor(o1[:], o1[:], gamma_sb[:], ALU.mult)
                if use_beta:
                    nc.vector.tensor_tensor(o1[:], o1[:], beta_sb[:], ALU.add)
                nc.sync.dma_start(out[128 * k:128 * (k + 1), :], o1[:])

    nc.compile()
    return nc


@functools.lru_cache(maxsize=4)
def _get_nc(use_gamma: bool, use_beta: bool):
    return _build(use_gamma, use_beta)


def kernel(**inputs) -> np.ndarray:
    y = np.asarray(inputs["y"], np.float32)
    Wqkv = np.asarray(inputs["Wqkv"], np.float32)
    bqkv = np.asarray(inputs["bqkv"], np.float32)
    Wmsa = np.asarray(inputs["Wmsa"], np.float32)
    Bq_, Aq_ = np.asarray(inputs["Bq"], np.float32), np.asarray(inputs["Aq"], np.float32)
    Bk_, Ak_ = np.asarray(inputs["Bk"], np.float32), np.asarray(inputs["Ak"], np.float32)
    Bv_, Av_ = np.asarray(inputs["Bv"], np.float32), np.asarray(inputs["Av"], np.float32)
    Bo_, Ao_ = np.asarray(inputs["Bo"], np.float32), np.asarray(inputs["Ao"], np.float32)
    gamma = np.asarray(inputs["gamma"], np.float32)
    beta = np.asarray(inputs["beta"], np.float32)

    # effective weights: qkv = y @ (Wqkv.T + blockdiag-ish LoRA) + bqkv
    # y @ W.T: W rows are output dims. LoRA adds y @ (B@A): effective W += (B@A).T
    W_eff = Wqkv.copy()
    W_eff[0:E] += (Bq_ @ Aq_).T
    W_eff[E:2 * E] += (Bk_ @ Ak_).T
    W_eff[2 * E:3 * E] += (Bv_ @ Av_).T
    # fp8 pre-scale: weights x32 (1/sqrt(D) and the scale unwind live in
    # the exp affine scale and the /32 on the msa weights)
    W_eff *= WSC
    bq_eff = bqkv[0:E] * WSC
    bk_eff = bqkv[E:2 * E] * WSC
    bv_eff = bqkv[2 * E:3 * E] * WSC
    # msa: o @ Wmsa.T + o @ (Bo@Ao) = o @ M with M = Wmsa.T + Bo@Ao  [E(d), E(out)].
    # stage carries 32*(o+bv); store 32*M in fp8 (good dynamic range) so the
    # msa psum is 1024*msa, and scale the residual y by 1024 to match --
    # LayerNorm is scale-invariant, so the final output is unchanged.
    M = (Wmsa.T + Bo_ @ Ao_) * WSC

    y_flat = y.reshape(T, E)
    yT_f8 = np.ascontiguousarray(y_flat.T).astype(NP_F8)
    M_f8 = np.ascontiguousarray(M).astype(NP_F8)

    use_gamma = not np.allclose(gamma, 1.0)
    use_beta = not np.allclose(beta, 0.0)
    nc = _get_nc(use_gamma, use_beta)

    in_maps = []
    for c in range(N_CORES):
        r0 = c * 128
        r1 = r0 + 128
        wq_c = np.ascontiguousarray(W_eff[0:E][r0:r1].T).astype(NP_F8)
        wk_c = np.ascontiguousarray(W_eff[E:2 * E][r0:r1].T).astype(NP_F8)
        wv_c = np.ascontiguousarray(W_eff[2 * E:3 * E][r0:r1].T).astype(NP_F8)
        tok = np.concatenate([
            np.arange(128 * c, 128 * c + 128),
            np.arange(1024 + 128 * c, 1024 + 128 * c + 128),
            np.arange(2048 + 128 * c, 2048 + 128 * c + 128),
            np.arange(3072 + 128 * c, 3072 + 128 * c + 128),
        ])
        m = {
            "yT": yT_f8,
            "wqT": wq_c,
            "wkT": wk_c,
            "wvT": wv_c,
            "bq": bq_eff[r0:r1].reshape(128, 1).copy(),
            "bk": bk_eff[r0:r1].reshape(128, 1).copy(),
            "bva": bv_eff[r0:r0 + 64].reshape(64, 1).copy(),
            "bvb": bv_eff[r0 + 64:r1].reshape(64, 1).copy(),
            "msa_w": M_f8,
            "y_shard": np.ascontiguousarray(y_flat[tok]) * (WSC * WSC),
        }
        if use_gamma:
            m["gamma_b"] = np.broadcast_to(gamma, (128, E)).copy()
        if use_beta:
            m["beta_b"] = np.broadcast_to(beta, (128, E)).copy()
        in_maps.append(m)

    res = bass_utils.run_bass_kernel_spmd(nc, in_maps, core_ids=list(range(N_CORES)))

    out_full = np.empty((T, E), np.float32)
    for c in range(N_CORES):
        oc = res.results[c]["out"]
        out_full[128 * c:128 * c + 128] = oc[0:128]
        out_full[1024 + 128 * c:1024 + 128 * c + 128] = oc[128:256]
        out_full[2048 + 128 * c:2048 + 128 * c + 128] = oc[256:384]
        out_full[3072 + 128 * c:3072 + 128 * c + 128] = oc[384:512]
    return out_full.reshape(B, S, E)
